# revision 9
# baseline (speedup 1.0000x reference)
"""AttnDecoderRNN step on 8 Trainium2 NeuronCores (Bass/Tile, SPMD).

Sharding strategy (tensor-parallel over output dims, vocab-sharded big matvec):
  - Embedding lookup is pure data movement: done host-side (one row of emb).
  - GRU gates: W_ih/W_hh row-sharded (each core owns a 128-slice of H for all
    three gates) -> each core computes h_new for its slice. No comm.
  - Attention: attn_W column-sharded against the local h_new slice, fused with
    encoder_outputs @ q so a single AllReduce combines scores [400], the
    ws.h_new dot partial, and re-assembles full h_new (mask trick).
  - pre-activation of the combine FF column-sharded -> AllReduce #2.
  - out projection [V,H] row-sharded 6250 rows/core (padded 6272), weights
    pre-transposed + bf16 on host; PE matvec with v on partitions so the
    softmax reduction is partition-parallel.
  - softmax over V: local sum of exp, AllGather of 8 scalars, log-sum-exp
    correction applied locally. (No max subtraction needed: logits are O(1)
    for this model scale; exp is safely inside fp32 range.)
Outputs: each core writes its vocab shard; core 0's h_new / attn_weights /
atten_p are used. Host gathers + undoes the column-major layout.
"""
import sys

sys.path.insert(0, "/opt/trn_rl_repo")

import numpy as np
import ml_dtypes

import concourse.bass as bass
import concourse.mybir as mybir
import concourse.tile as tile
from concourse.vector_clock import ScopedClock
from concourse import bass_utils
from concourse.bass_utils import run_bass_kernel_spmd

# ---------------------------------------------------------------- patches ---
# This walrus build rejects >1 sync wait on a TPB_CTRL (Drain) instruction;
# TileContext's tail drain accumulates every outstanding sem wait onto it.
# Split the waits onto single-wait nops emitted just before the drain.


def _patched_drain_and_barrier(self, tick_clock, wait_clock):
    nc = self.nc
    carrier = nc.sync.nop(nofuse=True)
    wait_clock.add_sem_waits(carrier.ins, ScopedClock({None: tick_clock.global_clock}))
    si = carrier.ins.sync_info
    waits = list(si.on_wait) if si and si.on_wait else []
    if len(waits) > 1:
        carrier.ins.sync_info = mybir.SyncInfo(
            on_wait=[waits[0]], on_update=si.on_update
        )
        for w in waits[1:]:
            extra = nc.sync.nop(nofuse=True)
            esi = extra.ins.sync_info
            extra.ins.sync_info = mybir.SyncInfo(
                on_wait=[w], on_update=esi.on_update if esi else []
            )
    nc.sync.drain()
    nc.all_engine_barrier()
    popped = nc._tile_sem_poison_stack.pop()
    assert popped is self._sem_poison
    nc.clear_and_free_semaphores(list(self.sems.allocated().values()))
    nc.all_engine_barrier()


tile.TileContext._drain_and_barrier = _patched_drain_and_barrier

# Artifact upload needs a fish bucket; not available (and not needed) here.
bass_utils.upload_artifacts = lambda tmpdir: tmpdir


# This container's antenv lacks axon_hooks; provide the NTFF profile hook via
# ctypes into libaxon_pjrt.so (same shim trn_agent_boot would install).
def _install_ntff_hook_shim():
    import types
    import contextlib
    import ctypes

    if "antenv.axon_hooks" in sys.modules:
        return
    hook = None
    try:
        lib = ctypes.CDLL("/opt/axon/libaxon_pjrt.so")
        if hasattr(lib, "axon_start_nrt_profile"):
            lib.axon_start_nrt_profile.argtypes = [
                ctypes.POINTER(ctypes.c_int64),
                ctypes.c_size_t,
            ]
            lib.axon_start_nrt_profile.restype = ctypes.c_int64
            lib.axon_stop_nrt_profile.argtypes = [ctypes.c_char_p]
            lib.axon_stop_nrt_profile.restype = ctypes.c_int64

            @contextlib.contextmanager
            def _hook(output_dir, device_ids):
                import jax

                jax.devices()
                if device_ids:
                    ids = (ctypes.c_int64 * len(device_ids))(*device_ids)
                    rc = lib.axon_start_nrt_profile(ids, len(device_ids))
                else:
                    rc = lib.axon_start_nrt_profile(None, 0)
                if rc != 0:
                    raise RuntimeError(f"axon_start_nrt_profile rc={rc}")
                try:
                    yield
                finally:
                    n = lib.axon_stop_nrt_profile(str(output_dir).encode())
                    print(f"ntff profile: {n} file(s) -> {output_dir}",
                          file=sys.stderr)

            hook = _hook
    except OSError:
        pass
    mod = types.ModuleType("antenv.axon_hooks")
    mod.get_axon_ntff_profile_hook = lambda: hook
    mod.set_axon_ntff_profile_hook = lambda h: None
    sys.modules["antenv.axon_hooks"] = mod
    import antenv

    antenv.axon_hooks = mod


_install_ntff_hook_shim()

# ------------------------------------------------------------- constants ---
NC_N = 8
H = 1024
V = 50000
E = 602
S = 400
P = 128
VR = V // NC_N        # 6250 real vocab rows per core
MT = 49               # vocab m-tiles per core
VP = MT * P           # 6272 padded vocab rows per core
SP = 512              # padded S
ET_M = 4              # s-tiles (512/128)
EP_COLS = 640         # padded E (5*128)
APT = 5               # atten m-tiles

F32 = mybir.dt.float32
BF16 = mybir.dt.bfloat16
NPBF = ml_dtypes.bfloat16

LAST_RESULT = None    # BassKernelResults of the most recent run (for test.py)
TRACE = False         # set True (e.g. by test.py) to profile
DEBUG = False         # add per-stage debug outputs

_NC_CACHE = None



def _split_multi_waits(nc):
    """This walrus build accepts a single sync wait per instruction; hoist
    extra waits onto same-engine nops placed just before the instruction."""
    for f in nc.m.functions:
        for bb in f.blocks:
            out = []
            for ins in bb.instructions:
                si = ins.sync_info
                waits = list(si.on_wait) if si and si.on_wait else []
                if len(waits) > 1:
                    for w in waits[:-1]:
                        nop = mybir.InstNoOp(
                            name=nc.get_next_instruction_name(),
                            engine=ins.engine,
                            ins=[],
                            outs=[],
                            sync_info=mybir.SyncInfo(on_wait=[w], on_update=[]),
                        )
                        out.append(nop)
                    ins.sync_info = mybir.SyncInfo(
                        on_wait=[waits[-1]], on_update=si.on_update
                    )
                out.append(ins)
            bb.instructions = out


# ------------------------------------------------------------ device code ---
def _build_nc():
    nc = bass.Bass()

    def di(name, shape, dt=BF16):
        return nc.dram_tensor(name, shape, dt, kind="ExternalInput")

    # per-core weights / data (values differ per core, names shared)
    wo_t = di("wo_t", [H, VP])
    out_b = di("out_b", [P, MT], F32)
    wih_t = di("wih_t", [2 * H, 3 * P])
    whh_t = di("whh_t", [H, 3 * P])
    b01 = di("b01", [P, 2], F32)
    b_ihn = di("b_ihn", [P, 1], F32)
    b_hhn = di("b_hhn", [P, 1], F32)
    attn_t = di("attn_t", [P, H])
    e_t = di("e_t", [H, SP])
    e_pad = di("e_pad", [SP, H])
    combh_t = di("combh_t", [P, H])
    comba_t = di("comba_t", [P, H])
    comb_b = di("comb_b", [P, 8], F32)
    pg_pad = di("pg_pad", [SP, EP_COLS])
    wh_w = di("wh_w", [P, 8])
    ws_w = di("ws_w", [P, 1])
    wx_w = di("wx_w", [P, 8])
    x_vec = di("x_vec", [P, 16])
    h_vec = di("h_vec", [P, 8])
    emb_vec = di("emb_vec", [P, 8])
    h_col = di("h_col", [P, 1], F32)
    colmask = di("colmask", [P, 8], F32)
    smask = di("smask", [P, 4], F32)
    consts = di("consts", [1, 8], F32)
    ident = di("ident", [P, P], F32)

    vocab_out = nc.dram_tensor("vocab_out", [P, MT], F32, kind="ExternalOutput")
    hnew_out = nc.dram_tensor("hnew_out", [P, 8], F32, kind="ExternalOutput")
    attnw_out = nc.dram_tensor("attnw_out", [P, 4], F32, kind="ExternalOutput")
    atten_out = nc.dram_tensor("atten_out", [P, APT], F32, kind="ExternalOutput")
    if DEBUG:
        dbg = {
            name: nc.dram_tensor(name, shape, F32, kind="ExternalOutput")
            for name, shape in [
                ("dbg_gi", [P, 3]), ("dbg_gh", [P, 3]), ("dbg_hnewl", [P, 1]),
                ("dbg_qp", [P, 8]), ("dbg_scl", [P, 4]), ("dbg_ar1", [P, 16]),
                ("dbg_aa", [P, 8]), ("dbg_ff", [P, 8]), ("dbg_pgen", [1, 1]),
            ]
        }

    with tile.TileContext(nc) as tc:
        with (
            tc.tile_pool(name="wp", bufs=1) as wp,
            tc.tile_pool(name="sp", bufs=1) as spool,
            tc.tile_pool(name="pp", bufs=1, space="PSUM") as pp,
            tc.tile_pool(name="bigp", bufs=1, space="PSUM") as bigp,
            tc.tile_pool(name="dram", bufs=1, space="DRAM") as dp,
        ):
            # ---- SBUF loads (DMA; scheduler overlaps with compute) ----
            def load(name, dram, shape3, rearr=None):
                t = wp.tile(list(shape3), dram.dtype, tag=name, name=name)
                src = dram[:] if rearr is None else dram[:].rearrange(rearr, p=P)
                nc.sync.dma_start(out=t[:], in_=src)
                return t

            # small weights first so they don't queue behind the 12.8MB wo_t
            wih_sb = load("wih", wih_t, [P, 16, 3 * P], "(k p) m -> p k m")
            whh_sb = load("whh", whh_t, [P, 8, 3 * P], "(k p) m -> p k m")
            attn_sb = load("attn", attn_t, [P, H])
            et_sb = load("et", e_t, [P, 8, SP], "(k p) m -> p k m")
            ep_sb = load("ep", e_pad, [P, ET_M, H], "(k p) m -> p k m")
            ch_sb = load("ch", combh_t, [P, H])
            ca_sb = load("ca", comba_t, [P, H])
            pg_sb = load("pg", pg_pad, [P, ET_M, EP_COLS], "(k p) m -> p k m")
            wh_sb = load("wh", wh_w, [P, 8])
            ws_sb = load("ws", ws_w, [P, 1])
            wx_sb = load("wx", wx_w, [P, 8])
            x_sb = load("x", x_vec, [P, 16])
            h_sb = load("h", h_vec, [P, 8])
            emb_sb = load("emb", emb_vec, [P, 8])
            hcol_sb = load("hcol", h_col, [P, 1])
            cmask_sb = load("cmask", colmask, [P, 8])
            smask_sb = load("smask", smask, [P, 4])
            consts_sb = load("consts", consts, [1, 8])
            id_sb = load("ident", ident, [P, P])
            outb_sb = load("outb", out_b, [P, MT])
            wo_sb = load("wo", wo_t, [P, 8, VP], "(k p) m -> p k m")

            onesc = spool.tile([P, 1], F32, tag="onesc")
            nc.vector.memset(onesc[:], 1.0)
            onesr = spool.tile([1, P], F32, tag="onesr")
            nc.vector.memset(onesr[:], 1.0)

            def psum(shape, tag="tiny", bufs=2, name="ps"):
                return pp.tile(list(shape), F32, tag=tag, bufs=bufs, name=name)

            def part_sum(vec_sb, k=P):
                """sum over partitions of [k,1] f32 -> [1,1] psum"""
                out = psum([1, 1])
                nc.tensor.matmul(out[:], onesc[:k, :], vec_sb, start=True, stop=True)
                return out

            def bcast(scalar_sb):
                """[1,1] sbuf f32 -> [128,1] sbuf f32"""
                pb = psum([P, 1])
                nc.tensor.matmul(pb[:], onesr[:], scalar_sb, start=True, stop=True)
                sb = spool.tile([P, 1], F32, tag="bc", name="bc")
                nc.vector.tensor_copy(sb[:], pb[:])
                return sb

            def to_sb(ps, shape, dt=F32, tag="cp"):
                sb = spool.tile(list(shape), dt, tag=tag, name=tag)
                nc.vector.tensor_copy(sb[:], ps)
                return sb

            # ---- stage 1: GRU slice (no comm) ----
            gi = psum([P, 3], tag="gates", name="gi")
            for k in range(16):
                for m in range(3):
                    nc.tensor.matmul(
                        gi[:, m : m + 1],
                        wih_sb[:, k, m * P : (m + 1) * P],
                        x_sb[:, k : k + 1],
                        start=(k == 0 and m == 0),
                        stop=(k == 15 and m == 2),
                    )
            gh = psum([P, 3], tag="gates", name="gh")
            for k in range(8):
                for m in range(3):
                    nc.tensor.matmul(
                        gh[:, m : m + 1],
                        whh_sb[:, k, m * P : (m + 1) * P],
                        h_sb[:, k : k + 1],
                        start=(k == 0 and m == 0),
                        stop=(k == 7 and m == 2),
                    )
            gi_sb = to_sb(gi[:], [P, 3], F32, tag="gisb")
            t01 = spool.tile([P, 2], F32, tag="t01")
            nc.vector.tensor_add(t01[:], gi_sb[:, 0:2], gh[:, 0:2])
            b01_sb = spool.tile([P, 2], F32, tag="b01")
            nc.sync.dma_start(out=b01_sb[:], in_=b01[:])
            nc.vector.tensor_add(t01[:], t01[:], b01_sb[:])
            rz = spool.tile([P, 2], F32, tag="rz")
            nc.scalar.activation(rz[:], t01[:], mybir.ActivationFunctionType.Sigmoid)

            bihn_sb = spool.tile([P, 1], F32, tag="bihn")
            nc.sync.dma_start(out=bihn_sb[:], in_=b_ihn[:])
            bhhn_sb = spool.tile([P, 1], F32, tag="bhhn")
            nc.sync.dma_start(out=bhhn_sb[:], in_=b_hhn[:])
            ghn = spool.tile([P, 1], F32, tag="ghn")
            nc.vector.tensor_add(ghn[:], gh[:, 2:3], bhhn_sb[:])
            tmp1 = spool.tile([P, 1], F32, tag="tmp1")
            nc.vector.tensor_mul(tmp1[:], rz[:, 0:1], ghn[:])
            npre = spool.tile([P, 1], F32, tag="npre")
            nc.vector.tensor_add(npre[:], gi_sb[:, 2:3], bihn_sb[:])
            nc.vector.tensor_add(npre[:], npre[:], tmp1[:])
            n_sb = spool.tile([P, 1], F32, tag="n")
            nc.scalar.activation(n_sb[:], npre[:], mybir.ActivationFunctionType.Tanh)
            # h_new = n + z*(h - n)
            d_sb = spool.tile([P, 1], F32, tag="d")
            nc.vector.tensor_sub(d_sb[:], hcol_sb[:], n_sb[:])
            zt = spool.tile([P, 1], F32, tag="zt")
            nc.vector.tensor_mul(zt[:], rz[:, 1:2], d_sb[:])
            hnew = spool.tile([P, 1], F32, tag="hnew")
            nc.vector.tensor_add(hnew[:], n_sb[:], zt[:])
            hnew_bf = to_sb(hnew[:], [P, 1], BF16, tag="hnewbf")
            if DEBUG:
                nc.sync.dma_start(out=dbg["dbg_hnewl"][:], in_=hnew[:])
                nc.sync.dma_start(out=dbg["dbg_gi"][:], in_=gi_sb[:])
                gh_dbg = to_sb(gh[:], [P, 3], F32, tag="ghdbg")
                nc.sync.dma_start(out=dbg["dbg_gh"][:], in_=gh_dbg[:])

            # ---- stage 2: partial attention scores ----
            qp = psum([P, 8], tag="vec8", name="qp")
            for m in range(8):
                nc.tensor.matmul(
                    qp[:, m : m + 1],
                    attn_sb[:, m * P : (m + 1) * P],
                    hnew_bf[:],
                    start=(m == 0),
                    stop=(m == 7),
                )
            qp_bf = to_sb(qp[:], [P, 8], BF16, tag="qpbf")
            sc = psum([P, 4], tag="vec8", name="sc")
            for k in range(8):
                for m in range(4):
                    nc.tensor.matmul(
                        sc[:, m : m + 1],
                        et_sb[:, k, m * P : (m + 1) * P],
                        qp_bf[:, k : k + 1],
                        start=(k == 0 and m == 0),
                        stop=(k == 7 and m == 3),
                    )
            if DEBUG:
                qp_dbg = to_sb(qp[:], [P, 8], F32, tag="qpdbg")
                nc.sync.dma_start(out=dbg["dbg_qp"][:], in_=qp_dbg[:])
                sc_dbg = to_sb(sc[:], [P, 4], F32, tag="scdbg")
                nc.sync.dma_start(out=dbg["dbg_scl"][:], in_=sc_dbg[:])
            wsp = psum([1, 1], name="wsp")
            nc.tensor.matmul(wsp[:], ws_sb[:], hnew_bf[:], start=True, stop=True)

            # ---- AllReduce #1: scores + ws_partial + h_new assembly ----
            ar1 = spool.tile([P, 16], F32, tag="ar1")
            nc.vector.memset(ar1[:], 0.0)
            nc.vector.tensor_copy(ar1[:, 0:4], sc[:])
            nc.vector.tensor_copy(ar1[0:1, 4:5], wsp[:])
            nc.vector.tensor_scalar_mul(ar1[:, 5:13], cmask_sb[:], hnew[:])
            ar1_in = dp.tile([P, 16], F32)
            ar1_out = dp.tile([P, 16], F32)
            nc.sync.dma_start(out=ar1_in[:], in_=ar1[:])
            nc.gpsimd.collective_compute(
                "AllReduce",
                mybir.AluOpType.add,
                replica_groups=[list(range(NC_N))],
                ins=[ar1_in.opt()],
                outs=[ar1_out.opt()],
            )
            ag1 = spool.tile([P, 16], F32, tag="ag1")
            nc.sync.dma_start(out=ag1[:], in_=ar1_out[:])
            nc.sync.dma_start(out=hnew_out[:], in_=ag1[:, 5:13])
            if DEBUG:
                nc.sync.dma_start(out=dbg["dbg_ar1"][:], in_=ag1[:])

            # ---- stage 3: softmax(scores), attn_applied, p_gen (replicated) ----
            scores = spool.tile([P, 4], F32, tag="scores")
            nc.vector.tensor_add(scores[:], ag1[:, 0:4], smask_sb[:])
            cmax = spool.tile([P, 1], F32, tag="cmax")
            nc.vector.reduce_max(out=cmax[:], in_=scores[:], axis=mybir.AxisListType.X)
            tp = psum([1, P], name="tp")
            nc.tensor.transpose(tp[:], cmax[:], id_sb[:])
            smax = spool.tile([1, 1], F32, tag="smax")
            nc.vector.reduce_max(out=smax[:], in_=tp[:], axis=mybir.AxisListType.X)
            smax_b = bcast(smax[:])
            shift = spool.tile([P, 4], F32, tag="shift")
            nc.vector.tensor_scalar_sub(shift[:], scores[:], smax_b[:])
            expsc = spool.tile([P, 4], F32, tag="expsc")
            rsum = spool.tile([P, 1], F32, tag="rsum")
            nc.scalar.activation(
                expsc[:], shift[:], mybir.ActivationFunctionType.Exp,
                accum_out=rsum[:],
            )
            stot = to_sb(part_sum(rsum[:])[:], [1, 1], tag="stot")
            rinv = spool.tile([1, 1], F32, tag="rinv")
            nc.vector.reciprocal(rinv[:], stot[:])
            rinv_b = bcast(rinv[:])
            aw = spool.tile([P, 4], F32, tag="aw")
            nc.vector.tensor_scalar_mul(aw[:], expsc[:], rinv_b[:])
            nc.sync.dma_start(out=attnw_out[:], in_=aw[:])
            aw_bf = to_sb(aw[:], [P, 4], BF16, tag="awbf")

            # attn_applied = attn_weights @ E  (full, replicated)
            aa = psum([P, 8], tag="vec8", name="aa")
            for k in range(ET_M):
                for m in range(8):
                    nc.tensor.matmul(
                        aa[:, m : m + 1],
                        ep_sb[:, k, m * P : (m + 1) * P],
                        aw_bf[:, k : k + 1],
                        start=(k == 0 and m == 0),
                        stop=(k == ET_M - 1 and m == 7),
                    )
            aa_f = to_sb(aa[:], [P, 8], F32, tag="aaf")
            aa_bf = to_sb(aa[:], [P, 8], BF16, tag="aabf")
            # select this core's h-slice of attn_applied
            t8 = spool.tile([P, 8], F32, tag="t8")
            nc.vector.tensor_mul(t8[:], aa_f[:], cmask_sb[:])
            aac = spool.tile([P, 1], F32, tag="aac")
            nc.vector.reduce_sum(out=aac[:], in_=t8[:], axis=mybir.AxisListType.X)
            aac_bf = to_sb(aac[:], [P, 1], BF16, tag="aacbf")
            if DEBUG:
                nc.sync.dma_start(out=dbg["dbg_aa"][:], in_=aa_f[:])

            # p_gen
            pgp = psum([1, 1], name="pgp")
            for k in range(8):
                nc.tensor.matmul(
                    pgp[:], wh_sb[:, k : k + 1], aa_bf[:, k : k + 1],
                    start=(k == 0), stop=False,
                )
            for k in range(8):
                nc.tensor.matmul(
                    pgp[:], wx_sb[:, k : k + 1], emb_sb[:, k : k + 1],
                    start=False, stop=(k == 7),
                )
            p1 = spool.tile([1, 1], F32, tag="p1")
            nc.vector.tensor_add(p1[:], pgp[:], ag1[0:1, 4:5])
            pgen = spool.tile([1, 1], F32, tag="pgen")
            nc.scalar.activation(
                pgen[:], p1[:], mybir.ActivationFunctionType.Sigmoid,
                bias=consts_sb[0:1, 1:2],
            )
            ln_pg = spool.tile([1, 1], F32, tag="lnpg")
            nc.scalar.activation(ln_pg[:], pgen[:], mybir.ActivationFunctionType.Ln)
            om = spool.tile([1, 1], F32, tag="om")
            nc.vector.tensor_sub(om[:], consts_sb[0:1, 0:1], pgen[:])
            ln_om = spool.tile([1, 1], F32, tag="lnom")
            nc.scalar.activation(ln_om[:], om[:], mybir.ActivationFunctionType.Ln)

            # atten_p = log(attn_weights @ pg_mat) + log(1-p_gen)
            app = psum([P, APT], tag="vec8", name="app")
            for k in range(ET_M):
                for m in range(APT):
                    nc.tensor.matmul(
                        app[:, m : m + 1],
                        pg_sb[:, k, m * P : (m + 1) * P],
                        aw_bf[:, k : k + 1],
                        start=(k == 0 and m == 0),
                        stop=(k == ET_M - 1 and m == APT - 1),
                    )
            ln_ap = spool.tile([P, APT], F32, tag="lnap")
            nc.scalar.activation(ln_ap[:], app[:], mybir.ActivationFunctionType.Ln)
            lnom_b = bcast(ln_om[:])
            apf = spool.tile([P, APT], F32, tag="apf")
            nc.vector.tensor_scalar_add(apf[:], ln_ap[:], lnom_b[:])
            nc.sync.dma_start(out=atten_out[:], in_=apf[:])

            # ---- pre-ff partial + AllReduce #2 ----
            pf = psum([P, 8], tag="vec8", name="pf")
            for m in range(8):
                nc.tensor.matmul(
                    pf[:, m : m + 1],
                    ch_sb[:, m * P : (m + 1) * P],
                    hnew_bf[:],
                    start=(m == 0),
                    stop=False,
                )
                nc.tensor.matmul(
                    pf[:, m : m + 1],
                    ca_sb[:, m * P : (m + 1) * P],
                    aac_bf[:],
                    start=False,
                    stop=(m == 7),
                )
            pf_sb = to_sb(pf[:], [P, 8], F32, tag="pfsb")
            ar2_in = dp.tile([P, 8], F32)
            ar2_out = dp.tile([P, 8], F32)
            nc.sync.dma_start(out=ar2_in[:], in_=pf_sb[:])
            nc.gpsimd.collective_compute(
                "AllReduce",
                mybir.AluOpType.add,
                replica_groups=[list(range(NC_N))],
                ins=[ar2_in.opt()],
                outs=[ar2_out.opt()],
            )
            pff = spool.tile([P, 8], F32, tag="pff")
            nc.sync.dma_start(out=pff[:], in_=ar2_out[:])
            cb_sb = spool.tile([P, 8], F32, tag="cb")
            nc.sync.dma_start(out=cb_sb[:], in_=comb_b[:])
            nc.vector.tensor_add(pff[:], pff[:], cb_sb[:])
            ff = spool.tile([P, 8], F32, tag="ff")
            nc.scalar.activation(ff[:], pff[:], mybir.ActivationFunctionType.Relu)
            ff_bf = to_sb(ff[:], [P, 8], BF16, tag="ffbf")
            if DEBUG:
                nc.sync.dma_start(out=dbg["dbg_ff"][:], in_=ff[:])
                nc.sync.dma_start(out=dbg["dbg_pgen"][:], in_=pgen[:])

            # ---- big matvec: logits shard [128, 49] ----
            big = bigp.tile([P, MT], F32, tag="big")
            for k in range(8):
                for j in range(MT):
                    nc.tensor.matmul(
                        big[:, j : j + 1],
                        wo_sb[:, k, j * P : (j + 1) * P],
                        ff_bf[:, k : k + 1],
                        start=(k == 0 and j == 0),
                        stop=(k == 7 and j == MT - 1),
                    )
            logits = spool.tile([P, MT], F32, tag="logits")
            nc.vector.tensor_add(logits[:], big[:], outb_sb[:])
            expv = spool.tile([P, MT], F32, tag="expv")
            esum = spool.tile([P, 1], F32, tag="esum")
            nc.scalar.activation(
                expv[:], logits[:], mybir.ActivationFunctionType.Exp,
                accum_out=esum[:],
            )
            se = to_sb(part_sum(esum[:])[:], [1, 1], tag="se")

            # ---- AllGather #3: per-core sumexp ----
            ag3 = spool.tile([1, 8], F32, tag="ag3")
            nc.vector.memset(ag3[:], 0.0)
            nc.vector.tensor_copy(ag3[0:1, 0:1], se[:])
            ag3_in = dp.tile([1, 8], F32)
            ag3_out = dp.tile([8, 8], F32)
            nc.sync.dma_start(out=ag3_in[:], in_=ag3[:])
            nc.gpsimd.collective_compute(
                "AllGather",
                mybir.AluOpType.bypass,
                replica_groups=[list(range(NC_N))],
                ins=[ag3_in.opt()],
                outs=[ag3_out.opt()],
            )
            agd = spool.tile([8, 8], F32, tag="agd")
            nc.sync.dma_start(out=agd[:], in_=ag3_out[:])
            tot = to_sb(part_sum(agd[:, 0:1], k=8)[:], [1, 1], tag="tot")
            lnz = spool.tile([1, 1], F32, tag="lnz")
            nc.scalar.activation(lnz[:], tot[:], mybir.ActivationFunctionType.Ln)
            corr = spool.tile([1, 1], F32, tag="corr")
            nc.vector.tensor_sub(corr[:], lnz[:], ln_pg[:])
            corr_b = bcast(corr[:])
            final = spool.tile([P, MT], F32, tag="final")
            nc.vector.tensor_scalar_sub(final[:], logits[:], corr_b[:])
            nc.sync.dma_start(out=vocab_out[:], in_=final[:])

    _split_multi_waits(nc)
    return nc


# -------------------------------------------------------------- host side ---
def _colmajor(v, ncol):
    return np.ascontiguousarray(v.reshape(ncol, P).T)


def _prep_inputs(inputs):
    f32 = np.float32
    idx = int(np.asarray(inputs["input_idx"]).ravel()[0])
    emb = np.asarray(inputs["emb"], f32)
    embedded = emb[idx]
    trigger = np.asarray(inputs["trigger"], f32)
    x = np.concatenate([embedded, trigger])
    h = np.asarray(inputs["hidden"], f32)[0, 0]
    enc = np.asarray(inputs["encoder_outputs"], f32)
    pg_mat = np.asarray(inputs["pg_mat"], f32)
    attn_W = np.asarray(inputs["attn_W"], f32)
    comb_W = np.asarray(inputs["comb_W"], f32)
    comb_b = np.asarray(inputs["comb_b"], f32)
    W_ih = np.asarray(inputs["W_ih"], f32)
    W_hh = np.asarray(inputs["W_hh"], f32)
    b_ih = np.asarray(inputs["b_ih"], f32)
    b_hh = np.asarray(inputs["b_hh"], f32)
    out_W = np.asarray(inputs["out_W"], f32)
    out_b = np.asarray(inputs["out_b"], f32)
    wh_W = np.asarray(inputs["wh_W"], f32)[0]
    ws_W = np.asarray(inputs["ws_W"], f32)[0]
    wx_W = np.asarray(inputs["wx_W"], f32)[0]
    wx_b = np.asarray(inputs["wx_b"], f32)[0]

    et = np.zeros((H, SP), f32)
    et[:, :S] = enc.T
    ep = np.zeros((SP, H), f32)
    ep[:S] = enc
    pgp = np.zeros((SP, EP_COLS), f32)
    pgp[:S, :E] = pg_mat
    pgp[:S, E:] = 1.0
    sm_flat = np.zeros(SP, f32)
    sm_flat[S:] = -1e30
    consts = np.zeros((1, 8), f32)
    consts[0, 0] = 1.0
    consts[0, 1] = wx_b

    shared = {
        "e_t": et.astype(NPBF),
        "e_pad": ep.astype(NPBF),
        "pg_pad": pgp.astype(NPBF),
        "comb_b": _colmajor(comb_b, 8),
        "wh_w": _colmajor(wh_W, 8).astype(NPBF),
        "wx_w": _colmajor(wx_W, 8).astype(NPBF),
        "x_vec": _colmajor(x, 16).astype(NPBF),
        "h_vec": _colmajor(h, 8).astype(NPBF),
        "emb_vec": _colmajor(embedded, 8).astype(NPBF),
        "smask": _colmajor(sm_flat, 4),
        "consts": consts,
        "ident": np.eye(P, dtype=f32),
    }

    in_maps = []
    for c in range(NC_N):
        s = slice(P * c, P * (c + 1))
        rows = np.r_[P * c : P * (c + 1), H + P * c : H + P * (c + 1),
                     2 * H + P * c : 2 * H + P * (c + 1)]
        wsh = np.zeros((VP, H), f32)
        wsh[:VR] = out_W[VR * c : VR * (c + 1)]
        ob = np.full(VP, -40.0, f32)
        ob[:VR] = out_b[VR * c : VR * (c + 1)]
        cmask = np.zeros((P, 8), f32)
        cmask[:, c] = 1.0
        m = {
            "wo_t": np.ascontiguousarray(wsh.T).astype(NPBF),
            "out_b": _colmajor(ob, MT),
            "wih_t": np.ascontiguousarray(W_ih[rows].T).astype(NPBF),
            "whh_t": np.ascontiguousarray(W_hh[rows].T).astype(NPBF),
            "b01": np.stack(
                [b_ih[s] + b_hh[s], b_ih[H + P * c : H + P * (c + 1)]
                 + b_hh[H + P * c : H + P * (c + 1)]], axis=1
            ).astype(f32),
            "b_ihn": b_ih[2 * H + P * c : 2 * H + P * (c + 1)][:, None].astype(f32),
            "b_hhn": b_hh[2 * H + P * c : 2 * H + P * (c + 1)][:, None].astype(f32),
            "attn_t": np.ascontiguousarray(attn_W[:, s].T).astype(NPBF),
            "combh_t": np.ascontiguousarray(comb_W[:, s].T).astype(NPBF),
            "comba_t": np.ascontiguousarray(comb_W[:, H + P * c : H + P * (c + 1)].T
                                            ).astype(NPBF),
            "ws_w": ws_W[s][:, None].astype(NPBF),
            "h_col": h[s][:, None].astype(f32),
            "colmask": cmask,
        }
        m.update(shared)
        in_maps.append(m)
    return in_maps


def kernel(**inputs):
    global _NC_CACHE, LAST_RESULT
    in_maps = _prep_inputs(inputs)
    if _NC_CACHE is None:
        _NC_CACHE = _build_nc()
    res = run_bass_kernel_spmd(
        _NC_CACHE, in_maps, list(range(NC_N)), trace=TRACE
    )
    LAST_RESULT = res

    vocab = np.concatenate(
        [res.results[c]["vocab_out"].T.reshape(-1)[:VR] for c in range(NC_N)]
    )
    atten = res.results[0]["atten_out"].T.reshape(-1)[:E]
    output = np.concatenate([vocab, atten])[None, :].astype(np.float32)
    h_new = res.results[0]["hnew_out"].T.reshape(-1)[None, None, :].astype(np.float32)
    attn_weights = (
        res.results[0]["attnw_out"].T.reshape(-1)[:S][None, :].astype(np.float32)
    )
    return output, h_new, attn_weights


# revision 11
# speedup vs baseline: 1.0971x; 1.0971x over previous
"""AttnDecoderRNN step on 8 Trainium2 NeuronCores (Bass/Tile, SPMD).

Sharding strategy (tensor-parallel over output dims, vocab-sharded big matvec):
  - Embedding lookup is pure data movement: done host-side (one row of emb).
  - GRU gates: W_ih/W_hh row-sharded (each core owns a 128-slice of H for all
    three gates) -> each core computes h_new for its slice. No comm.
  - Attention: attn_W column-sharded against the local h_new slice, fused with
    encoder_outputs @ q so a single AllReduce combines scores [400], the
    ws.h_new dot partial, and re-assembles full h_new (mask trick).
  - pre-activation of the combine FF column-sharded -> AllReduce #2.
  - out projection [V,H] row-sharded 6250 rows/core (padded 6272), weights
    pre-transposed + bf16 on host; PE matvec with v on partitions so the
    softmax reduction is partition-parallel.
  - softmax over V: local sum of exp, AllGather of 8 scalars, log-sum-exp
    correction applied locally. (No max subtraction needed: logits are O(1)
    for this model scale; exp is safely inside fp32 range.)
Outputs: each core writes its vocab shard; core 0's h_new / attn_weights /
atten_p are used. Host gathers + undoes the column-major layout.
"""
import sys

sys.path.insert(0, "/opt/trn_rl_repo")

import numpy as np
import ml_dtypes

import concourse.bass as bass
import concourse.mybir as mybir
import concourse.tile as tile
from concourse.vector_clock import ScopedClock
from concourse import bass_utils
from concourse.bass_utils import run_bass_kernel_spmd

# ---------------------------------------------------------------- patches ---
# This walrus build rejects >1 sync wait on a TPB_CTRL (Drain) instruction;
# TileContext's tail drain accumulates every outstanding sem wait onto it.
# Split the waits onto single-wait nops emitted just before the drain.


def _patched_drain_and_barrier(self, tick_clock, wait_clock):
    nc = self.nc
    carrier = nc.sync.nop(nofuse=True)
    wait_clock.add_sem_waits(carrier.ins, ScopedClock({None: tick_clock.global_clock}))
    si = carrier.ins.sync_info
    waits = list(si.on_wait) if si and si.on_wait else []
    if len(waits) > 1:
        carrier.ins.sync_info = mybir.SyncInfo(
            on_wait=[waits[0]], on_update=si.on_update
        )
        for w in waits[1:]:
            extra = nc.sync.nop(nofuse=True)
            esi = extra.ins.sync_info
            extra.ins.sync_info = mybir.SyncInfo(
                on_wait=[w], on_update=esi.on_update if esi else []
            )
    nc.sync.drain()
    nc.all_engine_barrier()
    popped = nc._tile_sem_poison_stack.pop()
    assert popped is self._sem_poison
    nc.clear_and_free_semaphores(list(self.sems.allocated().values()))
    nc.all_engine_barrier()


tile.TileContext._drain_and_barrier = _patched_drain_and_barrier

# Artifact upload needs a fish bucket; not available (and not needed) here.
bass_utils.upload_artifacts = lambda tmpdir: tmpdir



# This container's antenv lacks axon_hooks; provide the NTFF profile hook via
# ctypes into libaxon_pjrt.so (same shim trn_agent_boot would install).
def _install_ntff_hook_shim():
    import types
    import contextlib
    import ctypes

    if "antenv.axon_hooks" in sys.modules:
        return
    hook = None
    try:
        lib = ctypes.CDLL("/opt/axon/libaxon_pjrt.so")
        if hasattr(lib, "axon_start_nrt_profile"):
            lib.axon_start_nrt_profile.argtypes = [
                ctypes.POINTER(ctypes.c_int64),
                ctypes.c_size_t,
            ]
            lib.axon_start_nrt_profile.restype = ctypes.c_int64
            lib.axon_stop_nrt_profile.argtypes = [ctypes.c_char_p]
            lib.axon_stop_nrt_profile.restype = ctypes.c_int64

            @contextlib.contextmanager
            def _hook(output_dir, device_ids):
                import jax

                jax.devices()
                if device_ids:
                    ids = (ctypes.c_int64 * len(device_ids))(*device_ids)
                    rc = lib.axon_start_nrt_profile(ids, len(device_ids))
                else:
                    rc = lib.axon_start_nrt_profile(None, 0)
                if rc != 0:
                    raise RuntimeError(f"axon_start_nrt_profile rc={rc}")
                try:
                    yield
                finally:
                    n = lib.axon_stop_nrt_profile(str(output_dir).encode())
                    print(f"ntff profile: {n} file(s) -> {output_dir}",
                          file=sys.stderr)

            hook = _hook
    except OSError:
        pass
    mod = types.ModuleType("antenv.axon_hooks")
    mod.get_axon_ntff_profile_hook = lambda: hook
    mod.set_axon_ntff_profile_hook = lambda h: None
    sys.modules["antenv.axon_hooks"] = mod
    import antenv

    antenv.axon_hooks = mod


_install_ntff_hook_shim()

# ------------------------------------------------------------- constants ---
NC_N = 8
H = 1024
V = 50000
E = 602
S = 400
P = 128
VR = V // NC_N        # 6250 real vocab rows per core
MT = 49               # vocab m-tiles per core
VP = MT * P           # 6272 padded vocab rows per core
SP = 512              # padded S
ET_M = 4              # s-tiles (512/128)
EP_COLS = 640         # padded E (5*128)
APT = 5               # atten m-tiles

F32 = mybir.dt.float32
BF16 = mybir.dt.bfloat16
NPBF = ml_dtypes.bfloat16

LAST_RESULT = None    # BassKernelResults of the most recent run (for test.py)
TRACE = False         # set True (e.g. by test.py) to profile
DEBUG = False         # add per-stage debug outputs

_NC_CACHE = None



def _split_multi_waits(nc):
    """This walrus build accepts a single sync wait per instruction; hoist
    extra waits onto same-engine nops placed just before the instruction."""
    for f in nc.m.functions:
        for bb in f.blocks:
            out = []
            for ins in bb.instructions:
                si = ins.sync_info
                waits = list(si.on_wait) if si and si.on_wait else []
                if len(waits) > 1:
                    for w in waits[:-1]:
                        nop = mybir.InstNoOp(
                            name=nc.get_next_instruction_name(),
                            engine=ins.engine,
                            ins=[],
                            outs=[],
                            sync_info=mybir.SyncInfo(on_wait=[w], on_update=[]),
                        )
                        out.append(nop)
                    ins.sync_info = mybir.SyncInfo(
                        on_wait=[waits[-1]], on_update=si.on_update
                    )
                out.append(ins)
            bb.instructions = out


# ------------------------------------------------------------ device code ---
def _build_nc():
    nc = bass.Bass()

    def di(name, shape, dt=BF16):
        return nc.dram_tensor(name, shape, dt, kind="ExternalInput")

    # per-core weights / data (values differ per core, names shared)
    wo_t = di("wo_t", [H, VP])
    out_b = di("out_b", [P, MT], F32)
    wih_t = di("wih_t", [2 * H, 3 * P])
    whh_t = di("whh_t", [H, 3 * P])
    b01 = di("b01", [P, 2], F32)
    b_ihn = di("b_ihn", [P, 1], F32)
    b_hhn = di("b_hhn", [P, 1], F32)
    attn_t = di("attn_t", [P, H])
    e_t = di("e_t", [H, SP])
    e_pad = di("e_pad", [SP, H])
    combh_t = di("combh_t", [P, H])
    comba_t = di("comba_t", [P, H])
    comb_b = di("comb_b", [P, 8], F32)
    pg_pad = di("pg_pad", [SP, EP_COLS])
    wh_w = di("wh_w", [P, 8])
    ws_w = di("ws_w", [P, 1])
    wx_w = di("wx_w", [P, 8])
    x_vec = di("x_vec", [P, 16])
    h_vec = di("h_vec", [P, 8])
    emb_vec = di("emb_vec", [P, 8])
    h_col = di("h_col", [P, 1], F32)
    colmask = di("colmask", [P, 8], F32)
    smask = di("smask", [P, 4], F32)
    consts = di("consts", [1, 8], F32)
    ident = di("ident", [P, P], F32)

    vocab_out = nc.dram_tensor("vocab_out", [P, MT], F32, kind="ExternalOutput")
    hnew_out = nc.dram_tensor("hnew_out", [P, 8], F32, kind="ExternalOutput")
    attnw_out = nc.dram_tensor("attnw_out", [P, 4], F32, kind="ExternalOutput")
    atten_out = nc.dram_tensor("atten_out", [P, APT], F32, kind="ExternalOutput")
    if DEBUG:
        dbg = {
            name: nc.dram_tensor(name, shape, F32, kind="ExternalOutput")
            for name, shape in [
                ("dbg_gi", [P, 3]), ("dbg_gh", [P, 3]), ("dbg_hnewl", [P, 1]),
                ("dbg_qp", [P, 8]), ("dbg_scl", [P, 4]), ("dbg_ar1", [P, 16]),
                ("dbg_aa", [P, 8]), ("dbg_ff", [P, 8]), ("dbg_pgen", [1, 1]),
            ]
        }

    with tile.TileContext(nc) as tc:
        with (
            tc.tile_pool(name="wp", bufs=1) as wp,
            tc.tile_pool(name="sp", bufs=1) as spool,
            tc.tile_pool(name="pp", bufs=1, space="PSUM") as pp,
            tc.tile_pool(name="bigp", bufs=1, space="PSUM") as bigp,
            tc.tile_pool(name="dram", bufs=1, space="DRAM") as dp,
        ):
            # ---- SBUF loads (DMA; scheduler overlaps with compute) ----
            def load(name, dram, shape3, rearr=None):
                t = wp.tile(list(shape3), dram.dtype, tag=name, name=name)
                src = dram[:] if rearr is None else dram[:].rearrange(rearr, p=P)
                nc.sync.dma_start(out=t[:], in_=src)
                return t

            # Dummy collective issued first: absorbs cross-core launch skew +
            # CC-init barrier while weights stream, so the real AllReduce #1
            # runs in lockstep at its latency floor.
            sync0_in = dp.tile([1, 8], F32, name="sync0_in")
            sync0_out = dp.tile([8, 8], F32, name="sync0_out")
            nc.gpsimd.collective_compute(
                "AllGather",
                mybir.AluOpType.bypass,
                replica_groups=[list(range(NC_N))],
                ins=[sync0_in.opt()],
                outs=[sync0_out.opt()],
            )

            # wo_t: 8 per-k-chunk DMAs on the scalar engine's queue -- off the
            # sync queue that issues the (critical-path) small-weight loads,
            # and chunked so PE can start as soon as chunk 0 lands.
            wo_sb = wp.tile([P, 8, VP], BF16, tag="wo", name="wo")
            for k in range(8):
                nc.scalar.dma_start(
                    out=wo_sb[:, k, :], in_=wo_t[P * k : P * (k + 1), :]
                )

            # small weights on the sync queue (critical-path head)
            wih_sb = load("wih", wih_t, [P, 16, 3 * P], "(k p) m -> p k m")
            whh_sb = load("whh", whh_t, [P, 8, 3 * P], "(k p) m -> p k m")
            attn_sb = load("attn", attn_t, [P, H])
            et_sb = load("et", e_t, [P, 8, SP], "(k p) m -> p k m")
            ep_sb = load("ep", e_pad, [P, ET_M, H], "(k p) m -> p k m")
            ch_sb = load("ch", combh_t, [P, H])
            ca_sb = load("ca", comba_t, [P, H])
            pg_sb = load("pg", pg_pad, [P, ET_M, EP_COLS], "(k p) m -> p k m")
            wh_sb = load("wh", wh_w, [P, 8])
            ws_sb = load("ws", ws_w, [P, 1])
            wx_sb = load("wx", wx_w, [P, 8])
            x_sb = load("x", x_vec, [P, 16])
            h_sb = load("h", h_vec, [P, 8])
            emb_sb = load("emb", emb_vec, [P, 8])
            hcol_sb = load("hcol", h_col, [P, 1])
            cmask_sb = load("cmask", colmask, [P, 8])
            smask_sb = load("smask", smask, [P, 4])
            consts_sb = load("consts", consts, [1, 8])
            id_sb = load("ident", ident, [P, P])
            outb_sb = load("outb", out_b, [P, MT])

            onesc = spool.tile([P, 1], F32, tag="onesc")
            nc.vector.memset(onesc[:], 1.0)
            onesr = spool.tile([1, P], F32, tag="onesr")
            nc.vector.memset(onesr[:], 1.0)

            def psum(shape, tag="tiny", bufs=2, name="ps"):
                return pp.tile(list(shape), F32, tag=tag, bufs=bufs, name=name)

            def part_sum(vec_sb, k=P):
                """sum over partitions of [k,1] f32 -> [1,1] psum"""
                out = psum([1, 1])
                nc.tensor.matmul(out[:], onesc[:k, :], vec_sb, start=True, stop=True)
                return out

            def bcast(scalar_sb):
                """[1,1] sbuf f32 -> [128,1] sbuf f32"""
                pb = psum([P, 1])
                nc.tensor.matmul(pb[:], onesr[:], scalar_sb, start=True, stop=True)
                sb = spool.tile([P, 1], F32, tag="bc", name="bc")
                nc.vector.tensor_copy(sb[:], pb[:])
                return sb

            def to_sb(ps, shape, dt=F32, tag="cp"):
                sb = spool.tile(list(shape), dt, tag=tag, name=tag)
                nc.vector.tensor_copy(sb[:], ps)
                return sb

            # ---- stage 1: GRU slice (no comm) ----
            gi = psum([P, 3], tag="gates", name="gi")
            for k in range(16):
                for m in range(3):
                    nc.tensor.matmul(
                        gi[:, m : m + 1],
                        wih_sb[:, k, m * P : (m + 1) * P],
                        x_sb[:, k : k + 1],
                        start=(k == 0 and m == 0),
                        stop=(k == 15 and m == 2),
                    )
            gh = psum([P, 3], tag="gates", name="gh")
            for k in range(8):
                for m in range(3):
                    nc.tensor.matmul(
                        gh[:, m : m + 1],
                        whh_sb[:, k, m * P : (m + 1) * P],
                        h_sb[:, k : k + 1],
                        start=(k == 0 and m == 0),
                        stop=(k == 7 and m == 2),
                    )
            gi_sb = to_sb(gi[:], [P, 3], F32, tag="gisb")
            t01 = spool.tile([P, 2], F32, tag="t01")
            nc.vector.tensor_add(t01[:], gi_sb[:, 0:2], gh[:, 0:2])
            b01_sb = spool.tile([P, 2], F32, tag="b01")
            nc.sync.dma_start(out=b01_sb[:], in_=b01[:])
            nc.vector.tensor_add(t01[:], t01[:], b01_sb[:])
            rz = spool.tile([P, 2], F32, tag="rz")
            nc.scalar.activation(rz[:], t01[:], mybir.ActivationFunctionType.Sigmoid)

            bihn_sb = spool.tile([P, 1], F32, tag="bihn")
            nc.sync.dma_start(out=bihn_sb[:], in_=b_ihn[:])
            bhhn_sb = spool.tile([P, 1], F32, tag="bhhn")
            nc.sync.dma_start(out=bhhn_sb[:], in_=b_hhn[:])
            ghn = spool.tile([P, 1], F32, tag="ghn")
            nc.vector.tensor_add(ghn[:], gh[:, 2:3], bhhn_sb[:])
            tmp1 = spool.tile([P, 1], F32, tag="tmp1")
            nc.vector.tensor_mul(tmp1[:], rz[:, 0:1], ghn[:])
            npre = spool.tile([P, 1], F32, tag="npre")
            nc.vector.tensor_add(npre[:], gi_sb[:, 2:3], bihn_sb[:])
            nc.vector.tensor_add(npre[:], npre[:], tmp1[:])
            n_sb = spool.tile([P, 1], F32, tag="n")
            nc.scalar.activation(n_sb[:], npre[:], mybir.ActivationFunctionType.Tanh)
            # h_new = n + z*(h - n)
            d_sb = spool.tile([P, 1], F32, tag="d")
            nc.vector.tensor_sub(d_sb[:], hcol_sb[:], n_sb[:])
            zt = spool.tile([P, 1], F32, tag="zt")
            nc.vector.tensor_mul(zt[:], rz[:, 1:2], d_sb[:])
            hnew = spool.tile([P, 1], F32, tag="hnew")
            nc.vector.tensor_add(hnew[:], n_sb[:], zt[:])
            hnew_bf = to_sb(hnew[:], [P, 1], BF16, tag="hnewbf")
            if DEBUG:
                nc.sync.dma_start(out=dbg["dbg_hnewl"][:], in_=hnew[:])
                nc.sync.dma_start(out=dbg["dbg_gi"][:], in_=gi_sb[:])
                gh_dbg = to_sb(gh[:], [P, 3], F32, tag="ghdbg")
                nc.sync.dma_start(out=dbg["dbg_gh"][:], in_=gh_dbg[:])

            # ---- stage 2: partial attention scores ----
            qp = psum([P, 8], tag="vec8", name="qp")
            for m in range(8):
                nc.tensor.matmul(
                    qp[:, m : m + 1],
                    attn_sb[:, m * P : (m + 1) * P],
                    hnew_bf[:],
                    start=(m == 0),
                    stop=(m == 7),
                )
            qp_bf = to_sb(qp[:], [P, 8], BF16, tag="qpbf")
            sc = psum([P, 4], tag="vec8", name="sc")
            for k in range(8):
                for m in range(4):
                    nc.tensor.matmul(
                        sc[:, m : m + 1],
                        et_sb[:, k, m * P : (m + 1) * P],
                        qp_bf[:, k : k + 1],
                        start=(k == 0 and m == 0),
                        stop=(k == 7 and m == 3),
                    )
            if DEBUG:
                qp_dbg = to_sb(qp[:], [P, 8], F32, tag="qpdbg")
                nc.sync.dma_start(out=dbg["dbg_qp"][:], in_=qp_dbg[:])
                sc_dbg = to_sb(sc[:], [P, 4], F32, tag="scdbg")
                nc.sync.dma_start(out=dbg["dbg_scl"][:], in_=sc_dbg[:])
            wsp = psum([1, 1], name="wsp")
            nc.tensor.matmul(wsp[:], ws_sb[:], hnew_bf[:], start=True, stop=True)

            # ---- AllReduce #1: scores + ws_partial + h_new assembly ----
            ar1 = spool.tile([P, 16], F32, tag="ar1")
            nc.vector.memset(ar1[:], 0.0)
            nc.vector.tensor_copy(ar1[:, 0:4], sc[:])
            nc.vector.tensor_copy(ar1[0:1, 4:5], wsp[:])
            nc.vector.tensor_scalar_mul(ar1[:, 5:13], cmask_sb[:], hnew[:])
            ar1_in = dp.tile([P, 16], F32)
            ar1_out = dp.tile([P, 16], F32)
            nc.gpsimd.dma_start(out=ar1_in[:], in_=ar1[:])
            nc.gpsimd.collective_compute(
                "AllReduce",
                mybir.AluOpType.add,
                replica_groups=[list(range(NC_N))],
                ins=[ar1_in.opt()],
                outs=[ar1_out.opt()],
            )
            ag1 = spool.tile([P, 16], F32, tag="ag1")
            nc.gpsimd.dma_start(out=ag1[:], in_=ar1_out[:])
            nc.sync.dma_start(out=hnew_out[:], in_=ag1[:, 5:13])
            if DEBUG:
                nc.sync.dma_start(out=dbg["dbg_ar1"][:], in_=ag1[:])

            # ---- stage 3: softmax(scores), attn_applied, p_gen (replicated) ----
            scores = spool.tile([P, 4], F32, tag="scores")
            nc.vector.tensor_add(scores[:], ag1[:, 0:4], smask_sb[:])
            cmax = spool.tile([P, 1], F32, tag="cmax")
            nc.vector.reduce_max(out=cmax[:], in_=scores[:], axis=mybir.AxisListType.X)
            tp = psum([1, P], name="tp")
            nc.tensor.transpose(tp[:], cmax[:], id_sb[:])
            smax = spool.tile([1, 1], F32, tag="smax")
            nc.vector.reduce_max(out=smax[:], in_=tp[:], axis=mybir.AxisListType.X)
            smax_b = bcast(smax[:])
            shift = spool.tile([P, 4], F32, tag="shift")
            nc.vector.tensor_scalar_sub(shift[:], scores[:], smax_b[:])
            expsc = spool.tile([P, 4], F32, tag="expsc")
            rsum = spool.tile([P, 1], F32, tag="rsum")
            nc.scalar.activation(
                expsc[:], shift[:], mybir.ActivationFunctionType.Exp,
                accum_out=rsum[:],
            )
            stot = to_sb(part_sum(rsum[:])[:], [1, 1], tag="stot")
            rinv = spool.tile([1, 1], F32, tag="rinv")
            nc.vector.reciprocal(rinv[:], stot[:])
            rinv_b = bcast(rinv[:])
            aw = spool.tile([P, 4], F32, tag="aw")
            nc.vector.tensor_scalar_mul(aw[:], expsc[:], rinv_b[:])
            nc.sync.dma_start(out=attnw_out[:], in_=aw[:])
            aw_bf = to_sb(aw[:], [P, 4], BF16, tag="awbf")

            # attn_applied = attn_weights @ E  (full, replicated)
            aa = psum([P, 8], tag="vec8", name="aa")
            for k in range(ET_M):
                for m in range(8):
                    nc.tensor.matmul(
                        aa[:, m : m + 1],
                        ep_sb[:, k, m * P : (m + 1) * P],
                        aw_bf[:, k : k + 1],
                        start=(k == 0 and m == 0),
                        stop=(k == ET_M - 1 and m == 7),
                    )
            aa_f = to_sb(aa[:], [P, 8], F32, tag="aaf")
            aa_bf = to_sb(aa[:], [P, 8], BF16, tag="aabf")
            # select this core's h-slice of attn_applied
            t8 = spool.tile([P, 8], F32, tag="t8")
            nc.vector.tensor_mul(t8[:], aa_f[:], cmask_sb[:])
            aac = spool.tile([P, 1], F32, tag="aac")
            nc.vector.reduce_sum(out=aac[:], in_=t8[:], axis=mybir.AxisListType.X)
            aac_bf = to_sb(aac[:], [P, 1], BF16, tag="aacbf")
            if DEBUG:
                nc.sync.dma_start(out=dbg["dbg_aa"][:], in_=aa_f[:])

            # p_gen
            pgp = psum([1, 1], name="pgp")
            for k in range(8):
                nc.tensor.matmul(
                    pgp[:], wh_sb[:, k : k + 1], aa_bf[:, k : k + 1],
                    start=(k == 0), stop=False,
                )
            for k in range(8):
                nc.tensor.matmul(
                    pgp[:], wx_sb[:, k : k + 1], emb_sb[:, k : k + 1],
                    start=False, stop=(k == 7),
                )
            p1 = spool.tile([1, 1], F32, tag="p1")
            nc.vector.tensor_add(p1[:], pgp[:], ag1[0:1, 4:5])
            pgen = spool.tile([1, 1], F32, tag="pgen")
            nc.scalar.activation(
                pgen[:], p1[:], mybir.ActivationFunctionType.Sigmoid,
                bias=consts_sb[0:1, 1:2],
            )
            ln_pg = spool.tile([1, 1], F32, tag="lnpg")
            nc.scalar.activation(ln_pg[:], pgen[:], mybir.ActivationFunctionType.Ln)
            om = spool.tile([1, 1], F32, tag="om")
            nc.vector.tensor_sub(om[:], consts_sb[0:1, 0:1], pgen[:])
            ln_om = spool.tile([1, 1], F32, tag="lnom")
            nc.scalar.activation(ln_om[:], om[:], mybir.ActivationFunctionType.Ln)

            # atten_p = log(attn_weights @ pg_mat) + log(1-p_gen)
            app = psum([P, APT], tag="vec8", name="app")
            for k in range(ET_M):
                for m in range(APT):
                    nc.tensor.matmul(
                        app[:, m : m + 1],
                        pg_sb[:, k, m * P : (m + 1) * P],
                        aw_bf[:, k : k + 1],
                        start=(k == 0 and m == 0),
                        stop=(k == ET_M - 1 and m == APT - 1),
                    )
            ln_ap = spool.tile([P, APT], F32, tag="lnap")
            nc.scalar.activation(ln_ap[:], app[:], mybir.ActivationFunctionType.Ln)
            lnom_b = bcast(ln_om[:])
            apf = spool.tile([P, APT], F32, tag="apf")
            nc.vector.tensor_scalar_add(apf[:], ln_ap[:], lnom_b[:])
            nc.sync.dma_start(out=atten_out[:], in_=apf[:])

            # ---- pre-ff partial + AllReduce #2 ----
            pf = psum([P, 8], tag="vec8", name="pf")
            for m in range(8):
                nc.tensor.matmul(
                    pf[:, m : m + 1],
                    ch_sb[:, m * P : (m + 1) * P],
                    hnew_bf[:],
                    start=(m == 0),
                    stop=False,
                )
                nc.tensor.matmul(
                    pf[:, m : m + 1],
                    ca_sb[:, m * P : (m + 1) * P],
                    aac_bf[:],
                    start=False,
                    stop=(m == 7),
                )
            pf_sb = to_sb(pf[:], [P, 8], F32, tag="pfsb")
            ar2_in = dp.tile([P, 8], F32)
            ar2_out = dp.tile([P, 8], F32)
            nc.gpsimd.dma_start(out=ar2_in[:], in_=pf_sb[:])
            nc.gpsimd.collective_compute(
                "AllReduce",
                mybir.AluOpType.add,
                replica_groups=[list(range(NC_N))],
                ins=[ar2_in.opt()],
                outs=[ar2_out.opt()],
            )
            pff = spool.tile([P, 8], F32, tag="pff")
            nc.gpsimd.dma_start(out=pff[:], in_=ar2_out[:])
            cb_sb = spool.tile([P, 8], F32, tag="cb")
            nc.sync.dma_start(out=cb_sb[:], in_=comb_b[:])
            nc.vector.tensor_add(pff[:], pff[:], cb_sb[:])
            ff = spool.tile([P, 8], F32, tag="ff")
            nc.scalar.activation(ff[:], pff[:], mybir.ActivationFunctionType.Relu)
            ff_bf = to_sb(ff[:], [P, 8], BF16, tag="ffbf")
            if DEBUG:
                nc.sync.dma_start(out=dbg["dbg_ff"][:], in_=ff[:])
                nc.sync.dma_start(out=dbg["dbg_pgen"][:], in_=pgen[:])

            # ---- big matvec: logits shard [128, 49] ----
            big = bigp.tile([P, MT], F32, tag="big")
            for k in range(8):
                for j in range(MT):
                    nc.tensor.matmul(
                        big[:, j : j + 1],
                        wo_sb[:, k, j * P : (j + 1) * P],
                        ff_bf[:, k : k + 1],
                        start=(k == 0 and j == 0),
                        stop=(k == 7 and j == MT - 1),
                    )
            logits = spool.tile([P, MT], F32, tag="logits")
            nc.vector.tensor_add(logits[:], big[:], outb_sb[:])
            expv = spool.tile([P, MT], F32, tag="expv")
            esum = spool.tile([P, 1], F32, tag="esum")
            nc.scalar.activation(
                expv[:], logits[:], mybir.ActivationFunctionType.Exp,
                accum_out=esum[:],
            )
            se = to_sb(part_sum(esum[:])[:], [1, 1], tag="se")

            # ---- AllGather #3: per-core sumexp ----
            ag3 = spool.tile([1, 8], F32, tag="ag3")
            nc.vector.memset(ag3[:], 0.0)
            nc.vector.tensor_copy(ag3[0:1, 0:1], se[:])
            ag3_in = dp.tile([1, 8], F32)
            ag3_out = dp.tile([8, 8], F32)
            nc.gpsimd.dma_start(out=ag3_in[:], in_=ag3[:])
            nc.gpsimd.collective_compute(
                "AllGather",
                mybir.AluOpType.bypass,
                replica_groups=[list(range(NC_N))],
                ins=[ag3_in.opt()],
                outs=[ag3_out.opt()],
            )
            agd = spool.tile([8, 8], F32, tag="agd")
            nc.gpsimd.dma_start(out=agd[:], in_=ag3_out[:])
            tot = to_sb(part_sum(agd[:, 0:1], k=8)[:], [1, 1], tag="tot")
            lnz = spool.tile([1, 1], F32, tag="lnz")
            nc.scalar.activation(lnz[:], tot[:], mybir.ActivationFunctionType.Ln)
            corr = spool.tile([1, 1], F32, tag="corr")
            nc.vector.tensor_sub(corr[:], lnz[:], ln_pg[:])
            corr_b = bcast(corr[:])
            final = spool.tile([P, MT], F32, tag="final")
            nc.vector.tensor_scalar_sub(final[:], logits[:], corr_b[:])
            nc.sync.dma_start(out=vocab_out[:], in_=final[:])

    _split_multi_waits(nc)
    return nc


# -------------------------------------------------------------- host side ---
def _colmajor(v, ncol):
    return np.ascontiguousarray(v.reshape(ncol, P).T)


def _prep_inputs(inputs):
    f32 = np.float32
    idx = int(np.asarray(inputs["input_idx"]).ravel()[0])
    emb = np.asarray(inputs["emb"], f32)
    embedded = emb[idx]
    trigger = np.asarray(inputs["trigger"], f32)
    x = np.concatenate([embedded, trigger])
    h = np.asarray(inputs["hidden"], f32)[0, 0]
    enc = np.asarray(inputs["encoder_outputs"], f32)
    pg_mat = np.asarray(inputs["pg_mat"], f32)
    attn_W = np.asarray(inputs["attn_W"], f32)
    comb_W = np.asarray(inputs["comb_W"], f32)
    comb_b = np.asarray(inputs["comb_b"], f32)
    W_ih = np.asarray(inputs["W_ih"], f32)
    W_hh = np.asarray(inputs["W_hh"], f32)
    b_ih = np.asarray(inputs["b_ih"], f32)
    b_hh = np.asarray(inputs["b_hh"], f32)
    out_W = np.asarray(inputs["out_W"], f32)
    out_b = np.asarray(inputs["out_b"], f32)
    wh_W = np.asarray(inputs["wh_W"], f32)[0]
    ws_W = np.asarray(inputs["ws_W"], f32)[0]
    wx_W = np.asarray(inputs["wx_W"], f32)[0]
    wx_b = np.asarray(inputs["wx_b"], f32)[0]

    et = np.zeros((H, SP), f32)
    et[:, :S] = enc.T
    ep = np.zeros((SP, H), f32)
    ep[:S] = enc
    pgp = np.zeros((SP, EP_COLS), f32)
    pgp[:S, :E] = pg_mat
    pgp[:S, E:] = 1.0
    sm_flat = np.zeros(SP, f32)
    sm_flat[S:] = -1e30
    consts = np.zeros((1, 8), f32)
    consts[0, 0] = 1.0
    consts[0, 1] = wx_b

    shared = {
        "e_t": et.astype(NPBF),
        "e_pad": ep.astype(NPBF),
        "pg_pad": pgp.astype(NPBF),
        "comb_b": _colmajor(comb_b, 8),
        "wh_w": _colmajor(wh_W, 8).astype(NPBF),
        "wx_w": _colmajor(wx_W, 8).astype(NPBF),
        "x_vec": _colmajor(x, 16).astype(NPBF),
        "h_vec": _colmajor(h, 8).astype(NPBF),
        "emb_vec": _colmajor(embedded, 8).astype(NPBF),
        "smask": _colmajor(sm_flat, 4),
        "consts": consts,
        "ident": np.eye(P, dtype=f32),
    }

    in_maps = []
    for c in range(NC_N):
        s = slice(P * c, P * (c + 1))
        rows = np.r_[P * c : P * (c + 1), H + P * c : H + P * (c + 1),
                     2 * H + P * c : 2 * H + P * (c + 1)]
        wsh = np.zeros((VP, H), f32)
        wsh[:VR] = out_W[VR * c : VR * (c + 1)]
        ob = np.full(VP, -40.0, f32)
        ob[:VR] = out_b[VR * c : VR * (c + 1)]
        cmask = np.zeros((P, 8), f32)
        cmask[:, c] = 1.0
        m = {
            "wo_t": np.ascontiguousarray(wsh.T).astype(NPBF),
            "out_b": _colmajor(ob, MT),
            "wih_t": np.ascontiguousarray(W_ih[rows].T).astype(NPBF),
            "whh_t": np.ascontiguousarray(W_hh[rows].T).astype(NPBF),
            "b01": np.stack(
                [b_ih[s] + b_hh[s], b_ih[H + P * c : H + P * (c + 1)]
                 + b_hh[H + P * c : H + P * (c + 1)]], axis=1
            ).astype(f32),
            "b_ihn": b_ih[2 * H + P * c : 2 * H + P * (c + 1)][:, None].astype(f32),
            "b_hhn": b_hh[2 * H + P * c : 2 * H + P * (c + 1)][:, None].astype(f32),
            "attn_t": np.ascontiguousarray(attn_W[:, s].T).astype(NPBF),
            "combh_t": np.ascontiguousarray(comb_W[:, s].T).astype(NPBF),
            "comba_t": np.ascontiguousarray(comb_W[:, H + P * c : H + P * (c + 1)].T
                                            ).astype(NPBF),
            "ws_w": ws_W[s][:, None].astype(NPBF),
            "h_col": h[s][:, None].astype(f32),
            "colmask": cmask,
        }
        m.update(shared)
        in_maps.append(m)
    return in_maps


def kernel(**inputs):
    global _NC_CACHE, LAST_RESULT
    in_maps = _prep_inputs(inputs)
    if _NC_CACHE is None:
        _NC_CACHE = _build_nc()
    res = run_bass_kernel_spmd(
        _NC_CACHE, in_maps, list(range(NC_N)), trace=TRACE
    )
    LAST_RESULT = res

    vocab = np.concatenate(
        [res.results[c]["vocab_out"].T.reshape(-1)[:VR] for c in range(NC_N)]
    )
    atten = res.results[0]["atten_out"].T.reshape(-1)[:E]
    output = np.concatenate([vocab, atten])[None, :].astype(np.float32)
    h_new = res.results[0]["hnew_out"].T.reshape(-1)[None, None, :].astype(np.float32)
    attn_weights = (
        res.results[0]["attnw_out"].T.reshape(-1)[:S][None, :].astype(np.float32)
    )
    return output, h_new, attn_weights


# revision 15
# speedup vs baseline: 1.1893x; 1.0840x over previous
"""AttnDecoderRNN step on 8 Trainium2 NeuronCores (Bass/Tile, SPMD).

Sharding strategy (tensor-parallel over output dims, vocab-sharded big matvec):
  - Embedding lookup is pure data movement: done host-side (one row of emb).
  - GRU gates: W_ih/W_hh row-sharded (each core owns a 128-slice of H for all
    three gates) -> each core computes h_new for its slice. No comm.
  - Attention: attn_W column-sharded against the local h_new slice, fused with
    encoder_outputs @ q so a single AllReduce combines scores [400], the
    ws.h_new dot partial, and re-assembles full h_new (mask trick).
  - pre-activation of the combine FF column-sharded -> AllReduce #2.
  - out projection [V,H] row-sharded 6250 rows/core (padded 6272), weights
    pre-transposed + bf16 on host; PE matvec with v on partitions so the
    softmax reduction is partition-parallel.
  - softmax over V: local sum of exp, AllGather of 8 scalars, log-sum-exp
    correction applied locally. (No max subtraction needed: logits are O(1)
    for this model scale; exp is safely inside fp32 range.)
Outputs: each core writes its vocab shard; core 0's h_new / attn_weights /
atten_p are used. Host gathers + undoes the column-major layout.
"""
import sys

sys.path.insert(0, "/opt/trn_rl_repo")

import numpy as np
import ml_dtypes

import concourse.bass as bass
import concourse.mybir as mybir
import concourse.tile as tile
from concourse.vector_clock import ScopedClock
from concourse import bass_utils
from concourse.bass_utils import run_bass_kernel_spmd

# ---------------------------------------------------------------- patches ---
# This walrus build rejects >1 sync wait on a TPB_CTRL (Drain) instruction;
# TileContext's tail drain accumulates every outstanding sem wait onto it.
# Split the waits onto single-wait nops emitted just before the drain.


def _patched_drain_and_barrier(self, tick_clock, wait_clock):
    nc = self.nc
    carrier = nc.sync.nop(nofuse=True)
    wait_clock.add_sem_waits(carrier.ins, ScopedClock({None: tick_clock.global_clock}))
    si = carrier.ins.sync_info
    waits = list(si.on_wait) if si and si.on_wait else []
    if len(waits) > 1:
        carrier.ins.sync_info = mybir.SyncInfo(
            on_wait=[waits[0]], on_update=si.on_update
        )
        for w in waits[1:]:
            extra = nc.sync.nop(nofuse=True)
            esi = extra.ins.sync_info
            extra.ins.sync_info = mybir.SyncInfo(
                on_wait=[w], on_update=esi.on_update if esi else []
            )
    nc.sync.drain()
    nc.all_engine_barrier()
    popped = nc._tile_sem_poison_stack.pop()
    assert popped is self._sem_poison
    nc.clear_and_free_semaphores(list(self.sems.allocated().values()))
    nc.all_engine_barrier()


tile.TileContext._drain_and_barrier = _patched_drain_and_barrier

# Artifact upload needs a fish bucket; not available (and not needed) here.
bass_utils.upload_artifacts = lambda tmpdir: tmpdir



# This container's antenv lacks axon_hooks; provide the NTFF profile hook via
# ctypes into libaxon_pjrt.so (same shim trn_agent_boot would install).
def _install_ntff_hook_shim():
    import types
    import contextlib
    import ctypes

    if "antenv.axon_hooks" in sys.modules:
        return
    hook = None
    try:
        lib = ctypes.CDLL("/opt/axon/libaxon_pjrt.so")
        if hasattr(lib, "axon_start_nrt_profile"):
            lib.axon_start_nrt_profile.argtypes = [
                ctypes.POINTER(ctypes.c_int64),
                ctypes.c_size_t,
            ]
            lib.axon_start_nrt_profile.restype = ctypes.c_int64
            lib.axon_stop_nrt_profile.argtypes = [ctypes.c_char_p]
            lib.axon_stop_nrt_profile.restype = ctypes.c_int64

            @contextlib.contextmanager
            def _hook(output_dir, device_ids):
                import jax

                jax.devices()
                if device_ids:
                    ids = (ctypes.c_int64 * len(device_ids))(*device_ids)
                    rc = lib.axon_start_nrt_profile(ids, len(device_ids))
                else:
                    rc = lib.axon_start_nrt_profile(None, 0)
                if rc != 0:
                    raise RuntimeError(f"axon_start_nrt_profile rc={rc}")
                try:
                    yield
                finally:
                    n = lib.axon_stop_nrt_profile(str(output_dir).encode())
                    print(f"ntff profile: {n} file(s) -> {output_dir}",
                          file=sys.stderr)

            hook = _hook
    except OSError:
        pass
    mod = types.ModuleType("antenv.axon_hooks")
    mod.get_axon_ntff_profile_hook = lambda: hook
    mod.set_axon_ntff_profile_hook = lambda h: None
    sys.modules["antenv.axon_hooks"] = mod
    import antenv

    antenv.axon_hooks = mod


_install_ntff_hook_shim()

# ------------------------------------------------------------- constants ---
NC_N = 8
H = 1024
V = 50000
E = 602
S = 400
P = 128
VR = V // NC_N        # 6250 real vocab rows per core
MT = 49               # vocab m-tiles per core
VP = MT * P           # 6272 padded vocab rows per core
SP = 512              # padded S
ET_M = 4              # s-tiles (512/128)
EP_COLS = 640         # padded E (5*128)
APT = 5               # atten m-tiles

# packed-input free-dim offsets (bf16 elements per partition)
OFF_WIH = 0
OFF_WHH = OFF_WIH + 16 * 384
OFF_ATTN = OFF_WHH + 8 * 384
OFF_ET = OFF_ATTN + 1024
OFF_EP = OFF_ET + 8 * 512
OFF_CH = OFF_EP + 4 * 1024
OFF_CA = OFF_CH + 1024
OFF_PG = OFF_CA + 1024
OFF_WH = OFF_PG + 4 * 640
OFF_WS = OFF_WH + 16
OFF_WX = OFF_WS + 16
OFF_X = OFF_WX + 16
OFF_HV = OFF_X + 16
OFF_EMB = OFF_HV + 16
NB = OFF_EMB + 16
# f32 pack offsets
OFF_OUTB = 0
OFF_B01 = 56
OFF_BIHN = 64
OFF_BHHN = 72
OFF_CB = 80
OFF_HCOL = 88
OFF_CMASK = 96
OFF_SMASK = 104
OFF_CONSTS = 112
OFF_IDENT = 120
NF = 248

F32 = mybir.dt.float32
BF16 = mybir.dt.bfloat16
NPBF = ml_dtypes.bfloat16

LAST_RESULT = None    # BassKernelResults of the most recent run (for test.py)
TRACE = False         # set True (e.g. by test.py) to profile
DEBUG = False         # add per-stage debug outputs

_NC_CACHE = None



def _split_multi_waits(nc):
    """This walrus build accepts a single sync wait per instruction; hoist
    extra waits onto same-engine nops placed just before the instruction."""
    for f in nc.m.functions:
        for bb in f.blocks:
            out = []
            for ins in bb.instructions:
                si = ins.sync_info
                waits = list(si.on_wait) if si and si.on_wait else []
                if len(waits) > 1:
                    for w in waits[:-1]:
                        nop = mybir.InstNoOp(
                            name=nc.get_next_instruction_name(),
                            engine=ins.engine,
                            ins=[],
                            outs=[],
                            sync_info=mybir.SyncInfo(on_wait=[w], on_update=[]),
                        )
                        out.append(nop)
                    ins.sync_info = mybir.SyncInfo(
                        on_wait=[waits[-1]], on_update=si.on_update
                    )
                out.append(ins)
            bb.instructions = out


# ------------------------------------------------------------ device code ---
def _build_nc():
    nc = bass.Bass()

    def di(name, shape, dt=BF16):
        return nc.dram_tensor(name, shape, dt, kind="ExternalInput")

    # per-core inputs: one packed bf16 tensor, one packed f32 tensor, big W
    wo_t = di("wo_t", [H, VP])
    pack_bf = di("pack_bf", [P, NB])
    pack_f32 = di("pack_f32", [P, NF], F32)

    vocab_out = nc.dram_tensor("vocab_out", [P, MT], F32, kind="ExternalOutput")
    hnew_out = nc.dram_tensor("hnew_out", [P, 8], F32, kind="ExternalOutput")
    attnw_out = nc.dram_tensor("attnw_out", [P, 4], F32, kind="ExternalOutput")
    atten_out = nc.dram_tensor("atten_out", [P, APT], F32, kind="ExternalOutput")
    if DEBUG:
        dbg = {
            name: nc.dram_tensor(name, shape, F32, kind="ExternalOutput")
            for name, shape in [
                ("dbg_gi", [P, 3]), ("dbg_gh", [P, 3]), ("dbg_hnewl", [P, 1]),
                ("dbg_qp", [P, 8]), ("dbg_scl", [P, 4]), ("dbg_ar1", [P, 16]),
                ("dbg_aa", [P, 8]), ("dbg_ff", [P, 8]), ("dbg_pgen", [1, 1]),
            ]
        }

    with tile.TileContext(nc) as tc:
        with (
            tc.tile_pool(name="wp", bufs=1) as wp,
            tc.tile_pool(name="sp", bufs=1) as spool,
            tc.tile_pool(name="pp", bufs=1, space="PSUM") as pp,
            tc.tile_pool(name="bigp", bufs=1, space="PSUM") as bigp,
            tc.tile_pool(name="dram", bufs=1, space="DRAM") as dp,
        ):
            # ---- SBUF loads: 2 packed DMAs + 8 big-W chunk DMAs ----
            # (each dma_start costs ~1.1us of serial issue time on its queue,
            # so everything small rides in two packed transfers)
            pk = wp.tile([P, NB], BF16, tag="pk", name="pk")
            nc.sync.dma_start(out=pk[:], in_=pack_bf[:])
            pf32 = wp.tile([P, NF], F32, tag="pf32", name="pf32")
            nc.sync.dma_start(out=pf32[:], in_=pack_f32[:])
            wo_sb = wp.tile([P, 8, VP], BF16, tag="wo", name="wo")
            for k in range(8):
                nc.scalar.dma_start(
                    out=wo_sb[:, k, :], in_=wo_t[P * k : P * (k + 1), :]
                )

            def bfs(off, n):
                return pk[:, off : off + n]

            wih = lambda k, m: bfs(OFF_WIH + k * 384 + m * P, P)
            whh = lambda k, m: bfs(OFF_WHH + k * 384 + m * P, P)
            attn_m = lambda m: bfs(OFF_ATTN + m * P, P)
            et_km = lambda k, m: bfs(OFF_ET + k * SP + m * P, P)
            ep_km = lambda k, m: bfs(OFF_EP + k * H + m * P, P)
            ch_m = lambda m: bfs(OFF_CH + m * P, P)
            ca_m = lambda m: bfs(OFF_CA + m * P, P)
            pg_km = lambda k, m: bfs(OFF_PG + k * EP_COLS + m * P, P)
            wh_k = lambda k: bfs(OFF_WH + k, 1)
            ws_sb = bfs(OFF_WS, 1)
            wx_k = lambda k: bfs(OFF_WX + k, 1)
            x_k = lambda k: bfs(OFF_X + k, 1)
            h_k = lambda k: bfs(OFF_HV + k, 1)
            emb_k = lambda k: bfs(OFF_EMB + k, 1)

            outb_sb = pf32[:, OFF_OUTB : OFF_OUTB + MT]
            b01_sb = pf32[:, OFF_B01 : OFF_B01 + 2]
            bihn_sb = pf32[:, OFF_BIHN : OFF_BIHN + 1]
            bhhn_sb = pf32[:, OFF_BHHN : OFF_BHHN + 1]
            cb_sb = pf32[:, OFF_CB : OFF_CB + 8]
            hcol_sb = pf32[:, OFF_HCOL : OFF_HCOL + 1]
            cmask_sb = pf32[:, OFF_CMASK : OFF_CMASK + 8]
            smask_sb = pf32[:, OFF_SMASK : OFF_SMASK + 4]
            consts_sb = pf32[0:1, OFF_CONSTS : OFF_CONSTS + 8]
            id_sb = pf32[:, OFF_IDENT : OFF_IDENT + P]

            onesc = spool.tile([P, 1], F32, tag="onesc")
            nc.vector.memset(onesc[:], 1.0)
            onesr = spool.tile([1, P], F32, tag="onesr")
            nc.vector.memset(onesr[:], 1.0)

            def psum(shape, tag="tiny", bufs=2, name="ps"):
                return pp.tile(list(shape), F32, tag=tag, bufs=bufs, name=name)

            def part_sum(vec_sb, k=P):
                """sum over partitions of [k,1] f32 -> [1,1] psum"""
                out = psum([1, 1])
                nc.tensor.matmul(out[:], onesc[:k, :], vec_sb, start=True, stop=True)
                return out

            def bcast(scalar_sb):
                """[1,1] sbuf f32 -> [128,1] sbuf f32"""
                pb = psum([P, 1])
                nc.tensor.matmul(pb[:], onesr[:], scalar_sb, start=True, stop=True)
                sb = spool.tile([P, 1], F32, tag="bc", name="bc")
                nc.vector.tensor_copy(sb[:], pb[:])
                return sb

            def to_sb(ps, shape, dt=F32, tag="cp"):
                sb = spool.tile(list(shape), dt, tag=tag, name=tag)
                nc.vector.tensor_copy(sb[:], ps)
                return sb

            # ---- stage 1: GRU slice (no comm) ----
            gi = psum([P, 3], tag="gates", name="gi")
            for k in range(16):
                for m in range(3):
                    nc.tensor.matmul(
                        gi[:, m : m + 1],
                        wih(k, m),
                        x_k(k),
                        start=(k == 0 and m == 0),
                        stop=(k == 15 and m == 2),
                    )
            gh = psum([P, 3], tag="gates", name="gh")
            for k in range(8):
                for m in range(3):
                    nc.tensor.matmul(
                        gh[:, m : m + 1],
                        whh(k, m),
                        h_k(k),
                        start=(k == 0 and m == 0),
                        stop=(k == 7 and m == 2),
                    )
            gi_sb = to_sb(gi[:], [P, 3], F32, tag="gisb")
            t01 = spool.tile([P, 2], F32, tag="t01")
            nc.vector.tensor_add(t01[:], gi_sb[:, 0:2], gh[:, 0:2])
            nc.vector.tensor_add(t01[:], t01[:], b01_sb)
            rz = spool.tile([P, 2], F32, tag="rz")
            nc.scalar.activation(rz[:], t01[:], mybir.ActivationFunctionType.Sigmoid)

            ghn = spool.tile([P, 1], F32, tag="ghn")
            nc.vector.tensor_add(ghn[:], gh[:, 2:3], bhhn_sb)
            tmp1 = spool.tile([P, 1], F32, tag="tmp1")
            nc.vector.tensor_mul(tmp1[:], rz[:, 0:1], ghn[:])
            npre = spool.tile([P, 1], F32, tag="npre")
            nc.vector.tensor_add(npre[:], gi_sb[:, 2:3], bihn_sb)
            nc.vector.tensor_add(npre[:], npre[:], tmp1[:])
            n_sb = spool.tile([P, 1], F32, tag="n")
            nc.scalar.activation(n_sb[:], npre[:], mybir.ActivationFunctionType.Tanh)
            # h_new = n + z*(h - n)
            d_sb = spool.tile([P, 1], F32, tag="d")
            nc.vector.tensor_sub(d_sb[:], hcol_sb, n_sb[:])
            zt = spool.tile([P, 1], F32, tag="zt")
            nc.vector.tensor_mul(zt[:], rz[:, 1:2], d_sb[:])
            hnew = spool.tile([P, 1], F32, tag="hnew")
            nc.vector.tensor_add(hnew[:], n_sb[:], zt[:])
            hnew_bf = to_sb(hnew[:], [P, 1], BF16, tag="hnewbf")
            if DEBUG:
                nc.sync.dma_start(out=dbg["dbg_hnewl"][:], in_=hnew[:])
                nc.sync.dma_start(out=dbg["dbg_gi"][:], in_=gi_sb[:])
                gh_dbg = to_sb(gh[:], [P, 3], F32, tag="ghdbg")
                nc.sync.dma_start(out=dbg["dbg_gh"][:], in_=gh_dbg[:])

            # ---- stage 2: partial attention scores ----
            qp = psum([P, 8], tag="vec8", name="qp")
            for m in range(8):
                nc.tensor.matmul(
                    qp[:, m : m + 1],
                    attn_m(m),
                    hnew_bf[:],
                    start=(m == 0),
                    stop=(m == 7),
                )
            qp_bf = to_sb(qp[:], [P, 8], BF16, tag="qpbf")
            sc = psum([P, 4], tag="vec8", name="sc")
            for k in range(8):
                for m in range(4):
                    nc.tensor.matmul(
                        sc[:, m : m + 1],
                        et_km(k, m),
                        qp_bf[:, k : k + 1],
                        start=(k == 0 and m == 0),
                        stop=(k == 7 and m == 3),
                    )
            if DEBUG:
                qp_dbg = to_sb(qp[:], [P, 8], F32, tag="qpdbg")
                nc.sync.dma_start(out=dbg["dbg_qp"][:], in_=qp_dbg[:])
                sc_dbg = to_sb(sc[:], [P, 4], F32, tag="scdbg")
                nc.sync.dma_start(out=dbg["dbg_scl"][:], in_=sc_dbg[:])
            wsp = psum([1, 1], name="wsp")
            nc.tensor.matmul(wsp[:], ws_sb, hnew_bf[:], start=True, stop=True)

            # ---- AllReduce #1: scores + ws_partial + h_new assembly ----
            ar1 = spool.tile([P, 16], F32, tag="ar1")
            nc.vector.memset(ar1[:], 0.0)
            nc.vector.tensor_copy(ar1[:, 0:4], sc[:])
            nc.vector.tensor_copy(ar1[0:1, 4:5], wsp[:])
            nc.vector.tensor_scalar_mul(ar1[:, 5:13], cmask_sb, hnew[:])
            ar1_in = dp.tile([P, 16], F32)
            ar1_out = dp.tile([P, 16], F32)
            nc.gpsimd.dma_start(out=ar1_in[:], in_=ar1[:])
            nc.gpsimd.collective_compute(
                "AllReduce",
                mybir.AluOpType.add,
                replica_groups=[list(range(NC_N))],
                ins=[ar1_in.opt()],
                outs=[ar1_out.opt()],
            )
            ag1 = spool.tile([P, 16], F32, tag="ag1")
            nc.gpsimd.dma_start(out=ag1[:], in_=ar1_out[:])
            nc.sync.dma_start(out=hnew_out[:], in_=ag1[:, 5:13])
            if DEBUG:
                nc.sync.dma_start(out=dbg["dbg_ar1"][:], in_=ag1[:])

            # ---- stage 3: softmax(scores), attn_applied, p_gen (replicated) ----
            scores = spool.tile([P, 4], F32, tag="scores")
            nc.vector.tensor_add(scores[:], ag1[:, 0:4], smask_sb)
            cmax = spool.tile([P, 1], F32, tag="cmax")
            nc.vector.reduce_max(out=cmax[:], in_=scores[:], axis=mybir.AxisListType.X)
            tp = psum([1, P], name="tp")
            nc.tensor.transpose(tp[:], cmax[:], id_sb)
            smax = spool.tile([1, 1], F32, tag="smax")
            nc.vector.reduce_max(out=smax[:], in_=tp[:], axis=mybir.AxisListType.X)
            smax_b = bcast(smax[:])
            shift = spool.tile([P, 4], F32, tag="shift")
            nc.vector.tensor_scalar_sub(shift[:], scores[:], smax_b[:])
            expsc = spool.tile([P, 4], F32, tag="expsc")
            rsum = spool.tile([P, 1], F32, tag="rsum")
            nc.scalar.activation(
                expsc[:], shift[:], mybir.ActivationFunctionType.Exp,
                accum_out=rsum[:],
            )
            stot = to_sb(part_sum(rsum[:])[:], [1, 1], tag="stot")
            rinv = spool.tile([1, 1], F32, tag="rinv")
            nc.vector.reciprocal(rinv[:], stot[:])
            rinv_b = bcast(rinv[:])
            aw = spool.tile([P, 4], F32, tag="aw")
            nc.vector.tensor_scalar_mul(aw[:], expsc[:], rinv_b[:])
            nc.sync.dma_start(out=attnw_out[:], in_=aw[:])
            aw_bf = to_sb(aw[:], [P, 4], BF16, tag="awbf")

            # attn_applied = attn_weights @ E  (full, replicated)
            aa = psum([P, 8], tag="vec8", name="aa")
            for k in range(ET_M):
                for m in range(8):
                    nc.tensor.matmul(
                        aa[:, m : m + 1],
                        ep_km(k, m),
                        aw_bf[:, k : k + 1],
                        start=(k == 0 and m == 0),
                        stop=(k == ET_M - 1 and m == 7),
                    )
            aa_f = to_sb(aa[:], [P, 8], F32, tag="aaf")
            aa_bf = to_sb(aa[:], [P, 8], BF16, tag="aabf")
            # select this core's h-slice of attn_applied
            t8 = spool.tile([P, 8], F32, tag="t8")
            nc.vector.tensor_mul(t8[:], aa_f[:], cmask_sb)
            aac = spool.tile([P, 1], F32, tag="aac")
            nc.vector.reduce_sum(out=aac[:], in_=t8[:], axis=mybir.AxisListType.X)
            aac_bf = to_sb(aac[:], [P, 1], BF16, tag="aacbf")
            if DEBUG:
                nc.sync.dma_start(out=dbg["dbg_aa"][:], in_=aa_f[:])

            # p_gen
            pgp = psum([1, 1], name="pgp")
            for k in range(8):
                nc.tensor.matmul(
                    pgp[:], wh_k(k), aa_bf[:, k : k + 1],
                    start=(k == 0), stop=False,
                )
            for k in range(8):
                nc.tensor.matmul(
                    pgp[:], wx_k(k), emb_k(k),
                    start=False, stop=(k == 7),
                )
            p1 = spool.tile([1, 1], F32, tag="p1")
            nc.vector.tensor_add(p1[:], pgp[:], ag1[0:1, 4:5])
            pgen = spool.tile([1, 1], F32, tag="pgen")
            nc.scalar.activation(
                pgen[:], p1[:], mybir.ActivationFunctionType.Sigmoid,
                bias=consts_sb[0:1, 1:2],
            )
            ln_pg = spool.tile([1, 1], F32, tag="lnpg")
            nc.scalar.activation(ln_pg[:], pgen[:], mybir.ActivationFunctionType.Ln)
            om = spool.tile([1, 1], F32, tag="om")
            nc.vector.tensor_sub(om[:], consts_sb[0:1, 0:1], pgen[:])
            ln_om = spool.tile([1, 1], F32, tag="lnom")
            nc.scalar.activation(ln_om[:], om[:], mybir.ActivationFunctionType.Ln)

            # atten_p = log(attn_weights @ pg_mat) + log(1-p_gen)
            app = psum([P, APT], tag="vec8", name="app")
            for k in range(ET_M):
                for m in range(APT):
                    nc.tensor.matmul(
                        app[:, m : m + 1],
                        pg_km(k, m),
                        aw_bf[:, k : k + 1],
                        start=(k == 0 and m == 0),
                        stop=(k == ET_M - 1 and m == APT - 1),
                    )
            ln_ap = spool.tile([P, APT], F32, tag="lnap")
            nc.scalar.activation(ln_ap[:], app[:], mybir.ActivationFunctionType.Ln)
            lnom_b = bcast(ln_om[:])
            apf = spool.tile([P, APT], F32, tag="apf")
            nc.vector.tensor_scalar_add(apf[:], ln_ap[:], lnom_b[:])
            nc.sync.dma_start(out=atten_out[:], in_=apf[:])

            # ---- pre-ff partial + AllReduce #2 ----
            pf = psum([P, 8], tag="vec8", name="pf")
            for m in range(8):
                nc.tensor.matmul(
                    pf[:, m : m + 1],
                    ch_m(m),
                    hnew_bf[:],
                    start=(m == 0),
                    stop=False,
                )
                nc.tensor.matmul(
                    pf[:, m : m + 1],
                    ca_m(m),
                    aac_bf[:],
                    start=False,
                    stop=(m == 7),
                )
            pf_sb = to_sb(pf[:], [P, 8], F32, tag="pfsb")
            ar2_in = dp.tile([P, 8], F32)
            ar2_out = dp.tile([P, 8], F32)
            nc.gpsimd.dma_start(out=ar2_in[:], in_=pf_sb[:])
            nc.gpsimd.collective_compute(
                "AllReduce",
                mybir.AluOpType.add,
                replica_groups=[list(range(NC_N))],
                ins=[ar2_in.opt()],
                outs=[ar2_out.opt()],
            )
            pff = spool.tile([P, 8], F32, tag="pff")
            nc.gpsimd.dma_start(out=pff[:], in_=ar2_out[:])
            nc.vector.tensor_add(pff[:], pff[:], cb_sb)
            ff = spool.tile([P, 8], F32, tag="ff")
            nc.scalar.activation(ff[:], pff[:], mybir.ActivationFunctionType.Relu)
            ff_bf = to_sb(ff[:], [P, 8], BF16, tag="ffbf")
            if DEBUG:
                nc.sync.dma_start(out=dbg["dbg_ff"][:], in_=ff[:])
                nc.sync.dma_start(out=dbg["dbg_pgen"][:], in_=pgen[:])

            # ---- big matvec: logits shard [128, 49] ----
            big = bigp.tile([P, MT], F32, tag="big")
            for k in range(8):
                for j in range(MT):
                    nc.tensor.matmul(
                        big[:, j : j + 1],
                        wo_sb[:, k, j * P : (j + 1) * P],
                        ff_bf[:, k : k + 1],
                        start=(k == 0 and j == 0),
                        stop=(k == 7 and j == MT - 1),
                    )
            logits = spool.tile([P, MT], F32, tag="logits")
            nc.vector.tensor_add(logits[:], big[:], outb_sb)
            expv = spool.tile([P, MT], F32, tag="expv")
            esum = spool.tile([P, 1], F32, tag="esum")
            nc.scalar.activation(
                expv[:], logits[:], mybir.ActivationFunctionType.Exp,
                accum_out=esum[:],
            )
            se = to_sb(part_sum(esum[:])[:], [1, 1], tag="se")

            # ---- AllGather #3: per-core sumexp ----
            ag3 = spool.tile([1, 8], F32, tag="ag3")
            nc.vector.memset(ag3[:], 0.0)
            nc.vector.tensor_copy(ag3[0:1, 0:1], se[:])
            ag3_in = dp.tile([1, 8], F32)
            ag3_out = dp.tile([8, 8], F32)
            nc.gpsimd.dma_start(out=ag3_in[:], in_=ag3[:])
            nc.gpsimd.collective_compute(
                "AllGather",
                mybir.AluOpType.bypass,
                replica_groups=[list(range(NC_N))],
                ins=[ag3_in.opt()],
                outs=[ag3_out.opt()],
            )
            agd = spool.tile([8, 8], F32, tag="agd")
            nc.gpsimd.dma_start(out=agd[:], in_=ag3_out[:])
            tot = to_sb(part_sum(agd[:, 0:1], k=8)[:], [1, 1], tag="tot")
            lnz = spool.tile([1, 1], F32, tag="lnz")
            nc.scalar.activation(lnz[:], tot[:], mybir.ActivationFunctionType.Ln)
            corr = spool.tile([1, 1], F32, tag="corr")
            nc.vector.tensor_sub(corr[:], lnz[:], ln_pg[:])
            corr_b = bcast(corr[:])
            final = spool.tile([P, MT], F32, tag="final")
            nc.vector.tensor_scalar_sub(final[:], logits[:], corr_b[:])
            nc.sync.dma_start(out=vocab_out[:], in_=final[:])

    _split_multi_waits(nc)
    return nc


# -------------------------------------------------------------- host side ---
def _colmajor(v, ncol):
    return np.ascontiguousarray(v.reshape(ncol, P).T)


def _prep_inputs(inputs):
    f32 = np.float32
    idx = int(np.asarray(inputs["input_idx"]).ravel()[0])
    emb = np.asarray(inputs["emb"], f32)
    embedded = emb[idx]
    trigger = np.asarray(inputs["trigger"], f32)
    x = np.concatenate([embedded, trigger])
    h = np.asarray(inputs["hidden"], f32)[0, 0]
    enc = np.asarray(inputs["encoder_outputs"], f32)
    pg_mat = np.asarray(inputs["pg_mat"], f32)
    attn_W = np.asarray(inputs["attn_W"], f32)
    comb_W = np.asarray(inputs["comb_W"], f32)
    comb_b = np.asarray(inputs["comb_b"], f32)
    W_ih = np.asarray(inputs["W_ih"], f32)
    W_hh = np.asarray(inputs["W_hh"], f32)
    b_ih = np.asarray(inputs["b_ih"], f32)
    b_hh = np.asarray(inputs["b_hh"], f32)
    out_W = np.asarray(inputs["out_W"], f32)
    out_b = np.asarray(inputs["out_b"], f32)
    wh_W = np.asarray(inputs["wh_W"], f32)[0]
    ws_W = np.asarray(inputs["ws_W"], f32)[0]
    wx_W = np.asarray(inputs["wx_W"], f32)[0]
    wx_b = np.asarray(inputs["wx_b"], f32)[0]

    et = np.zeros((H, SP), f32)
    et[:, :S] = enc.T
    ep = np.zeros((SP, H), f32)
    ep[:S] = enc
    pgp = np.zeros((SP, EP_COLS), f32)
    pgp[:S, :E] = pg_mat
    pgp[:S, E:] = 1.0
    sm_flat = np.zeros(SP, f32)
    sm_flat[S:] = -1e30

    def chunked(a, k, m):
        """[k*128, m] row-major -> [128, k*m]  ("(k p) m -> p (k m)")"""
        return a.reshape(k, P, m).transpose(1, 0, 2).reshape(P, k * m)

    def pad16(col):
        out = np.zeros((P, 16), f32)
        out[:, : col.shape[1]] = col
        return out

    # shared bf16 pack pieces (order must match OFF_* in the device code)
    et_pk = chunked(et, 8, SP)
    ep_pk = chunked(ep, ET_M, H)
    pg_pk = chunked(pgp, ET_M, EP_COLS)
    wh_pk = pad16(_colmajor(wh_W, 8))
    wx_pk = pad16(_colmajor(wx_W, 8))
    x_pk = pad16(_colmajor(x, 16))
    h_pk = pad16(_colmajor(h, 8))
    emb_pk = pad16(_colmajor(embedded, 8))

    ident = np.eye(P, dtype=f32)
    consts_col = np.zeros((P, 8), f32)
    consts_col[0, 0] = 1.0
    consts_col[0, 1] = wx_b
    smask_col = _colmajor(sm_flat, 4)
    cb_col = _colmajor(comb_b, 8)

    in_maps = []
    for c in range(NC_N):
        s = slice(P * c, P * (c + 1))
        rows = np.r_[P * c : P * (c + 1), H + P * c : H + P * (c + 1),
                     2 * H + P * c : 2 * H + P * (c + 1)]
        wsh = np.zeros((VP, H), f32)
        wsh[:VR] = out_W[VR * c : VR * (c + 1)]
        ob = np.full(VP, -40.0, f32)
        ob[:VR] = out_b[VR * c : VR * (c + 1)]
        cmask = np.zeros((P, 8), f32)
        cmask[:, c] = 1.0

        pack_bf = np.zeros((P, NB), f32)
        pack_bf[:, OFF_WIH : OFF_WIH + 16 * 384] = chunked(
            np.ascontiguousarray(W_ih[rows].T), 16, 384)
        pack_bf[:, OFF_WHH : OFF_WHH + 8 * 384] = chunked(
            np.ascontiguousarray(W_hh[rows].T), 8, 384)
        pack_bf[:, OFF_ATTN : OFF_ATTN + H] = attn_W[:, s].T
        pack_bf[:, OFF_ET : OFF_ET + 8 * SP] = et_pk
        pack_bf[:, OFF_EP : OFF_EP + ET_M * H] = ep_pk
        pack_bf[:, OFF_CH : OFF_CH + H] = comb_W[:, s].T
        pack_bf[:, OFF_CA : OFF_CA + H] = comb_W[:, H + P * c : H + P * (c + 1)].T
        pack_bf[:, OFF_PG : OFF_PG + ET_M * EP_COLS] = pg_pk
        pack_bf[:, OFF_WH : OFF_WH + 16] = wh_pk
        pack_bf[:, OFF_WS] = ws_W[s]
        pack_bf[:, OFF_WX : OFF_WX + 16] = wx_pk
        pack_bf[:, OFF_X : OFF_X + 16] = x_pk
        pack_bf[:, OFF_HV : OFF_HV + 16] = h_pk
        pack_bf[:, OFF_EMB : OFF_EMB + 16] = emb_pk

        pack_f32 = np.zeros((P, NF), f32)
        pack_f32[:, OFF_OUTB : OFF_OUTB + MT] = _colmajor(ob, MT)
        pack_f32[:, OFF_B01] = b_ih[s] + b_hh[s]
        pack_f32[:, OFF_B01 + 1] = (b_ih[H + P * c : H + P * (c + 1)]
                                    + b_hh[H + P * c : H + P * (c + 1)])
        pack_f32[:, OFF_BIHN] = b_ih[2 * H + P * c : 2 * H + P * (c + 1)]
        pack_f32[:, OFF_BHHN] = b_hh[2 * H + P * c : 2 * H + P * (c + 1)]
        pack_f32[:, OFF_CB : OFF_CB + 8] = cb_col
        pack_f32[:, OFF_HCOL] = h[s]
        pack_f32[:, OFF_CMASK : OFF_CMASK + 8] = cmask
        pack_f32[:, OFF_SMASK : OFF_SMASK + 4] = smask_col
        pack_f32[:, OFF_CONSTS : OFF_CONSTS + 8] = consts_col
        pack_f32[:, OFF_IDENT : OFF_IDENT + P] = ident

        m = {
            "wo_t": np.ascontiguousarray(wsh.T).astype(NPBF),
            "pack_bf": pack_bf.astype(NPBF),
            "pack_f32": pack_f32,
        }
        in_maps.append(m)
    return in_maps


def kernel(**inputs):
    global _NC_CACHE, LAST_RESULT
    in_maps = _prep_inputs(inputs)
    if _NC_CACHE is None:
        _NC_CACHE = _build_nc()
    res = run_bass_kernel_spmd(
        _NC_CACHE, in_maps, list(range(NC_N)), trace=TRACE
    )
    LAST_RESULT = res

    vocab = np.concatenate(
        [res.results[c]["vocab_out"].T.reshape(-1)[:VR] for c in range(NC_N)]
    )
    atten = res.results[0]["atten_out"].T.reshape(-1)[:E]
    output = np.concatenate([vocab, atten])[None, :].astype(np.float32)
    h_new = res.results[0]["hnew_out"].T.reshape(-1)[None, None, :].astype(np.float32)
    attn_weights = (
        res.results[0]["attnw_out"].T.reshape(-1)[:S][None, :].astype(np.float32)
    )
    return output, h_new, attn_weights


# revision 17
# speedup vs baseline: 1.2019x; 1.0106x over previous
"""AttnDecoderRNN step on 8 Trainium2 NeuronCores (Bass/Tile, SPMD).

Sharding strategy (tensor-parallel over output dims, vocab-sharded big matvec):
  - Embedding lookup is pure data movement: done host-side (one row of emb).
  - GRU gates: W_ih/W_hh row-sharded (each core owns a 128-slice of H for all
    three gates) -> each core computes h_new for its slice. No comm.
  - Attention: attn_W column-sharded against the local h_new slice, fused with
    encoder_outputs @ q so a single AllReduce combines scores [400], the
    ws.h_new dot partial, and re-assembles full h_new (mask trick).
  - pre-activation of the combine FF column-sharded -> AllReduce #2.
  - out projection [V,H] row-sharded 6250 rows/core (padded 6272), weights
    pre-transposed + bf16 on host; PE matvec with v on partitions so the
    softmax reduction is partition-parallel.
  - softmax over V: local sum of exp, AllGather of 8 scalars, log-sum-exp
    correction applied locally. (No max subtraction needed: logits are O(1)
    for this model scale; exp is safely inside fp32 range.)
Outputs: each core writes its vocab shard; core 0's h_new / attn_weights /
atten_p are used. Host gathers + undoes the column-major layout.
"""
import sys

sys.path.insert(0, "/opt/trn_rl_repo")

import numpy as np
import ml_dtypes

import concourse.bass as bass
import concourse.mybir as mybir
import concourse.tile as tile
from concourse.vector_clock import ScopedClock
from concourse import bass_utils
from concourse.bass_utils import run_bass_kernel_spmd

# ---------------------------------------------------------------- patches ---
# This walrus build rejects >1 sync wait on a TPB_CTRL (Drain) instruction;
# TileContext's tail drain accumulates every outstanding sem wait onto it.
# Split the waits onto single-wait nops emitted just before the drain.


def _patched_drain_and_barrier(self, tick_clock, wait_clock):
    nc = self.nc
    carrier = nc.sync.nop(nofuse=True)
    wait_clock.add_sem_waits(carrier.ins, ScopedClock({None: tick_clock.global_clock}))
    si = carrier.ins.sync_info
    waits = list(si.on_wait) if si and si.on_wait else []
    if len(waits) > 1:
        carrier.ins.sync_info = mybir.SyncInfo(
            on_wait=[waits[0]], on_update=si.on_update
        )
        for w in waits[1:]:
            extra = nc.sync.nop(nofuse=True)
            esi = extra.ins.sync_info
            extra.ins.sync_info = mybir.SyncInfo(
                on_wait=[w], on_update=esi.on_update if esi else []
            )
    nc.sync.drain()
    nc.all_engine_barrier()
    popped = nc._tile_sem_poison_stack.pop()
    assert popped is self._sem_poison
    nc.clear_and_free_semaphores(list(self.sems.allocated().values()))
    nc.all_engine_barrier()


tile.TileContext._drain_and_barrier = _patched_drain_and_barrier

# Artifact upload needs a fish bucket; not available (and not needed) here.
bass_utils.upload_artifacts = lambda tmpdir: tmpdir



# This container's antenv lacks axon_hooks; provide the NTFF profile hook via
# ctypes into libaxon_pjrt.so (same shim trn_agent_boot would install).
def _install_ntff_hook_shim():
    import types
    import contextlib
    import ctypes

    if "antenv.axon_hooks" in sys.modules:
        return
    hook = None
    try:
        lib = ctypes.CDLL("/opt/axon/libaxon_pjrt.so")
        if hasattr(lib, "axon_start_nrt_profile"):
            lib.axon_start_nrt_profile.argtypes = [
                ctypes.POINTER(ctypes.c_int64),
                ctypes.c_size_t,
            ]
            lib.axon_start_nrt_profile.restype = ctypes.c_int64
            lib.axon_stop_nrt_profile.argtypes = [ctypes.c_char_p]
            lib.axon_stop_nrt_profile.restype = ctypes.c_int64

            @contextlib.contextmanager
            def _hook(output_dir, device_ids):
                import jax

                jax.devices()
                if device_ids:
                    ids = (ctypes.c_int64 * len(device_ids))(*device_ids)
                    rc = lib.axon_start_nrt_profile(ids, len(device_ids))
                else:
                    rc = lib.axon_start_nrt_profile(None, 0)
                if rc != 0:
                    raise RuntimeError(f"axon_start_nrt_profile rc={rc}")
                try:
                    yield
                finally:
                    n = lib.axon_stop_nrt_profile(str(output_dir).encode())
                    print(f"ntff profile: {n} file(s) -> {output_dir}",
                          file=sys.stderr)

            hook = _hook
    except OSError:
        pass
    mod = types.ModuleType("antenv.axon_hooks")
    mod.get_axon_ntff_profile_hook = lambda: hook
    mod.set_axon_ntff_profile_hook = lambda h: None
    sys.modules["antenv.axon_hooks"] = mod
    import antenv

    antenv.axon_hooks = mod


_install_ntff_hook_shim()

# ------------------------------------------------------------- constants ---
NC_N = 8
H = 1024
V = 50000
E = 602
S = 400
P = 128
VR = V // NC_N        # 6250 real vocab rows per core
MT = 49               # vocab m-tiles per core
VP = MT * P           # 6272 padded vocab rows per core
SP = 512              # padded S
ET_M = 4              # s-tiles (512/128)
EP_COLS = 640         # padded E (5*128)
APT = 5               # atten m-tiles

# packed-input free-dim offsets (bf16 elements per partition)
# pack1: stage-1 critical weights (GRU); pack2: the rest
OFF_WIH = 0
OFF_WHH = OFF_WIH + 16 * 384
OFF_X = OFF_WHH + 8 * 384
OFF_HV = OFF_X + 16
NB1 = OFF_HV + 16
OFF_ATTN = 0
OFF_ET = OFF_ATTN + 1024
OFF_EP = OFF_ET + 8 * 512
OFF_CH = OFF_EP + 4 * 1024
OFF_CA = OFF_CH + 1024
OFF_PG = OFF_CA + 1024
OFF_WH = OFF_PG + 4 * 640
OFF_WS = OFF_WH + 16
OFF_WX = OFF_WS + 16
OFF_EMB = OFF_WX + 16
NB2 = OFF_EMB + 16
# f32 pack offsets
OFF_OUTB = 0
OFF_B01 = 56
OFF_BIHN = 64
OFF_BHHN = 72
OFF_CB = 80
OFF_HCOL = 88
OFF_CMASK = 96
OFF_SMASK = 104
OFF_CONSTS = 112
OFF_IDENT = 120
NF = 248

F32 = mybir.dt.float32
BF16 = mybir.dt.bfloat16
NPBF = ml_dtypes.bfloat16

LAST_RESULT = None    # BassKernelResults of the most recent run (for test.py)
TRACE = False         # set True (e.g. by test.py) to profile
DEBUG = False         # add per-stage debug outputs

_NC_CACHE = None



def _split_multi_waits(nc):
    """This walrus build accepts a single sync wait per instruction; hoist
    extra waits onto same-engine nops placed just before the instruction."""
    for f in nc.m.functions:
        for bb in f.blocks:
            out = []
            for ins in bb.instructions:
                si = ins.sync_info
                waits = list(si.on_wait) if si and si.on_wait else []
                if len(waits) > 1:
                    for w in waits[:-1]:
                        nop = mybir.InstNoOp(
                            name=nc.get_next_instruction_name(),
                            engine=ins.engine,
                            ins=[],
                            outs=[],
                            sync_info=mybir.SyncInfo(on_wait=[w], on_update=[]),
                        )
                        out.append(nop)
                    ins.sync_info = mybir.SyncInfo(
                        on_wait=[waits[-1]], on_update=si.on_update
                    )
                out.append(ins)
            bb.instructions = out


# ------------------------------------------------------------ device code ---
def _build_nc():
    nc = bass.Bass()

    def di(name, shape, dt=BF16):
        return nc.dram_tensor(name, shape, dt, kind="ExternalInput")

    # per-core inputs: one packed bf16 tensor, one packed f32 tensor, big W
    wo_t = di("wo_t", [H, VP])
    pack1 = di("pack1", [P, NB1])
    pack2 = di("pack2", [P, NB2])
    pack_f32 = di("pack_f32", [P, NF], F32)

    vocab_out = nc.dram_tensor("vocab_out", [P, MT], F32, kind="ExternalOutput")
    hnew_out = nc.dram_tensor("hnew_out", [P, 8], F32, kind="ExternalOutput")
    attnw_out = nc.dram_tensor("attnw_out", [P, 4], F32, kind="ExternalOutput")
    atten_out = nc.dram_tensor("atten_out", [P, APT], F32, kind="ExternalOutput")
    if DEBUG:
        dbg = {
            name: nc.dram_tensor(name, shape, F32, kind="ExternalOutput")
            for name, shape in [
                ("dbg_gi", [P, 3]), ("dbg_gh", [P, 3]), ("dbg_hnewl", [P, 1]),
                ("dbg_qp", [P, 8]), ("dbg_scl", [P, 4]), ("dbg_ar1", [P, 16]),
                ("dbg_aa", [P, 8]), ("dbg_ff", [P, 8]), ("dbg_pgen", [1, 1]),
            ]
        }

    with tile.TileContext(nc) as tc:
        with (
            tc.tile_pool(name="wp", bufs=1) as wp,
            tc.tile_pool(name="sp", bufs=1) as spool,
            tc.tile_pool(name="pp", bufs=1, space="PSUM") as pp,
            tc.tile_pool(name="bigp", bufs=1, space="PSUM") as bigp,
            tc.tile_pool(name="dram", bufs=1, space="DRAM") as dp,
        ):
            # ---- SBUF loads: 2 packed DMAs + 8 big-W chunk DMAs ----
            # (each dma_start costs ~1.1us of serial issue time on its queue,
            # so everything small rides in two packed transfers)
            pf32 = wp.tile([P, NF], F32, tag="pf32", name="pf32")
            nc.sync.dma_start(out=pf32[:], in_=pack_f32[:])
            pk1 = wp.tile([P, NB1], BF16, tag="pk1", name="pk1")
            nc.sync.dma_start(out=pk1[:], in_=pack1[:])
            pk2 = wp.tile([P, NB2], BF16, tag="pk2", name="pk2")
            nc.sync.dma_start(out=pk2[:], in_=pack2[:])
            wo_sb = wp.tile([P, 8, VP], BF16, tag="wo", name="wo")
            for k in range(8):
                nc.scalar.dma_start(
                    out=wo_sb[:, k, :], in_=wo_t[P * k : P * (k + 1), :]
                )

            def b1(off, n):
                return pk1[:, off : off + n]

            def b2(off, n):
                return pk2[:, off : off + n]

            wih = lambda k, m: b1(OFF_WIH + k * 384 + m * P, P)
            whh = lambda k, m: b1(OFF_WHH + k * 384 + m * P, P)
            x_k = lambda k: b1(OFF_X + k, 1)
            h_k = lambda k: b1(OFF_HV + k, 1)
            attn_m = lambda m: b2(OFF_ATTN + m * P, P)
            et_km = lambda k, m: b2(OFF_ET + k * SP + m * P, P)
            ep_km = lambda k, m: b2(OFF_EP + k * H + m * P, P)
            ch_m = lambda m: b2(OFF_CH + m * P, P)
            ca_m = lambda m: b2(OFF_CA + m * P, P)
            pg_km = lambda k, m: b2(OFF_PG + k * EP_COLS + m * P, P)
            wh_k = lambda k: b2(OFF_WH + k, 1)
            ws_sb = b2(OFF_WS, 1)
            wx_k = lambda k: b2(OFF_WX + k, 1)
            emb_k = lambda k: b2(OFF_EMB + k, 1)

            outb_sb = pf32[:, OFF_OUTB : OFF_OUTB + MT]
            b01_sb = pf32[:, OFF_B01 : OFF_B01 + 2]
            bihn_sb = pf32[:, OFF_BIHN : OFF_BIHN + 1]
            bhhn_sb = pf32[:, OFF_BHHN : OFF_BHHN + 1]
            cb_sb = pf32[:, OFF_CB : OFF_CB + 8]
            hcol_sb = pf32[:, OFF_HCOL : OFF_HCOL + 1]
            cmask_sb = pf32[:, OFF_CMASK : OFF_CMASK + 8]
            smask_sb = pf32[:, OFF_SMASK : OFF_SMASK + 4]
            consts_sb = pf32[0:1, OFF_CONSTS : OFF_CONSTS + 8]
            id_sb = pf32[:, OFF_IDENT : OFF_IDENT + P]

            onesc = spool.tile([P, 1], F32, tag="onesc")
            nc.vector.memset(onesc[:], 1.0)
            onesr = spool.tile([1, P], F32, tag="onesr")
            nc.vector.memset(onesr[:], 1.0)

            def psum(shape, tag="tiny", bufs=2, name="ps"):
                return pp.tile(list(shape), F32, tag=tag, bufs=bufs, name=name)

            def part_sum(vec_sb, k=P):
                """sum over partitions of [k,1] f32 -> [1,1] psum"""
                out = psum([1, 1])
                nc.tensor.matmul(out[:], onesc[:k, :], vec_sb, start=True, stop=True)
                return out

            def bcast(scalar_sb):
                """[1,1] sbuf f32 -> [128,1] sbuf f32"""
                pb = psum([P, 1])
                nc.tensor.matmul(pb[:], onesr[:], scalar_sb, start=True, stop=True)
                sb = spool.tile([P, 1], F32, tag="bc", name="bc")
                nc.vector.tensor_copy(sb[:], pb[:])
                return sb

            def to_sb(ps, shape, dt=F32, tag="cp"):
                sb = spool.tile(list(shape), dt, tag=tag, name=tag)
                nc.vector.tensor_copy(sb[:], ps)
                return sb

            # ---- stage 1: GRU slice (no comm) ----
            gi = psum([P, 3], tag="gates", name="gi")
            for k in range(16):
                for m in range(3):
                    nc.tensor.matmul(
                        gi[:, m : m + 1],
                        wih(k, m),
                        x_k(k),
                        start=(k == 0 and m == 0),
                        stop=(k == 15 and m == 2),
                    )
            gh = psum([P, 3], tag="gates", name="gh")
            for k in range(8):
                for m in range(3):
                    nc.tensor.matmul(
                        gh[:, m : m + 1],
                        whh(k, m),
                        h_k(k),
                        start=(k == 0 and m == 0),
                        stop=(k == 7 and m == 2),
                    )
            gi_sb = to_sb(gi[:], [P, 3], F32, tag="gisb")
            t01 = spool.tile([P, 2], F32, tag="t01")
            nc.vector.tensor_add(t01[:], gi_sb[:, 0:2], gh[:, 0:2])
            nc.vector.tensor_add(t01[:], t01[:], b01_sb)
            rz = spool.tile([P, 2], F32, tag="rz")
            nc.scalar.activation(rz[:], t01[:], mybir.ActivationFunctionType.Sigmoid)

            ghn = spool.tile([P, 1], F32, tag="ghn")
            nc.vector.tensor_add(ghn[:], gh[:, 2:3], bhhn_sb)
            tmp1 = spool.tile([P, 1], F32, tag="tmp1")
            nc.vector.tensor_mul(tmp1[:], rz[:, 0:1], ghn[:])
            npre = spool.tile([P, 1], F32, tag="npre")
            nc.vector.tensor_add(npre[:], gi_sb[:, 2:3], bihn_sb)
            nc.vector.tensor_add(npre[:], npre[:], tmp1[:])
            n_sb = spool.tile([P, 1], F32, tag="n")
            nc.scalar.activation(n_sb[:], npre[:], mybir.ActivationFunctionType.Tanh)
            # h_new = n + z*(h - n)
            d_sb = spool.tile([P, 1], F32, tag="d")
            nc.vector.tensor_sub(d_sb[:], hcol_sb, n_sb[:])
            zt = spool.tile([P, 1], F32, tag="zt")
            nc.vector.tensor_mul(zt[:], rz[:, 1:2], d_sb[:])
            hnew = spool.tile([P, 1], F32, tag="hnew")
            nc.vector.tensor_add(hnew[:], n_sb[:], zt[:])
            hnew_bf = to_sb(hnew[:], [P, 1], BF16, tag="hnewbf")
            if DEBUG:
                nc.sync.dma_start(out=dbg["dbg_hnewl"][:], in_=hnew[:])
                nc.sync.dma_start(out=dbg["dbg_gi"][:], in_=gi_sb[:])
                gh_dbg = to_sb(gh[:], [P, 3], F32, tag="ghdbg")
                nc.sync.dma_start(out=dbg["dbg_gh"][:], in_=gh_dbg[:])

            # ---- stage 2: partial attention scores ----
            qp = psum([P, 8], tag="vec8", name="qp")
            for m in range(8):
                nc.tensor.matmul(
                    qp[:, m : m + 1],
                    attn_m(m),
                    hnew_bf[:],
                    start=(m == 0),
                    stop=(m == 7),
                )
            qp_bf = to_sb(qp[:], [P, 8], BF16, tag="qpbf")
            sc = psum([P, 4], tag="vec8", name="sc")
            for k in range(8):
                for m in range(4):
                    nc.tensor.matmul(
                        sc[:, m : m + 1],
                        et_km(k, m),
                        qp_bf[:, k : k + 1],
                        start=(k == 0 and m == 0),
                        stop=(k == 7 and m == 3),
                    )
            if DEBUG:
                qp_dbg = to_sb(qp[:], [P, 8], F32, tag="qpdbg")
                nc.sync.dma_start(out=dbg["dbg_qp"][:], in_=qp_dbg[:])
                sc_dbg = to_sb(sc[:], [P, 4], F32, tag="scdbg")
                nc.sync.dma_start(out=dbg["dbg_scl"][:], in_=sc_dbg[:])
            wsp = psum([1, 1], name="wsp")
            nc.tensor.matmul(wsp[:], ws_sb, hnew_bf[:], start=True, stop=True)

            # ---- AllReduce #1: scores + ws_partial + h_new assembly ----
            ar1 = spool.tile([P, 16], F32, tag="ar1")
            nc.vector.memset(ar1[:], 0.0)
            nc.vector.tensor_copy(ar1[:, 0:4], sc[:])
            nc.vector.tensor_copy(ar1[0:1, 4:5], wsp[:])
            nc.vector.tensor_scalar_mul(ar1[:, 5:13], cmask_sb, hnew[:])
            ar1_in = dp.tile([P, 16], F32)
            ar1_out = dp.tile([P, 16], F32)
            nc.gpsimd.dma_start(out=ar1_in[:], in_=ar1[:])
            nc.gpsimd.collective_compute(
                "AllReduce",
                mybir.AluOpType.add,
                replica_groups=[list(range(NC_N))],
                ins=[ar1_in.opt()],
                outs=[ar1_out.opt()],
            )
            ag1 = spool.tile([P, 16], F32, tag="ag1")
            nc.gpsimd.dma_start(out=ag1[:], in_=ar1_out[:])
            nc.sync.dma_start(out=hnew_out[:], in_=ag1[:, 5:13])
            if DEBUG:
                nc.sync.dma_start(out=dbg["dbg_ar1"][:], in_=ag1[:])

            # ---- stage 3: softmax(scores), attn_applied, p_gen (replicated) ----
            scores = spool.tile([P, 4], F32, tag="scores")
            nc.vector.tensor_add(scores[:], ag1[:, 0:4], smask_sb)
            cmax = spool.tile([P, 1], F32, tag="cmax")
            nc.vector.reduce_max(out=cmax[:], in_=scores[:], axis=mybir.AxisListType.X)
            tp = psum([1, P], name="tp")
            nc.tensor.transpose(tp[:], cmax[:], id_sb)
            smax = spool.tile([1, 1], F32, tag="smax")
            nc.vector.reduce_max(out=smax[:], in_=tp[:], axis=mybir.AxisListType.X)
            smax_b = bcast(smax[:])
            shift = spool.tile([P, 4], F32, tag="shift")
            nc.vector.tensor_scalar_sub(shift[:], scores[:], smax_b[:])
            expsc = spool.tile([P, 4], F32, tag="expsc")
            rsum = spool.tile([P, 1], F32, tag="rsum")
            nc.scalar.activation(
                expsc[:], shift[:], mybir.ActivationFunctionType.Exp,
                accum_out=rsum[:],
            )
            stot = to_sb(part_sum(rsum[:])[:], [1, 1], tag="stot")
            rinv = spool.tile([1, 1], F32, tag="rinv")
            nc.vector.reciprocal(rinv[:], stot[:])
            rinv_b = bcast(rinv[:])
            aw = spool.tile([P, 4], F32, tag="aw")
            nc.vector.tensor_scalar_mul(aw[:], expsc[:], rinv_b[:])
            nc.sync.dma_start(out=attnw_out[:], in_=aw[:])
            aw_bf = to_sb(aw[:], [P, 4], BF16, tag="awbf")

            # attn_applied = attn_weights @ E  (full, replicated)
            aa = psum([P, 8], tag="vec8", name="aa")
            for k in range(ET_M):
                for m in range(8):
                    nc.tensor.matmul(
                        aa[:, m : m + 1],
                        ep_km(k, m),
                        aw_bf[:, k : k + 1],
                        start=(k == 0 and m == 0),
                        stop=(k == ET_M - 1 and m == 7),
                    )
            aa_f = to_sb(aa[:], [P, 8], F32, tag="aaf")
            aa_bf = to_sb(aa[:], [P, 8], BF16, tag="aabf")
            # select this core's h-slice of attn_applied
            t8 = spool.tile([P, 8], F32, tag="t8")
            nc.vector.tensor_mul(t8[:], aa_f[:], cmask_sb)
            aac = spool.tile([P, 1], F32, tag="aac")
            nc.vector.reduce_sum(out=aac[:], in_=t8[:], axis=mybir.AxisListType.X)
            aac_bf = to_sb(aac[:], [P, 1], BF16, tag="aacbf")
            if DEBUG:
                nc.sync.dma_start(out=dbg["dbg_aa"][:], in_=aa_f[:])

            # ---- pre-ff partial + AllReduce #2 ----
            pf = psum([P, 8], tag="vec8", name="pf")
            for m in range(8):
                nc.tensor.matmul(
                    pf[:, m : m + 1],
                    ch_m(m),
                    hnew_bf[:],
                    start=(m == 0),
                    stop=False,
                )
                nc.tensor.matmul(
                    pf[:, m : m + 1],
                    ca_m(m),
                    aac_bf[:],
                    start=False,
                    stop=(m == 7),
                )
            pf_sb = to_sb(pf[:], [P, 8], F32, tag="pfsb")
            ar2_in = dp.tile([P, 8], F32)
            ar2_out = dp.tile([P, 8], F32)
            nc.gpsimd.dma_start(out=ar2_in[:], in_=pf_sb[:])
            nc.gpsimd.collective_compute(
                "AllReduce",
                mybir.AluOpType.add,
                replica_groups=[list(range(NC_N))],
                ins=[ar2_in.opt()],
                outs=[ar2_out.opt()],
            )
            # p_gen
            pgp = psum([1, 1], name="pgp")
            for k in range(8):
                nc.tensor.matmul(
                    pgp[:], wh_k(k), aa_bf[:, k : k + 1],
                    start=(k == 0), stop=False,
                )
            for k in range(8):
                nc.tensor.matmul(
                    pgp[:], wx_k(k), emb_k(k),
                    start=False, stop=(k == 7),
                )
            p1 = spool.tile([1, 1], F32, tag="p1")
            nc.vector.tensor_add(p1[:], pgp[:], ag1[0:1, 4:5])
            pgen = spool.tile([1, 1], F32, tag="pgen")
            nc.scalar.activation(
                pgen[:], p1[:], mybir.ActivationFunctionType.Sigmoid,
                bias=consts_sb[0:1, 1:2],
            )
            ln_pg = spool.tile([1, 1], F32, tag="lnpg")
            nc.scalar.activation(ln_pg[:], pgen[:], mybir.ActivationFunctionType.Ln)
            om = spool.tile([1, 1], F32, tag="om")
            nc.vector.tensor_sub(om[:], consts_sb[0:1, 0:1], pgen[:])
            ln_om = spool.tile([1, 1], F32, tag="lnom")
            nc.scalar.activation(ln_om[:], om[:], mybir.ActivationFunctionType.Ln)

            # atten_p = log(attn_weights @ pg_mat) + log(1-p_gen)
            app = psum([P, APT], tag="vec8", name="app")
            for k in range(ET_M):
                for m in range(APT):
                    nc.tensor.matmul(
                        app[:, m : m + 1],
                        pg_km(k, m),
                        aw_bf[:, k : k + 1],
                        start=(k == 0 and m == 0),
                        stop=(k == ET_M - 1 and m == APT - 1),
                    )
            ln_ap = spool.tile([P, APT], F32, tag="lnap")
            nc.scalar.activation(ln_ap[:], app[:], mybir.ActivationFunctionType.Ln)
            lnom_b = bcast(ln_om[:])
            apf = spool.tile([P, APT], F32, tag="apf")
            nc.vector.tensor_scalar_add(apf[:], ln_ap[:], lnom_b[:])
            nc.sync.dma_start(out=atten_out[:], in_=apf[:])

            pff = spool.tile([P, 8], F32, tag="pff")
            nc.gpsimd.dma_start(out=pff[:], in_=ar2_out[:])
            nc.vector.tensor_add(pff[:], pff[:], cb_sb)
            ff = spool.tile([P, 8], F32, tag="ff")
            nc.scalar.activation(ff[:], pff[:], mybir.ActivationFunctionType.Relu)
            ff_bf = to_sb(ff[:], [P, 8], BF16, tag="ffbf")
            if DEBUG:
                nc.sync.dma_start(out=dbg["dbg_ff"][:], in_=ff[:])
                nc.sync.dma_start(out=dbg["dbg_pgen"][:], in_=pgen[:])

            # ---- big matvec: logits shard [128, 49] ----
            big = bigp.tile([P, MT], F32, tag="big")
            for k in range(8):
                for j in range(MT):
                    nc.tensor.matmul(
                        big[:, j : j + 1],
                        wo_sb[:, k, j * P : (j + 1) * P],
                        ff_bf[:, k : k + 1],
                        start=(k == 0 and j == 0),
                        stop=(k == 7 and j == MT - 1),
                    )
            logits = spool.tile([P, MT], F32, tag="logits")
            nc.vector.tensor_add(logits[:], big[:], outb_sb)
            expv = spool.tile([P, MT], F32, tag="expv")
            esum = spool.tile([P, 1], F32, tag="esum")
            nc.scalar.activation(
                expv[:], logits[:], mybir.ActivationFunctionType.Exp,
                accum_out=esum[:],
            )
            se = to_sb(part_sum(esum[:])[:], [1, 1], tag="se")

            # ---- AllGather #3: per-core sumexp ----
            ag3 = spool.tile([1, 8], F32, tag="ag3")
            nc.vector.memset(ag3[:], 0.0)
            nc.vector.tensor_copy(ag3[0:1, 0:1], se[:])
            ag3_in = dp.tile([1, 8], F32)
            ag3_out = dp.tile([8, 8], F32)
            nc.gpsimd.dma_start(out=ag3_in[:], in_=ag3[:])
            nc.gpsimd.collective_compute(
                "AllGather",
                mybir.AluOpType.bypass,
                replica_groups=[list(range(NC_N))],
                ins=[ag3_in.opt()],
                outs=[ag3_out.opt()],
            )
            agd = spool.tile([8, 8], F32, tag="agd")
            nc.gpsimd.dma_start(out=agd[:], in_=ag3_out[:])
            tot = to_sb(part_sum(agd[:, 0:1], k=8)[:], [1, 1], tag="tot")
            lnz = spool.tile([1, 1], F32, tag="lnz")
            nc.scalar.activation(lnz[:], tot[:], mybir.ActivationFunctionType.Ln)
            corr = spool.tile([1, 1], F32, tag="corr")
            nc.vector.tensor_sub(corr[:], lnz[:], ln_pg[:])
            corr_b = bcast(corr[:])
            final = spool.tile([P, MT], F32, tag="final")
            nc.vector.tensor_scalar_sub(final[:], logits[:], corr_b[:])
            nc.sync.dma_start(out=vocab_out[:], in_=final[:])

    _split_multi_waits(nc)
    return nc


# -------------------------------------------------------------- host side ---
def _colmajor(v, ncol):
    return np.ascontiguousarray(v.reshape(ncol, P).T)


def _prep_inputs(inputs):
    f32 = np.float32
    idx = int(np.asarray(inputs["input_idx"]).ravel()[0])
    emb = np.asarray(inputs["emb"], f32)
    embedded = emb[idx]
    trigger = np.asarray(inputs["trigger"], f32)
    x = np.concatenate([embedded, trigger])
    h = np.asarray(inputs["hidden"], f32)[0, 0]
    enc = np.asarray(inputs["encoder_outputs"], f32)
    pg_mat = np.asarray(inputs["pg_mat"], f32)
    attn_W = np.asarray(inputs["attn_W"], f32)
    comb_W = np.asarray(inputs["comb_W"], f32)
    comb_b = np.asarray(inputs["comb_b"], f32)
    W_ih = np.asarray(inputs["W_ih"], f32)
    W_hh = np.asarray(inputs["W_hh"], f32)
    b_ih = np.asarray(inputs["b_ih"], f32)
    b_hh = np.asarray(inputs["b_hh"], f32)
    out_W = np.asarray(inputs["out_W"], f32)
    out_b = np.asarray(inputs["out_b"], f32)
    wh_W = np.asarray(inputs["wh_W"], f32)[0]
    ws_W = np.asarray(inputs["ws_W"], f32)[0]
    wx_W = np.asarray(inputs["wx_W"], f32)[0]
    wx_b = np.asarray(inputs["wx_b"], f32)[0]

    et = np.zeros((H, SP), f32)
    et[:, :S] = enc.T
    ep = np.zeros((SP, H), f32)
    ep[:S] = enc
    pgp = np.zeros((SP, EP_COLS), f32)
    pgp[:S, :E] = pg_mat
    pgp[:S, E:] = 1.0
    sm_flat = np.zeros(SP, f32)
    sm_flat[S:] = -1e30

    def chunked(a, k, m):
        """[k*128, m] row-major -> [128, k*m]  ("(k p) m -> p (k m)")"""
        return a.reshape(k, P, m).transpose(1, 0, 2).reshape(P, k * m)

    def pad16(col):
        out = np.zeros((P, 16), f32)
        out[:, : col.shape[1]] = col
        return out

    # shared bf16 pack pieces (order must match OFF_* in the device code)
    et_pk = chunked(et, 8, SP)
    ep_pk = chunked(ep, ET_M, H)
    pg_pk = chunked(pgp, ET_M, EP_COLS)
    wh_pk = pad16(_colmajor(wh_W, 8))
    wx_pk = pad16(_colmajor(wx_W, 8))
    x_pk = pad16(_colmajor(x, 16))
    h_pk = pad16(_colmajor(h, 8))
    emb_pk = pad16(_colmajor(embedded, 8))

    ident = np.eye(P, dtype=f32)
    consts_col = np.zeros((P, 8), f32)
    consts_col[0, 0] = 1.0
    consts_col[0, 1] = wx_b
    smask_col = _colmajor(sm_flat, 4)
    cb_col = _colmajor(comb_b, 8)

    in_maps = []
    for c in range(NC_N):
        s = slice(P * c, P * (c + 1))
        rows = np.r_[P * c : P * (c + 1), H + P * c : H + P * (c + 1),
                     2 * H + P * c : 2 * H + P * (c + 1)]
        wsh = np.zeros((VP, H), f32)
        wsh[:VR] = out_W[VR * c : VR * (c + 1)]
        ob = np.full(VP, -40.0, f32)
        ob[:VR] = out_b[VR * c : VR * (c + 1)]
        cmask = np.zeros((P, 8), f32)
        cmask[:, c] = 1.0

        p1 = np.zeros((P, NB1), f32)
        p1[:, OFF_WIH : OFF_WIH + 16 * 384] = chunked(
            np.ascontiguousarray(W_ih[rows].T), 16, 384)
        p1[:, OFF_WHH : OFF_WHH + 8 * 384] = chunked(
            np.ascontiguousarray(W_hh[rows].T), 8, 384)
        p1[:, OFF_X : OFF_X + 16] = x_pk
        p1[:, OFF_HV : OFF_HV + 16] = h_pk

        p2 = np.zeros((P, NB2), f32)
        p2[:, OFF_ATTN : OFF_ATTN + H] = attn_W[:, s].T
        p2[:, OFF_ET : OFF_ET + 8 * SP] = et_pk
        p2[:, OFF_EP : OFF_EP + ET_M * H] = ep_pk
        p2[:, OFF_CH : OFF_CH + H] = comb_W[:, s].T
        p2[:, OFF_CA : OFF_CA + H] = comb_W[:, H + P * c : H + P * (c + 1)].T
        p2[:, OFF_PG : OFF_PG + ET_M * EP_COLS] = pg_pk
        p2[:, OFF_WH : OFF_WH + 16] = wh_pk
        p2[:, OFF_WS] = ws_W[s]
        p2[:, OFF_WX : OFF_WX + 16] = wx_pk
        p2[:, OFF_EMB : OFF_EMB + 16] = emb_pk

        pack_f32 = np.zeros((P, NF), f32)
        pack_f32[:, OFF_OUTB : OFF_OUTB + MT] = _colmajor(ob, MT)
        pack_f32[:, OFF_B01] = b_ih[s] + b_hh[s]
        pack_f32[:, OFF_B01 + 1] = (b_ih[H + P * c : H + P * (c + 1)]
                                    + b_hh[H + P * c : H + P * (c + 1)])
        pack_f32[:, OFF_BIHN] = b_ih[2 * H + P * c : 2 * H + P * (c + 1)]
        pack_f32[:, OFF_BHHN] = b_hh[2 * H + P * c : 2 * H + P * (c + 1)]
        pack_f32[:, OFF_CB : OFF_CB + 8] = cb_col
        pack_f32[:, OFF_HCOL] = h[s]
        pack_f32[:, OFF_CMASK : OFF_CMASK + 8] = cmask
        pack_f32[:, OFF_SMASK : OFF_SMASK + 4] = smask_col
        pack_f32[:, OFF_CONSTS : OFF_CONSTS + 8] = consts_col
        pack_f32[:, OFF_IDENT : OFF_IDENT + P] = ident

        m = {
            "wo_t": np.ascontiguousarray(wsh.T).astype(NPBF),
            "pack1": p1.astype(NPBF),
            "pack2": p2.astype(NPBF),
            "pack_f32": pack_f32,
        }
        in_maps.append(m)
    return in_maps


def kernel(**inputs):
    global _NC_CACHE, LAST_RESULT
    in_maps = _prep_inputs(inputs)
    if _NC_CACHE is None:
        _NC_CACHE = _build_nc()
    res = run_bass_kernel_spmd(
        _NC_CACHE, in_maps, list(range(NC_N)), trace=TRACE
    )
    LAST_RESULT = res

    vocab = np.concatenate(
        [res.results[c]["vocab_out"].T.reshape(-1)[:VR] for c in range(NC_N)]
    )
    atten = res.results[0]["atten_out"].T.reshape(-1)[:E]
    output = np.concatenate([vocab, atten])[None, :].astype(np.float32)
    h_new = res.results[0]["hnew_out"].T.reshape(-1)[None, None, :].astype(np.float32)
    attn_weights = (
        res.results[0]["attnw_out"].T.reshape(-1)[:S][None, :].astype(np.float32)
    )
    return output, h_new, attn_weights


# revision 19
# speedup vs baseline: 1.2092x; 1.0061x over previous
"""AttnDecoderRNN step on 8 Trainium2 NeuronCores (Bass/Tile, SPMD).

Sharding strategy (tensor-parallel over output dims, vocab-sharded big matvec):
  - Embedding lookup is pure data movement: done host-side (one row of emb).
  - GRU gates: W_ih/W_hh row-sharded (each core owns a 128-slice of H for all
    three gates) -> each core computes h_new for its slice. No comm.
  - Attention: attn_W column-sharded against the local h_new slice, fused with
    encoder_outputs @ q so a single AllReduce combines scores [400], the
    ws.h_new dot partial, and re-assembles full h_new (mask trick).
  - pre-activation of the combine FF column-sharded -> AllReduce #2.
  - out projection [V,H] row-sharded 6250 rows/core (padded 6272), weights
    pre-transposed + bf16 on host; PE matvec with v on partitions so the
    softmax reduction is partition-parallel.
  - softmax over V: local sum of exp, AllGather of 8 scalars, log-sum-exp
    correction applied locally. (No max subtraction needed: logits are O(1)
    for this model scale; exp is safely inside fp32 range.)
Outputs: each core writes its vocab shard; core 0's h_new / attn_weights /
atten_p are used. Host gathers + undoes the column-major layout.
"""
import sys

sys.path.insert(0, "/opt/trn_rl_repo")

import numpy as np
import ml_dtypes

import concourse.bass as bass
import concourse.mybir as mybir
import concourse.tile as tile
from concourse.tile_rust import add_dep_helper
from concourse.vector_clock import ScopedClock
from concourse import bass_utils
from concourse.bass_utils import run_bass_kernel_spmd

# ---------------------------------------------------------------- patches ---
# This walrus build rejects >1 sync wait on a TPB_CTRL (Drain) instruction;
# TileContext's tail drain accumulates every outstanding sem wait onto it.
# Split the waits onto single-wait nops emitted just before the drain.


def _patched_drain_and_barrier(self, tick_clock, wait_clock):
    nc = self.nc
    carrier = nc.sync.nop(nofuse=True)
    wait_clock.add_sem_waits(carrier.ins, ScopedClock({None: tick_clock.global_clock}))
    si = carrier.ins.sync_info
    waits = list(si.on_wait) if si and si.on_wait else []
    if len(waits) > 1:
        carrier.ins.sync_info = mybir.SyncInfo(
            on_wait=[waits[0]], on_update=si.on_update
        )
        for w in waits[1:]:
            extra = nc.sync.nop(nofuse=True)
            esi = extra.ins.sync_info
            extra.ins.sync_info = mybir.SyncInfo(
                on_wait=[w], on_update=esi.on_update if esi else []
            )
    nc.sync.drain()
    nc.all_engine_barrier()
    popped = nc._tile_sem_poison_stack.pop()
    assert popped is self._sem_poison
    nc.clear_and_free_semaphores(list(self.sems.allocated().values()))
    nc.all_engine_barrier()


tile.TileContext._drain_and_barrier = _patched_drain_and_barrier

# Artifact upload needs a fish bucket; not available (and not needed) here.
bass_utils.upload_artifacts = lambda tmpdir: tmpdir



# This container's antenv lacks axon_hooks; provide the NTFF profile hook via
# ctypes into libaxon_pjrt.so (same shim trn_agent_boot would install).
def _install_ntff_hook_shim():
    import types
    import contextlib
    import ctypes

    if "antenv.axon_hooks" in sys.modules:
        return
    hook = None
    try:
        lib = ctypes.CDLL("/opt/axon/libaxon_pjrt.so")
        if hasattr(lib, "axon_start_nrt_profile"):
            lib.axon_start_nrt_profile.argtypes = [
                ctypes.POINTER(ctypes.c_int64),
                ctypes.c_size_t,
            ]
            lib.axon_start_nrt_profile.restype = ctypes.c_int64
            lib.axon_stop_nrt_profile.argtypes = [ctypes.c_char_p]
            lib.axon_stop_nrt_profile.restype = ctypes.c_int64

            @contextlib.contextmanager
            def _hook(output_dir, device_ids):
                import jax

                jax.devices()
                if device_ids:
                    ids = (ctypes.c_int64 * len(device_ids))(*device_ids)
                    rc = lib.axon_start_nrt_profile(ids, len(device_ids))
                else:
                    rc = lib.axon_start_nrt_profile(None, 0)
                if rc != 0:
                    raise RuntimeError(f"axon_start_nrt_profile rc={rc}")
                try:
                    yield
                finally:
                    n = lib.axon_stop_nrt_profile(str(output_dir).encode())
                    print(f"ntff profile: {n} file(s) -> {output_dir}",
                          file=sys.stderr)

            hook = _hook
    except OSError:
        pass
    mod = types.ModuleType("antenv.axon_hooks")
    mod.get_axon_ntff_profile_hook = lambda: hook
    mod.set_axon_ntff_profile_hook = lambda h: None
    sys.modules["antenv.axon_hooks"] = mod
    import antenv

    antenv.axon_hooks = mod


_install_ntff_hook_shim()

# ------------------------------------------------------------- constants ---
NC_N = 8
H = 1024
V = 50000
E = 602
S = 400
P = 128
VR = V // NC_N        # 6250 real vocab rows per core
MT = 49               # vocab m-tiles per core
VP = MT * P           # 6272 padded vocab rows per core
SP = 512              # padded S
ET_M = 4              # s-tiles (512/128)
EP_COLS = 640         # padded E (5*128)
APT = 5               # atten m-tiles

# packed-input free-dim offsets (bf16 elements per partition)
# pack1: stage-1 critical weights (GRU); pack2: the rest
OFF_WIH = 0
OFF_WHH = OFF_WIH + 16 * 384
OFF_X = OFF_WHH + 8 * 384
OFF_HV = OFF_X + 16
NB1 = OFF_HV + 16
OFF_ATTN = 0
OFF_ET = OFF_ATTN + 1024
OFF_EP = OFF_ET + 8 * 512
OFF_CH = OFF_EP + 4 * 1024
OFF_CA = OFF_CH + 1024
OFF_PG = OFF_CA + 1024
OFF_WH = OFF_PG + 4 * 640
OFF_WS = OFF_WH + 16
OFF_WX = OFF_WS + 16
OFF_EMB = OFF_WX + 16
NB2 = OFF_EMB + 16
# f32 pack offsets
OFF_OUTB = 0
OFF_B01 = 56
OFF_BIHN = 64
OFF_BHHN = 72
OFF_CB = 80
OFF_HCOL = 88
OFF_CMASK = 96
OFF_SMASK = 104
OFF_CONSTS = 112
OFF_IDENT = 120
NF = 248

F32 = mybir.dt.float32
BF16 = mybir.dt.bfloat16
NPBF = ml_dtypes.bfloat16

LAST_RESULT = None    # BassKernelResults of the most recent run (for test.py)
TRACE = False         # set True (e.g. by test.py) to profile
DEBUG = False         # add per-stage debug outputs

_NC_CACHE = None



def _split_multi_waits(nc):
    """This walrus build accepts a single sync wait per instruction; hoist
    extra waits onto same-engine nops placed just before the instruction."""
    for f in nc.m.functions:
        for bb in f.blocks:
            out = []
            for ins in bb.instructions:
                si = ins.sync_info
                waits = list(si.on_wait) if si and si.on_wait else []
                if len(waits) > 1:
                    for w in waits[:-1]:
                        nop = mybir.InstNoOp(
                            name=nc.get_next_instruction_name(),
                            engine=ins.engine,
                            ins=[],
                            outs=[],
                            sync_info=mybir.SyncInfo(on_wait=[w], on_update=[]),
                        )
                        out.append(nop)
                    ins.sync_info = mybir.SyncInfo(
                        on_wait=[waits[-1]], on_update=si.on_update
                    )
                out.append(ins)
            bb.instructions = out


# ------------------------------------------------------------ device code ---
def _build_nc():
    nc = bass.Bass()

    def di(name, shape, dt=BF16):
        return nc.dram_tensor(name, shape, dt, kind="ExternalInput")

    # per-core inputs: one packed bf16 tensor, one packed f32 tensor, big W
    wo_t = di("wo_t", [H, VP])
    pack1 = di("pack1", [P, NB1])
    pack2 = di("pack2", [P, NB2])
    pack_f32 = di("pack_f32", [P, NF], F32)

    vocab_out = nc.dram_tensor("vocab_out", [P, MT], F32, kind="ExternalOutput")
    hnew_out = nc.dram_tensor("hnew_out", [P, 8], F32, kind="ExternalOutput")
    attnw_out = nc.dram_tensor("attnw_out", [P, 4], F32, kind="ExternalOutput")
    atten_out = nc.dram_tensor("atten_out", [P, APT], F32, kind="ExternalOutput")
    if DEBUG:
        dbg = {
            name: nc.dram_tensor(name, shape, F32, kind="ExternalOutput")
            for name, shape in [
                ("dbg_gi", [P, 3]), ("dbg_gh", [P, 3]), ("dbg_hnewl", [P, 1]),
                ("dbg_qp", [P, 8]), ("dbg_scl", [P, 4]), ("dbg_ar1", [P, 16]),
                ("dbg_aa", [P, 8]), ("dbg_ff", [P, 8]), ("dbg_pgen", [1, 1]),
            ]
        }

    with tile.TileContext(nc) as tc:
        with (
            tc.tile_pool(name="wp", bufs=1) as wp,
            tc.tile_pool(name="sp", bufs=1) as spool,
            tc.tile_pool(name="pp", bufs=1, space="PSUM") as pp,
            tc.tile_pool(name="bigp", bufs=1, space="PSUM") as bigp,
            tc.tile_pool(name="dram", bufs=1, space="DRAM") as dp,
        ):
            # ---- SBUF loads: 2 packed DMAs + 8 big-W chunk DMAs ----
            # (each dma_start costs ~1.1us of serial issue time on its queue,
            # so everything small rides in two packed transfers)
            # dummy collective: starts CC init / absorbs launch skew while
            # the local chain runs, so AllReduce #1 runs closer to its floor
            sync0_in = dp.tile([1, 8], F32, name="sync0_in")
            sync0_out = dp.tile([8, 8], F32, name="sync0_out")
            nc.gpsimd.collective_compute(
                "AllGather",
                mybir.AluOpType.bypass,
                replica_groups=[list(range(NC_N))],
                ins=[sync0_in.opt()],
                outs=[sync0_out.opt()],
            )

            pf32 = wp.tile([P, NF], F32, tag="pf32", name="pf32")
            nc.sync.dma_start(out=pf32[:], in_=pack_f32[:])
            pk1 = wp.tile([P, NB1], BF16, tag="pk1", name="pk1")
            d_pk1 = nc.sync.dma_start(out=pk1[:], in_=pack1[:])
            pk2 = wp.tile([P, NB2], BF16, tag="pk2", name="pk2")
            d_pk2 = nc.sync.dma_start(out=pk2[:], in_=pack2[:])
            add_dep_helper(d_pk2.ins, d_pk1.ins, reason="pk1 gets HBM first")
            wo_sb = wp.tile([P, 8, VP], BF16, tag="wo", name="wo")
            for k in range(8):
                d_wo = nc.scalar.dma_start(
                    out=wo_sb[:, k, :], in_=wo_t[P * k : P * (k + 1), :]
                )
                add_dep_helper(d_wo.ins, d_pk1.ins, reason="pk1 gets HBM first")

            def b1(off, n):
                return pk1[:, off : off + n]

            def b2(off, n):
                return pk2[:, off : off + n]

            wih = lambda k, m: b1(OFF_WIH + k * 384 + m * P, P)
            whh = lambda k, m: b1(OFF_WHH + k * 384 + m * P, P)
            x_k = lambda k: b1(OFF_X + k, 1)
            h_k = lambda k: b1(OFF_HV + k, 1)
            attn_m = lambda m: b2(OFF_ATTN + m * P, P)
            et_km = lambda k, m: b2(OFF_ET + k * SP + m * P, P)
            ep_km = lambda k, m: b2(OFF_EP + k * H + m * P, P)
            ch_m = lambda m: b2(OFF_CH + m * P, P)
            ca_m = lambda m: b2(OFF_CA + m * P, P)
            pg_km = lambda k, m: b2(OFF_PG + k * EP_COLS + m * P, P)
            wh_k = lambda k: b2(OFF_WH + k, 1)
            ws_sb = b2(OFF_WS, 1)
            wx_k = lambda k: b2(OFF_WX + k, 1)
            emb_k = lambda k: b2(OFF_EMB + k, 1)

            outb_sb = pf32[:, OFF_OUTB : OFF_OUTB + MT]
            b01_sb = pf32[:, OFF_B01 : OFF_B01 + 2]
            bihn_sb = pf32[:, OFF_BIHN : OFF_BIHN + 1]
            bhhn_sb = pf32[:, OFF_BHHN : OFF_BHHN + 1]
            cb_sb = pf32[:, OFF_CB : OFF_CB + 8]
            hcol_sb = pf32[:, OFF_HCOL : OFF_HCOL + 1]
            cmask_sb = pf32[:, OFF_CMASK : OFF_CMASK + 8]
            smask_sb = pf32[:, OFF_SMASK : OFF_SMASK + 4]
            consts_sb = pf32[0:1, OFF_CONSTS : OFF_CONSTS + 8]
            id_sb = pf32[:, OFF_IDENT : OFF_IDENT + P]

            onesc = spool.tile([P, 1], F32, tag="onesc")
            nc.vector.memset(onesc[:], 1.0)
            onesr = spool.tile([1, P], F32, tag="onesr")
            nc.vector.memset(onesr[:], 1.0)

            def psum(shape, tag="tiny", bufs=2, name="ps"):
                return pp.tile(list(shape), F32, tag=tag, bufs=bufs, name=name)

            def part_sum(vec_sb, k=P):
                """sum over partitions of [k,1] f32 -> [1,1] psum"""
                out = psum([1, 1])
                nc.tensor.matmul(out[:], onesc[:k, :], vec_sb, start=True, stop=True)
                return out

            def bcast(scalar_sb):
                """[1,1] sbuf f32 -> [128,1] sbuf f32"""
                pb = psum([P, 1])
                nc.tensor.matmul(pb[:], onesr[:], scalar_sb, start=True, stop=True)
                sb = spool.tile([P, 1], F32, tag="bc", name="bc")
                nc.vector.tensor_copy(sb[:], pb[:])
                return sb

            def to_sb(ps, shape, dt=F32, tag="cp"):
                sb = spool.tile(list(shape), dt, tag=tag, name=tag)
                nc.vector.tensor_copy(sb[:], ps)
                return sb

            # ---- stage 1: GRU slice (no comm) ----
            gi = psum([P, 3], tag="gates", name="gi")
            for k in range(16):
                for m in range(3):
                    nc.tensor.matmul(
                        gi[:, m : m + 1],
                        wih(k, m),
                        x_k(k),
                        start=(k == 0 and m == 0),
                        stop=(k == 15 and m == 2),
                    )
            gh = psum([P, 3], tag="gates", name="gh")
            for k in range(8):
                for m in range(3):
                    nc.tensor.matmul(
                        gh[:, m : m + 1],
                        whh(k, m),
                        h_k(k),
                        start=(k == 0 and m == 0),
                        stop=(k == 7 and m == 2),
                    )
            gi_sb = to_sb(gi[:], [P, 3], F32, tag="gisb")
            t01 = spool.tile([P, 2], F32, tag="t01")
            nc.vector.tensor_add(t01[:], gi_sb[:, 0:2], gh[:, 0:2])
            nc.vector.tensor_add(t01[:], t01[:], b01_sb)
            rz = spool.tile([P, 2], F32, tag="rz")
            nc.scalar.activation(rz[:], t01[:], mybir.ActivationFunctionType.Sigmoid)

            ghn = spool.tile([P, 1], F32, tag="ghn")
            nc.vector.tensor_add(ghn[:], gh[:, 2:3], bhhn_sb)
            tmp1 = spool.tile([P, 1], F32, tag="tmp1")
            nc.vector.tensor_mul(tmp1[:], rz[:, 0:1], ghn[:])
            npre = spool.tile([P, 1], F32, tag="npre")
            nc.vector.tensor_add(npre[:], gi_sb[:, 2:3], bihn_sb)
            nc.vector.tensor_add(npre[:], npre[:], tmp1[:])
            n_sb = spool.tile([P, 1], F32, tag="n")
            nc.scalar.activation(n_sb[:], npre[:], mybir.ActivationFunctionType.Tanh)
            # h_new = n + z*(h - n)
            d_sb = spool.tile([P, 1], F32, tag="d")
            nc.vector.tensor_sub(d_sb[:], hcol_sb, n_sb[:])
            zt = spool.tile([P, 1], F32, tag="zt")
            nc.vector.tensor_mul(zt[:], rz[:, 1:2], d_sb[:])
            hnew = spool.tile([P, 1], F32, tag="hnew")
            nc.vector.tensor_add(hnew[:], n_sb[:], zt[:])
            hnew_bf = to_sb(hnew[:], [P, 1], BF16, tag="hnewbf")
            if DEBUG:
                nc.sync.dma_start(out=dbg["dbg_hnewl"][:], in_=hnew[:])
                nc.sync.dma_start(out=dbg["dbg_gi"][:], in_=gi_sb[:])
                gh_dbg = to_sb(gh[:], [P, 3], F32, tag="ghdbg")
                nc.sync.dma_start(out=dbg["dbg_gh"][:], in_=gh_dbg[:])

            # ---- stage 2: partial attention scores ----
            qp = psum([P, 8], tag="vec8", name="qp")
            for m in range(8):
                nc.tensor.matmul(
                    qp[:, m : m + 1],
                    attn_m(m),
                    hnew_bf[:],
                    start=(m == 0),
                    stop=(m == 7),
                )
            qp_bf = to_sb(qp[:], [P, 8], BF16, tag="qpbf")
            sc = psum([P, 4], tag="vec8", name="sc")
            for k in range(8):
                for m in range(4):
                    nc.tensor.matmul(
                        sc[:, m : m + 1],
                        et_km(k, m),
                        qp_bf[:, k : k + 1],
                        start=(k == 0 and m == 0),
                        stop=(k == 7 and m == 3),
                    )
            if DEBUG:
                qp_dbg = to_sb(qp[:], [P, 8], F32, tag="qpdbg")
                nc.sync.dma_start(out=dbg["dbg_qp"][:], in_=qp_dbg[:])
                sc_dbg = to_sb(sc[:], [P, 4], F32, tag="scdbg")
                nc.sync.dma_start(out=dbg["dbg_scl"][:], in_=sc_dbg[:])
            wsp = psum([1, 1], name="wsp")
            nc.tensor.matmul(wsp[:], ws_sb, hnew_bf[:], start=True, stop=True)

            # ---- AllReduce #1: scores + ws_partial + h_new assembly ----
            ar1 = spool.tile([P, 16], F32, tag="ar1")
            nc.vector.memset(ar1[:], 0.0)
            nc.vector.tensor_copy(ar1[:, 0:4], sc[:])
            nc.vector.tensor_copy(ar1[0:1, 4:5], wsp[:])
            nc.vector.tensor_scalar_mul(ar1[:, 5:13], cmask_sb, hnew[:])
            ar1_in = dp.tile([P, 16], F32)
            ar1_out = dp.tile([P, 16], F32)
            nc.gpsimd.dma_start(out=ar1_in[:], in_=ar1[:])
            nc.gpsimd.collective_compute(
                "AllReduce",
                mybir.AluOpType.add,
                replica_groups=[list(range(NC_N))],
                ins=[ar1_in.opt()],
                outs=[ar1_out.opt()],
            )
            ag1 = spool.tile([P, 16], F32, tag="ag1")
            nc.gpsimd.dma_start(out=ag1[:], in_=ar1_out[:])
            nc.sync.dma_start(out=hnew_out[:], in_=ag1[:, 5:13])
            if DEBUG:
                nc.sync.dma_start(out=dbg["dbg_ar1"][:], in_=ag1[:])

            # ---- stage 3: softmax(scores), attn_applied, p_gen (replicated) ----
            # scores span +-45 for this model scale -> exp safe in fp32
            # without max subtraction; softmax normalization is deferred so
            # the attn-applied matmuls start straight off the exp.
            scores = spool.tile([P, 4], F32, tag="scores")
            nc.vector.tensor_add(scores[:], ag1[:, 0:4], smask_sb)
            expsc = spool.tile([P, 4], F32, tag="expsc")
            rsum = spool.tile([P, 1], F32, tag="rsum")
            nc.scalar.activation(
                expsc[:], scores[:], mybir.ActivationFunctionType.Exp,
                accum_out=rsum[:],
            )
            aw_bf = to_sb(expsc[:], [P, 4], BF16, tag="awbf")  # unnormalized
            stot = to_sb(part_sum(rsum[:])[:], [1, 1], tag="stot")
            rinv = spool.tile([1, 1], F32, tag="rinv")
            nc.vector.reciprocal(rinv[:], stot[:])
            rinv_b = bcast(rinv[:])
            aw = spool.tile([P, 4], F32, tag="aw")
            nc.vector.tensor_scalar_mul(aw[:], expsc[:], rinv_b[:])
            nc.sync.dma_start(out=attnw_out[:], in_=aw[:])
            lnsc = spool.tile([1, 1], F32, tag="lnsc")
            nc.scalar.activation(lnsc[:], stot[:], mybir.ActivationFunctionType.Ln)

            # attn_applied = attn_weights @ E  (full, replicated)
            aa = psum([P, 8], tag="vec8", name="aa")
            for k in range(ET_M):
                for m in range(8):
                    nc.tensor.matmul(
                        aa[:, m : m + 1],
                        ep_km(k, m),
                        aw_bf[:, k : k + 1],
                        start=(k == 0 and m == 0),
                        stop=(k == ET_M - 1 and m == 7),
                    )
            aa_u = to_sb(aa[:], [P, 8], F32, tag="aau")  # unnormalized
            aa_f = spool.tile([P, 8], F32, tag="aaf")
            nc.vector.tensor_scalar_mul(aa_f[:], aa_u[:], rinv_b[:])
            aa_bf = to_sb(aa_f[:], [P, 8], BF16, tag="aabf")
            # select this core's h-slice of attn_applied (normalize the slice)
            t8 = spool.tile([P, 8], F32, tag="t8")
            nc.vector.tensor_mul(t8[:], aa_u[:], cmask_sb)
            aac_u = spool.tile([P, 1], F32, tag="aacu")
            nc.vector.reduce_sum(out=aac_u[:], in_=t8[:], axis=mybir.AxisListType.X)
            aac = spool.tile([P, 1], F32, tag="aac")
            nc.vector.tensor_scalar_mul(aac[:], aac_u[:], rinv_b[:])
            aac_bf = to_sb(aac[:], [P, 1], BF16, tag="aacbf")
            if DEBUG:
                nc.sync.dma_start(out=dbg["dbg_aa"][:], in_=aa_f[:])

            # ---- pre-ff partial + AllReduce #2 ----
            pf = psum([P, 8], tag="vec8", name="pf")
            for m in range(8):
                nc.tensor.matmul(
                    pf[:, m : m + 1],
                    ch_m(m),
                    hnew_bf[:],
                    start=(m == 0),
                    stop=False,
                )
                nc.tensor.matmul(
                    pf[:, m : m + 1],
                    ca_m(m),
                    aac_bf[:],
                    start=False,
                    stop=(m == 7),
                )
            pf_sb = to_sb(pf[:], [P, 8], F32, tag="pfsb")
            ar2_in = dp.tile([P, 8], F32)
            ar2_out = dp.tile([P, 8], F32)
            nc.gpsimd.dma_start(out=ar2_in[:], in_=pf_sb[:])
            nc.gpsimd.collective_compute(
                "AllReduce",
                mybir.AluOpType.add,
                replica_groups=[list(range(NC_N))],
                ins=[ar2_in.opt()],
                outs=[ar2_out.opt()],
            )
            # p_gen
            pgp = psum([1, 1], name="pgp")
            for k in range(8):
                nc.tensor.matmul(
                    pgp[:], wh_k(k), aa_bf[:, k : k + 1],
                    start=(k == 0), stop=False,
                )
            for k in range(8):
                nc.tensor.matmul(
                    pgp[:], wx_k(k), emb_k(k),
                    start=False, stop=(k == 7),
                )
            p1 = spool.tile([1, 1], F32, tag="p1")
            nc.vector.tensor_add(p1[:], pgp[:], ag1[0:1, 4:5])
            pgen = spool.tile([1, 1], F32, tag="pgen")
            nc.scalar.activation(
                pgen[:], p1[:], mybir.ActivationFunctionType.Sigmoid,
                bias=consts_sb[0:1, 1:2],
            )
            ln_pg = spool.tile([1, 1], F32, tag="lnpg")
            nc.scalar.activation(ln_pg[:], pgen[:], mybir.ActivationFunctionType.Ln)
            om = spool.tile([1, 1], F32, tag="om")
            nc.vector.tensor_sub(om[:], consts_sb[0:1, 0:1], pgen[:])
            ln_om = spool.tile([1, 1], F32, tag="lnom")
            nc.scalar.activation(ln_om[:], om[:], mybir.ActivationFunctionType.Ln)

            # atten_p = log(attn_weights @ pg_mat) + log(1-p_gen)
            app = psum([P, APT], tag="vec8", name="app")
            for k in range(ET_M):
                for m in range(APT):
                    nc.tensor.matmul(
                        app[:, m : m + 1],
                        pg_km(k, m),
                        aw_bf[:, k : k + 1],
                        start=(k == 0 and m == 0),
                        stop=(k == ET_M - 1 and m == APT - 1),
                    )
            ln_ap = spool.tile([P, APT], F32, tag="lnap")
            nc.scalar.activation(ln_ap[:], app[:], mybir.ActivationFunctionType.Ln)
            omr = spool.tile([1, 1], F32, tag="omr")
            nc.vector.tensor_sub(omr[:], ln_om[:], lnsc[:])
            lnom_b = bcast(omr[:])
            apf = spool.tile([P, APT], F32, tag="apf")
            nc.vector.tensor_scalar_add(apf[:], ln_ap[:], lnom_b[:])
            nc.sync.dma_start(out=atten_out[:], in_=apf[:])

            pff = spool.tile([P, 8], F32, tag="pff")
            nc.gpsimd.dma_start(out=pff[:], in_=ar2_out[:])
            nc.vector.tensor_add(pff[:], pff[:], cb_sb)
            ff = spool.tile([P, 8], F32, tag="ff")
            nc.scalar.activation(ff[:], pff[:], mybir.ActivationFunctionType.Relu)
            ff_bf = to_sb(ff[:], [P, 8], BF16, tag="ffbf")
            if DEBUG:
                nc.sync.dma_start(out=dbg["dbg_ff"][:], in_=ff[:])
                nc.sync.dma_start(out=dbg["dbg_pgen"][:], in_=pgen[:])

            # ---- big matvec: logits shard [128, 49] ----
            big = bigp.tile([P, MT], F32, tag="big")
            for k in range(8):
                for j in range(MT):
                    nc.tensor.matmul(
                        big[:, j : j + 1],
                        wo_sb[:, k, j * P : (j + 1) * P],
                        ff_bf[:, k : k + 1],
                        start=(k == 0 and j == 0),
                        stop=(k == 7 and j == MT - 1),
                    )
            logits = spool.tile([P, MT], F32, tag="logits")
            nc.vector.tensor_add(logits[:], big[:], outb_sb)
            expv = spool.tile([P, MT], F32, tag="expv")
            esum = spool.tile([P, 1], F32, tag="esum")
            nc.scalar.activation(
                expv[:], logits[:], mybir.ActivationFunctionType.Exp,
                accum_out=esum[:],
            )
            se = to_sb(part_sum(esum[:])[:], [1, 1], tag="se")

            # ---- AllGather #3: per-core sumexp ----
            ag3 = spool.tile([1, 8], F32, tag="ag3")
            nc.vector.memset(ag3[:], 0.0)
            nc.vector.tensor_copy(ag3[0:1, 0:1], se[:])
            ag3_in = dp.tile([1, 8], F32)
            # AG concat is on the partition axis, but DRAM is linear: a
            # [1, 64] view of the same bytes gives the 8 rank rows flat.
            ag3_out = dp.tile([1, 64], F32)
            nc.gpsimd.dma_start(out=ag3_in[:], in_=ag3[:])
            nc.gpsimd.collective_compute(
                "AllGather",
                mybir.AluOpType.bypass,
                replica_groups=[list(range(NC_N))],
                ins=[ag3_in.opt()],
                outs=[ag3_out.opt()],
            )
            agd = spool.tile([1, 64], F32, tag="agd")
            nc.gpsimd.dma_start(out=agd[:], in_=ag3_out[:])
            tot = spool.tile([1, 1], F32, tag="tot")
            nc.vector.reduce_sum(out=tot[:], in_=agd[:], axis=mybir.AxisListType.X)
            lnz = spool.tile([1, 1], F32, tag="lnz")
            nc.scalar.activation(lnz[:], tot[:], mybir.ActivationFunctionType.Ln)
            corr = spool.tile([1, 1], F32, tag="corr")
            nc.vector.tensor_sub(corr[:], lnz[:], ln_pg[:])
            corr_b = bcast(corr[:])
            final = spool.tile([P, MT], F32, tag="final")
            nc.vector.tensor_scalar_sub(final[:], logits[:], corr_b[:])
            nc.sync.dma_start(out=vocab_out[:], in_=final[:])

    _split_multi_waits(nc)
    return nc


# -------------------------------------------------------------- host side ---
def _colmajor(v, ncol):
    return np.ascontiguousarray(v.reshape(ncol, P).T)


def _prep_inputs(inputs):
    f32 = np.float32
    idx = int(np.asarray(inputs["input_idx"]).ravel()[0])
    emb = np.asarray(inputs["emb"], f32)
    embedded = emb[idx]
    trigger = np.asarray(inputs["trigger"], f32)
    x = np.concatenate([embedded, trigger])
    h = np.asarray(inputs["hidden"], f32)[0, 0]
    enc = np.asarray(inputs["encoder_outputs"], f32)
    pg_mat = np.asarray(inputs["pg_mat"], f32)
    attn_W = np.asarray(inputs["attn_W"], f32)
    comb_W = np.asarray(inputs["comb_W"], f32)
    comb_b = np.asarray(inputs["comb_b"], f32)
    W_ih = np.asarray(inputs["W_ih"], f32)
    W_hh = np.asarray(inputs["W_hh"], f32)
    b_ih = np.asarray(inputs["b_ih"], f32)
    b_hh = np.asarray(inputs["b_hh"], f32)
    out_W = np.asarray(inputs["out_W"], f32)
    out_b = np.asarray(inputs["out_b"], f32)
    wh_W = np.asarray(inputs["wh_W"], f32)[0]
    ws_W = np.asarray(inputs["ws_W"], f32)[0]
    wx_W = np.asarray(inputs["wx_W"], f32)[0]
    wx_b = np.asarray(inputs["wx_b"], f32)[0]

    et = np.zeros((H, SP), f32)
    et[:, :S] = enc.T
    ep = np.zeros((SP, H), f32)
    ep[:S] = enc
    pgp = np.zeros((SP, EP_COLS), f32)
    pgp[:S, :E] = pg_mat
    pgp[:S, E:] = 1.0
    sm_flat = np.zeros(SP, f32)
    sm_flat[S:] = -1e30

    def chunked(a, k, m):
        """[k*128, m] row-major -> [128, k*m]  ("(k p) m -> p (k m)")"""
        return a.reshape(k, P, m).transpose(1, 0, 2).reshape(P, k * m)

    def pad16(col):
        out = np.zeros((P, 16), f32)
        out[:, : col.shape[1]] = col
        return out

    # shared bf16 pack pieces (order must match OFF_* in the device code)
    et_pk = chunked(et, 8, SP)
    ep_pk = chunked(ep, ET_M, H)
    pg_pk = chunked(pgp, ET_M, EP_COLS)
    wh_pk = pad16(_colmajor(wh_W, 8))
    wx_pk = pad16(_colmajor(wx_W, 8))
    x_pk = pad16(_colmajor(x, 16))
    h_pk = pad16(_colmajor(h, 8))
    emb_pk = pad16(_colmajor(embedded, 8))

    ident = np.eye(P, dtype=f32)
    consts_col = np.zeros((P, 8), f32)
    consts_col[0, 0] = 1.0
    consts_col[0, 1] = wx_b
    smask_col = _colmajor(sm_flat, 4)
    cb_col = _colmajor(comb_b, 8)

    in_maps = []
    for c in range(NC_N):
        s = slice(P * c, P * (c + 1))
        rows = np.r_[P * c : P * (c + 1), H + P * c : H + P * (c + 1),
                     2 * H + P * c : 2 * H + P * (c + 1)]
        wsh = np.zeros((VP, H), f32)
        wsh[:VR] = out_W[VR * c : VR * (c + 1)]
        ob = np.full(VP, -40.0, f32)
        ob[:VR] = out_b[VR * c : VR * (c + 1)]
        cmask = np.zeros((P, 8), f32)
        cmask[:, c] = 1.0

        p1 = np.zeros((P, NB1), f32)
        p1[:, OFF_WIH : OFF_WIH + 16 * 384] = chunked(
            np.ascontiguousarray(W_ih[rows].T), 16, 384)
        p1[:, OFF_WHH : OFF_WHH + 8 * 384] = chunked(
            np.ascontiguousarray(W_hh[rows].T), 8, 384)
        p1[:, OFF_X : OFF_X + 16] = x_pk
        p1[:, OFF_HV : OFF_HV + 16] = h_pk

        p2 = np.zeros((P, NB2), f32)
        p2[:, OFF_ATTN : OFF_ATTN + H] = attn_W[:, s].T
        p2[:, OFF_ET : OFF_ET + 8 * SP] = et_pk
        p2[:, OFF_EP : OFF_EP + ET_M * H] = ep_pk
        p2[:, OFF_CH : OFF_CH + H] = comb_W[:, s].T
        p2[:, OFF_CA : OFF_CA + H] = comb_W[:, H + P * c : H + P * (c + 1)].T
        p2[:, OFF_PG : OFF_PG + ET_M * EP_COLS] = pg_pk
        p2[:, OFF_WH : OFF_WH + 16] = wh_pk
        p2[:, OFF_WS] = ws_W[s]
        p2[:, OFF_WX : OFF_WX + 16] = wx_pk
        p2[:, OFF_EMB : OFF_EMB + 16] = emb_pk

        pack_f32 = np.zeros((P, NF), f32)
        pack_f32[:, OFF_OUTB : OFF_OUTB + MT] = _colmajor(ob, MT)
        pack_f32[:, OFF_B01] = b_ih[s] + b_hh[s]
        pack_f32[:, OFF_B01 + 1] = (b_ih[H + P * c : H + P * (c + 1)]
                                    + b_hh[H + P * c : H + P * (c + 1)])
        pack_f32[:, OFF_BIHN] = b_ih[2 * H + P * c : 2 * H + P * (c + 1)]
        pack_f32[:, OFF_BHHN] = b_hh[2 * H + P * c : 2 * H + P * (c + 1)]
        pack_f32[:, OFF_CB : OFF_CB + 8] = cb_col
        pack_f32[:, OFF_HCOL] = h[s]
        pack_f32[:, OFF_CMASK : OFF_CMASK + 8] = cmask
        pack_f32[:, OFF_SMASK : OFF_SMASK + 4] = smask_col
        pack_f32[:, OFF_CONSTS : OFF_CONSTS + 8] = consts_col
        pack_f32[:, OFF_IDENT : OFF_IDENT + P] = ident

        m = {
            "wo_t": np.ascontiguousarray(wsh.T).astype(NPBF),
            "pack1": p1.astype(NPBF),
            "pack2": p2.astype(NPBF),
            "pack_f32": pack_f32,
        }
        in_maps.append(m)
    return in_maps


def kernel(**inputs):
    global _NC_CACHE, LAST_RESULT
    in_maps = _prep_inputs(inputs)
    if _NC_CACHE is None:
        _NC_CACHE = _build_nc()
    res = run_bass_kernel_spmd(
        _NC_CACHE, in_maps, list(range(NC_N)), trace=TRACE
    )
    LAST_RESULT = res

    vocab = np.concatenate(
        [res.results[c]["vocab_out"].T.reshape(-1)[:VR] for c in range(NC_N)]
    )
    atten = res.results[0]["atten_out"].T.reshape(-1)[:E]
    output = np.concatenate([vocab, atten])[None, :].astype(np.float32)
    h_new = res.results[0]["hnew_out"].T.reshape(-1)[None, None, :].astype(np.float32)
    attn_weights = (
        res.results[0]["attnw_out"].T.reshape(-1)[:S][None, :].astype(np.float32)
    )
    return output, h_new, attn_weights


# revision 22
# speedup vs baseline: 1.3838x; 1.1444x over previous
"""AttnDecoderRNN step on 8 Trainium2 NeuronCores (Bass/Tile, SPMD).

Sharding strategy (tensor-parallel over output dims, vocab-sharded big matvec):
  - Embedding lookup is pure data movement: done host-side (one row of emb).
  - GRU gates: W_ih/W_hh row-sharded (each core owns a 128-slice of H for all
    three gates) -> each core computes h_new for its slice. No comm.
  - Attention: attn_W column-sharded against the local h_new slice, fused with
    encoder_outputs @ q so a single AllReduce combines scores [400], the
    ws.h_new dot partial, and re-assembles full h_new (mask trick).
  - pre-activation of the combine FF column-sharded -> AllReduce #2.
  - out projection [V,H] row-sharded 6250 rows/core (padded 6272), weights
    pre-transposed + bf16 on host; PE matvec with v on partitions so the
    softmax reduction is partition-parallel.
  - softmax over V: local sum of exp, AllGather of 8 scalars, log-sum-exp
    correction applied locally. (No max subtraction needed: logits are O(1)
    for this model scale; exp is safely inside fp32 range.)
Outputs: each core writes its vocab shard; core 0's h_new / attn_weights /
atten_p are used. Host gathers + undoes the column-major layout.
"""
import sys

sys.path.insert(0, "/opt/trn_rl_repo")

import numpy as np
import ml_dtypes

import concourse.bass as bass
import concourse.mybir as mybir
import concourse.tile as tile
from concourse.tile_rust import add_dep_helper
from concourse.vector_clock import ScopedClock
from concourse import bass_utils
from concourse.bass_utils import run_bass_kernel_spmd

# ---------------------------------------------------------------- patches ---
# This walrus build rejects >1 sync wait on a TPB_CTRL (Drain) instruction;
# TileContext's tail drain accumulates every outstanding sem wait onto it.
# Split the waits onto single-wait nops emitted just before the drain.


def _patched_drain_and_barrier(self, tick_clock, wait_clock):
    nc = self.nc
    carrier = nc.sync.nop(nofuse=True)
    wait_clock.add_sem_waits(carrier.ins, ScopedClock({None: tick_clock.global_clock}))
    si = carrier.ins.sync_info
    waits = list(si.on_wait) if si and si.on_wait else []
    if len(waits) > 1:
        carrier.ins.sync_info = mybir.SyncInfo(
            on_wait=[waits[0]], on_update=si.on_update
        )
        for w in waits[1:]:
            extra = nc.sync.nop(nofuse=True)
            esi = extra.ins.sync_info
            extra.ins.sync_info = mybir.SyncInfo(
                on_wait=[w], on_update=esi.on_update if esi else []
            )
    nc.sync.drain()
    nc.all_engine_barrier()
    popped = nc._tile_sem_poison_stack.pop()
    assert popped is self._sem_poison
    nc.clear_and_free_semaphores(list(self.sems.allocated().values()))
    nc.all_engine_barrier()


tile.TileContext._drain_and_barrier = _patched_drain_and_barrier

# Artifact upload needs a fish bucket; not available (and not needed) here.
bass_utils.upload_artifacts = lambda tmpdir: tmpdir



# This container's antenv lacks axon_hooks; provide the NTFF profile hook via
# ctypes into libaxon_pjrt.so (same shim trn_agent_boot would install).
def _install_ntff_hook_shim():
    import types
    import contextlib
    import ctypes

    if "antenv.axon_hooks" in sys.modules:
        return
    hook = None
    try:
        lib = ctypes.CDLL("/opt/axon/libaxon_pjrt.so")
        if hasattr(lib, "axon_start_nrt_profile"):
            lib.axon_start_nrt_profile.argtypes = [
                ctypes.POINTER(ctypes.c_int64),
                ctypes.c_size_t,
            ]
            lib.axon_start_nrt_profile.restype = ctypes.c_int64
            lib.axon_stop_nrt_profile.argtypes = [ctypes.c_char_p]
            lib.axon_stop_nrt_profile.restype = ctypes.c_int64

            @contextlib.contextmanager
            def _hook(output_dir, device_ids):
                import jax

                jax.devices()
                if device_ids:
                    ids = (ctypes.c_int64 * len(device_ids))(*device_ids)
                    rc = lib.axon_start_nrt_profile(ids, len(device_ids))
                else:
                    rc = lib.axon_start_nrt_profile(None, 0)
                if rc != 0:
                    raise RuntimeError(f"axon_start_nrt_profile rc={rc}")
                try:
                    yield
                finally:
                    n = lib.axon_stop_nrt_profile(str(output_dir).encode())
                    print(f"ntff profile: {n} file(s) -> {output_dir}",
                          file=sys.stderr)

            hook = _hook
    except OSError:
        pass
    mod = types.ModuleType("antenv.axon_hooks")
    mod.get_axon_ntff_profile_hook = lambda: hook
    mod.set_axon_ntff_profile_hook = lambda h: None
    sys.modules["antenv.axon_hooks"] = mod
    import antenv

    antenv.axon_hooks = mod


_install_ntff_hook_shim()

# ------------------------------------------------------------- constants ---
NC_N = 8
H = 1024
V = 50000
E = 602
S = 400
P = 128
VR = V // NC_N        # 6250 real vocab rows per core
MT = 49               # vocab m-tiles per core
VP = MT * P           # 6272 padded vocab rows per core
SP = 512              # padded S
ET_M = 4              # s-tiles (512/128)
EP_COLS = 640         # padded E (5*128)
APT = 5               # atten m-tiles

# packed-input free-dim offsets (bf16 elements per partition)
# pack1: stage-1 critical weights (GRU); pack2: the rest
OFF_WIH = 0
OFF_WHH = OFF_WIH + 16 * 384
OFF_X = OFF_WHH + 8 * 384
OFF_HV = OFF_X + 16
NB1 = OFF_HV + 16
OFF_ATTN = 0
OFF_ET = OFF_ATTN + 1024
OFF_EP = OFF_ET + 8 * 512
OFF_CH = OFF_EP + 4 * 1024
OFF_CA = OFF_CH + 1024
OFF_PG = OFF_CA + 1024
OFF_WH = OFF_PG + 4 * 640
OFF_WS = OFF_WH + 16
OFF_WX = OFF_WS + 16
OFF_EMB = OFF_WX + 16
NB2 = OFF_EMB + 16
# f32 pack offsets
OFF_OUTB = 0
OFF_B01 = 56
OFF_BIHN = 64
OFF_BHHN = 72
OFF_CB = 80
OFF_HCOL = 88
OFF_CMASK = 96
OFF_SMASK = 104
OFF_CONSTS = 112
OFF_IDENT = 120
NF = 248

F32 = mybir.dt.float32
BF16 = mybir.dt.bfloat16
NPBF = ml_dtypes.bfloat16

LAST_RESULT = None    # BassKernelResults of the most recent run (for test.py)
TRACE = False         # set True (e.g. by test.py) to profile
DEBUG = False         # add per-stage debug outputs

_NC_CACHE = None



def _split_multi_waits(nc):
    """This walrus build accepts a single sync wait per instruction; hoist
    extra waits onto same-engine nops placed just before the instruction."""
    for f in nc.m.functions:
        for bb in f.blocks:
            out = []
            for ins in bb.instructions:
                si = ins.sync_info
                waits = list(si.on_wait) if si and si.on_wait else []
                if len(waits) > 1:
                    for w in waits[:-1]:
                        nop = mybir.InstNoOp(
                            name=nc.get_next_instruction_name(),
                            engine=ins.engine,
                            ins=[],
                            outs=[],
                            sync_info=mybir.SyncInfo(on_wait=[w], on_update=[]),
                        )
                        out.append(nop)
                    ins.sync_info = mybir.SyncInfo(
                        on_wait=[waits[-1]], on_update=si.on_update
                    )
                out.append(ins)
            bb.instructions = out


# ------------------------------------------------------------ device code ---
def _build_nc():
    nc = bass.Bass()

    def di(name, shape, dt=BF16):
        return nc.dram_tensor(name, shape, dt, kind="ExternalInput")

    # per-core inputs: one packed bf16 tensor, one packed f32 tensor, big W
    wo_t = di("wo_t", [H, VP])
    pack1 = di("pack1", [P, NB1])
    pack2 = di("pack2", [P, NB2])
    pack_f32 = di("pack_f32", [P, NF], F32)

    vocab_out = nc.dram_tensor("vocab_out", [P, MT], F32, kind="ExternalOutput")
    hnew_out = nc.dram_tensor("hnew_out", [P, 8], F32, kind="ExternalOutput")
    attnw_out = nc.dram_tensor("attnw_out", [P, 4], F32, kind="ExternalOutput")
    atten_out = nc.dram_tensor("atten_out", [P, APT], F32, kind="ExternalOutput")
    if DEBUG:
        dbg = {
            name: nc.dram_tensor(name, shape, F32, kind="ExternalOutput")
            for name, shape in [
                ("dbg_gi", [P, 3]), ("dbg_gh", [P, 3]), ("dbg_hnewl", [P, 1]),
                ("dbg_qp", [P, 8]), ("dbg_scl", [P, 4]), ("dbg_ar1", [P, 16]),
                ("dbg_aa", [P, 8]), ("dbg_ff", [P, 8]), ("dbg_pgen", [1, 1]),
            ]
        }

    with tile.TileContext(nc) as tc:
        with (
            tc.tile_pool(name="wp", bufs=1) as wp,
            tc.tile_pool(name="sp", bufs=1) as spool,
            tc.tile_pool(name="pp", bufs=1, space="PSUM") as pp,
            tc.tile_pool(name="bigp", bufs=1, space="PSUM") as bigp,
            tc.tile_pool(name="dram", bufs=1, space="DRAM") as dp,
        ):
            # ---- SBUF loads: 2 packed DMAs + 8 big-W chunk DMAs ----
            # (each dma_start costs ~1.1us of serial issue time on its queue,
            # so everything small rides in two packed transfers)
            pf32 = wp.tile([P, NF], F32, tag="pf32", name="pf32")
            nc.sync.dma_start(out=pf32[:], in_=pack_f32[:])
            pk1 = wp.tile([P, NB1], BF16, tag="pk1", name="pk1")
            d_pk1 = nc.sync.dma_start(out=pk1[:], in_=pack1[:])
            pk2 = wp.tile([P, NB2], BF16, tag="pk2", name="pk2")
            d_pk2 = nc.sync.dma_start(out=pk2[:], in_=pack2[:])
            add_dep_helper(d_pk2.ins, d_pk1.ins, reason="pk1 gets HBM first")
            wo_sb = wp.tile([P, 8, VP], BF16, tag="wo", name="wo")
            for k in range(8):
                d_wo = nc.scalar.dma_start(
                    out=wo_sb[:, k, :], in_=wo_t[P * k : P * (k + 1), :]
                )
                add_dep_helper(d_wo.ins, d_pk2.ins, reason="packs get HBM first")

            def b1(off, n):
                return pk1[:, off : off + n]

            def b2(off, n):
                return pk2[:, off : off + n]

            wih = lambda k, m: b1(OFF_WIH + k * 384 + m * P, P)
            whh = lambda k, m: b1(OFF_WHH + k * 384 + m * P, P)
            x_k = lambda k: b1(OFF_X + k, 1)
            h_k = lambda k: b1(OFF_HV + k, 1)
            attn_m = lambda m: b2(OFF_ATTN + m * P, P)
            et_km = lambda k, m: b2(OFF_ET + k * SP + m * P, P)
            ep_km = lambda k, m: b2(OFF_EP + k * H + m * P, P)
            ch_m = lambda m: b2(OFF_CH + m * P, P)
            ca_m = lambda m: b2(OFF_CA + m * P, P)
            pg_km = lambda k, m: b2(OFF_PG + k * EP_COLS + m * P, P)
            wh_k = lambda k: b2(OFF_WH + k, 1)
            ws_sb = b2(OFF_WS, 1)
            wx_k = lambda k: b2(OFF_WX + k, 1)
            emb_k = lambda k: b2(OFF_EMB + k, 1)

            outb_sb = pf32[:, OFF_OUTB : OFF_OUTB + MT]
            b01_sb = pf32[:, OFF_B01 : OFF_B01 + 2]
            bihn_sb = pf32[:, OFF_BIHN : OFF_BIHN + 1]
            bhhn_sb = pf32[:, OFF_BHHN : OFF_BHHN + 1]
            cb_sb = pf32[:, OFF_CB : OFF_CB + 8]
            hcol_sb = pf32[:, OFF_HCOL : OFF_HCOL + 1]
            cmask_sb = pf32[:, OFF_CMASK : OFF_CMASK + 8]
            smask_sb = pf32[:, OFF_SMASK : OFF_SMASK + 4]
            consts_sb = pf32[0:1, OFF_CONSTS : OFF_CONSTS + 8]
            id_sb = pf32[:, OFF_IDENT : OFF_IDENT + P]

            onesc = spool.tile([P, 1], F32, tag="onesc")
            nc.vector.memset(onesc[:], 1.0)
            onesr = spool.tile([1, P], F32, tag="onesr")
            nc.vector.memset(onesr[:], 1.0)

            def psum(shape, tag="tiny", bufs=2, name="ps"):
                return pp.tile(list(shape), F32, tag=tag, bufs=bufs, name=name)

            def part_sum(vec_sb, k=P):
                """sum over partitions of [k,1] f32 -> [1,1] psum"""
                out = psum([1, 1])
                nc.tensor.matmul(out[:], onesc[:k, :], vec_sb, start=True, stop=True)
                return out

            def bcast(scalar_sb):
                """[1,1] sbuf f32 -> [128,1] sbuf f32"""
                pb = psum([P, 1])
                nc.tensor.matmul(pb[:], onesr[:], scalar_sb, start=True, stop=True)
                sb = spool.tile([P, 1], F32, tag="bc", name="bc")
                nc.vector.tensor_copy(sb[:], pb[:])
                return sb

            def to_sb(ps, shape, dt=F32, tag="cp"):
                sb = spool.tile(list(shape), dt, tag=tag, name=tag)
                nc.vector.tensor_copy(sb[:], ps)
                return sb

            # ---- stage 1: GRU slice (no comm) ----
            gi = psum([P, 3], tag="gates", name="gi")
            for k in range(16):
                for m in range(3):
                    nc.tensor.matmul(
                        gi[:, m : m + 1],
                        wih(k, m),
                        x_k(k),
                        start=(k == 0 and m == 0),
                        stop=(k == 15 and m == 2),
                    )
            gh = psum([P, 3], tag="gates", name="gh")
            for k in range(8):
                for m in range(3):
                    nc.tensor.matmul(
                        gh[:, m : m + 1],
                        whh(k, m),
                        h_k(k),
                        start=(k == 0 and m == 0),
                        stop=(k == 7 and m == 2),
                    )
            gi_sb = to_sb(gi[:], [P, 3], F32, tag="gisb")
            t01 = spool.tile([P, 2], F32, tag="t01")
            nc.vector.tensor_add(t01[:], gi_sb[:, 0:2], gh[:, 0:2])
            nc.vector.tensor_add(t01[:], t01[:], b01_sb)
            rz = spool.tile([P, 2], F32, tag="rz")
            nc.scalar.activation(rz[:], t01[:], mybir.ActivationFunctionType.Sigmoid)

            ghn = spool.tile([P, 1], F32, tag="ghn")
            nc.vector.tensor_add(ghn[:], gh[:, 2:3], bhhn_sb)
            tmp1 = spool.tile([P, 1], F32, tag="tmp1")
            nc.vector.tensor_mul(tmp1[:], rz[:, 0:1], ghn[:])
            npre = spool.tile([P, 1], F32, tag="npre")
            nc.vector.tensor_add(npre[:], gi_sb[:, 2:3], bihn_sb)
            nc.vector.tensor_add(npre[:], npre[:], tmp1[:])
            n_sb = spool.tile([P, 1], F32, tag="n")
            nc.scalar.activation(n_sb[:], npre[:], mybir.ActivationFunctionType.Tanh)
            warm = spool.tile([1, 1], F32, tag="warm")
            nc.scalar.activation(warm[:], consts_sb[0:1, 0:1],
                                 mybir.ActivationFunctionType.Exp)
            nc.scalar.activation(warm[:], consts_sb[0:1, 0:1],
                                 mybir.ActivationFunctionType.Ln)
            # h_new = n + z*(h - n)
            d_sb = spool.tile([P, 1], F32, tag="d")
            nc.vector.tensor_sub(d_sb[:], hcol_sb, n_sb[:])
            zt = spool.tile([P, 1], F32, tag="zt")
            nc.vector.tensor_mul(zt[:], rz[:, 1:2], d_sb[:])
            hnew = spool.tile([P, 1], F32, tag="hnew")
            nc.vector.tensor_add(hnew[:], n_sb[:], zt[:])
            hnew_bf = to_sb(hnew[:], [P, 1], BF16, tag="hnewbf")
            if DEBUG:
                nc.sync.dma_start(out=dbg["dbg_hnewl"][:], in_=hnew[:])
                nc.sync.dma_start(out=dbg["dbg_gi"][:], in_=gi_sb[:])
                gh_dbg = to_sb(gh[:], [P, 3], F32, tag="ghdbg")
                nc.sync.dma_start(out=dbg["dbg_gh"][:], in_=gh_dbg[:])

            # ---- stage 2: partial attention scores ----
            qp = psum([P, 8], tag="vec8", name="qp")
            for m in range(8):
                nc.tensor.matmul(
                    qp[:, m : m + 1],
                    attn_m(m),
                    hnew_bf[:],
                    start=(m == 0),
                    stop=(m == 7),
                )
            qp_bf = to_sb(qp[:], [P, 8], BF16, tag="qpbf")
            sc = psum([P, 4], tag="vec8", name="sc")
            for k in range(8):
                for m in range(4):
                    nc.tensor.matmul(
                        sc[:, m : m + 1],
                        et_km(k, m),
                        qp_bf[:, k : k + 1],
                        start=(k == 0 and m == 0),
                        stop=(k == 7 and m == 3),
                    )
            if DEBUG:
                qp_dbg = to_sb(qp[:], [P, 8], F32, tag="qpdbg")
                nc.sync.dma_start(out=dbg["dbg_qp"][:], in_=qp_dbg[:])
                sc_dbg = to_sb(sc[:], [P, 4], F32, tag="scdbg")
                nc.sync.dma_start(out=dbg["dbg_scl"][:], in_=sc_dbg[:])
            wsp = psum([1, 1], name="wsp")
            nc.tensor.matmul(wsp[:], ws_sb, hnew_bf[:], start=True, stop=True)

            # ---- AllReduce #1: scores + ws_partial + h_new assembly ----
            ar1 = spool.tile([P, 16], F32, tag="ar1")
            nc.vector.memset(ar1[:], 0.0)
            nc.vector.tensor_copy(ar1[:, 0:4], sc[:])
            nc.vector.tensor_copy(ar1[0:1, 4:5], wsp[:])
            nc.vector.tensor_scalar_mul(ar1[:, 5:13], cmask_sb, hnew[:])
            ar1_in = dp.tile([P, 16], F32)
            ar1_out = dp.tile([P, 16], F32)
            nc.sync.dma_start(out=ar1_in[:], in_=ar1[:])
            nc.gpsimd.collective_compute(
                "AllReduce",
                mybir.AluOpType.add,
                replica_groups=[list(range(NC_N))],
                ins=[ar1_in.opt()],
                outs=[ar1_out.opt()],
            )
            ag1 = spool.tile([P, 16], F32, tag="ag1")
            nc.sync.dma_start(out=ag1[:], in_=ar1_out[:])
            nc.sync.dma_start(out=hnew_out[:], in_=ag1[:, 5:13])
            if DEBUG:
                nc.sync.dma_start(out=dbg["dbg_ar1"][:], in_=ag1[:])

            # ---- stage 3: softmax(scores), attn_applied, p_gen (replicated) ----
            # scores span +-45 for this model scale -> exp safe in fp32
            # without max subtraction; softmax normalization is deferred so
            # the attn-applied matmuls start straight off the exp.
            scores = spool.tile([P, 4], F32, tag="scores")
            nc.vector.tensor_add(scores[:], ag1[:, 0:4], smask_sb)
            expsc = spool.tile([P, 4], F32, tag="expsc")
            rsum = spool.tile([P, 1], F32, tag="rsum")
            nc.scalar.activation(
                expsc[:], scores[:], mybir.ActivationFunctionType.Exp,
                accum_out=rsum[:],
            )
            aw_bf = to_sb(expsc[:], [P, 4], BF16, tag="awbf")  # unnormalized
            stot = to_sb(part_sum(rsum[:])[:], [1, 1], tag="stot")
            rinv = spool.tile([1, 1], F32, tag="rinv")
            nc.vector.reciprocal(rinv[:], stot[:])
            rinv_b = bcast(rinv[:])
            aw = spool.tile([P, 4], F32, tag="aw")
            nc.vector.tensor_scalar_mul(aw[:], expsc[:], rinv_b[:])
            nc.sync.dma_start(out=attnw_out[:], in_=aw[:])
            awn_bf = to_sb(aw[:], [P, 4], BF16, tag="awnbf")

            # attn_applied = attn_weights @ E  (full, replicated)
            aa = psum([P, 8], tag="vec8", name="aa")
            for k in range(ET_M):
                for m in range(8):
                    nc.tensor.matmul(
                        aa[:, m : m + 1],
                        ep_km(k, m),
                        aw_bf[:, k : k + 1],
                        start=(k == 0 and m == 0),
                        stop=(k == ET_M - 1 and m == 7),
                    )
            # select this core's h-slice of attn_applied straight from psum,
            # normalizing and casting in the last step
            t8 = spool.tile([P, 8], F32, tag="t8")
            nc.vector.tensor_mul(t8[:], aa[:], cmask_sb)
            aac_u = spool.tile([P, 1], F32, tag="aacu")
            nc.vector.reduce_sum(out=aac_u[:], in_=t8[:], axis=mybir.AxisListType.X)
            aac_bf = spool.tile([P, 1], BF16, tag="aacbf")
            nc.vector.tensor_scalar_mul(aac_bf[:], aac_u[:], rinv_b[:])
            if DEBUG:
                nc.sync.dma_start(out=dbg["dbg_aa"][:], in_=aa_f[:])

            # ---- pre-ff partial + AllReduce #2 ----
            pf = psum([P, 8], tag="vec8", name="pf")
            for m in range(8):
                nc.tensor.matmul(
                    pf[:, m : m + 1],
                    ch_m(m),
                    hnew_bf[:],
                    start=(m == 0),
                    stop=False,
                )
                nc.tensor.matmul(
                    pf[:, m : m + 1],
                    ca_m(m),
                    aac_bf[:],
                    start=False,
                    stop=(m == 7),
                )
            pf_sb = to_sb(pf[:], [P, 8], F32, tag="pfsb")
            ar2_in = dp.tile([P, 8], F32)
            ar2_out = dp.tile([P, 8], F32)
            nc.sync.dma_start(out=ar2_in[:], in_=pf_sb[:])
            nc.gpsimd.collective_compute(
                "AllReduce",
                mybir.AluOpType.add,
                replica_groups=[list(range(NC_N))],
                ins=[ar2_in.opt()],
                outs=[ar2_out.opt()],
            )
            # p_gen (runs under AR2's latency)
            aa_f = spool.tile([P, 8], F32, tag="aaf")
            nc.vector.tensor_scalar_mul(aa_f[:], aa[:], rinv_b[:])
            aa_bf = to_sb(aa_f[:], [P, 8], BF16, tag="aabf")
            pgp = psum([1, 1], name="pgp")
            for k in range(8):
                nc.tensor.matmul(
                    pgp[:], wh_k(k), aa_bf[:, k : k + 1],
                    start=(k == 0), stop=False,
                )
            for k in range(8):
                nc.tensor.matmul(
                    pgp[:], wx_k(k), emb_k(k),
                    start=False, stop=(k == 7),
                )
            p1 = spool.tile([1, 1], F32, tag="p1")
            nc.vector.tensor_add(p1[:], pgp[:], ag1[0:1, 4:5])
            pgen = spool.tile([1, 1], F32, tag="pgen")
            nc.scalar.activation(
                pgen[:], p1[:], mybir.ActivationFunctionType.Sigmoid,
                bias=consts_sb[0:1, 1:2],
            )
            ln_pg = spool.tile([1, 1], F32, tag="lnpg")
            nc.scalar.activation(ln_pg[:], pgen[:], mybir.ActivationFunctionType.Ln)
            om = spool.tile([1, 1], F32, tag="om")
            nc.vector.tensor_sub(om[:], consts_sb[0:1, 0:1], pgen[:])
            ln_om = spool.tile([1, 1], F32, tag="lnom")
            nc.scalar.activation(ln_om[:], om[:], mybir.ActivationFunctionType.Ln)

            # atten_p = log(attn_weights @ pg_mat) + log(1-p_gen)
            app = psum([P, APT], tag="vec8", name="app")
            for k in range(ET_M):
                for m in range(APT):
                    nc.tensor.matmul(
                        app[:, m : m + 1],
                        pg_km(k, m),
                        awn_bf[:, k : k + 1],
                        start=(k == 0 and m == 0),
                        stop=(k == ET_M - 1 and m == APT - 1),
                    )
            ln_ap = spool.tile([P, APT], F32, tag="lnap")
            nc.scalar.activation(ln_ap[:], app[:], mybir.ActivationFunctionType.Ln)
            lnom_b = bcast(ln_om[:])
            apf = spool.tile([P, APT], F32, tag="apf")
            nc.vector.tensor_scalar_add(apf[:], ln_ap[:], lnom_b[:])
            nc.sync.dma_start(out=atten_out[:], in_=apf[:])

            pff = spool.tile([P, 8], F32, tag="pff")
            nc.sync.dma_start(out=pff[:], in_=ar2_out[:])
            nc.vector.tensor_add(pff[:], pff[:], cb_sb)
            ff = spool.tile([P, 8], F32, tag="ff")
            nc.scalar.activation(ff[:], pff[:], mybir.ActivationFunctionType.Relu)
            ff_bf = to_sb(ff[:], [P, 8], BF16, tag="ffbf")
            if DEBUG:
                nc.sync.dma_start(out=dbg["dbg_ff"][:], in_=ff[:])
                nc.sync.dma_start(out=dbg["dbg_pgen"][:], in_=pgen[:])

            # ---- big matvec: logits shard [128, 49] ----
            big = bigp.tile([P, MT], F32, tag="big")
            for k in range(8):
                for j in range(MT):
                    nc.tensor.matmul(
                        big[:, j : j + 1],
                        wo_sb[:, k, j * P : (j + 1) * P],
                        ff_bf[:, k : k + 1],
                        start=(k == 0 and j == 0),
                        stop=(k == 7 and j == MT - 1),
                    )
            logits = spool.tile([P, MT], F32, tag="logits")
            nc.vector.tensor_add(logits[:], big[:], outb_sb)
            expv = spool.tile([P, MT], F32, tag="expv")
            esum = spool.tile([P, 1], F32, tag="esum")
            nc.scalar.activation(
                expv[:], logits[:], mybir.ActivationFunctionType.Exp,
                accum_out=esum[:],
            )
            se = to_sb(part_sum(esum[:])[:], [1, 1], tag="se")

            # ---- AllGather #3: per-core sumexp ----
            ag3 = spool.tile([1, 8], F32, tag="ag3")
            nc.vector.memset(ag3[:], 0.0)
            nc.vector.tensor_copy(ag3[0:1, 0:1], se[:])
            ag3_in = dp.tile([1, 8], F32)
            # AG concat is on the partition axis, but DRAM is linear: a
            # [1, 64] view of the same bytes gives the 8 rank rows flat.
            ag3_out = dp.tile([1, 64], F32)
            nc.sync.dma_start(out=ag3_in[:], in_=ag3[:])
            nc.gpsimd.collective_compute(
                "AllGather",
                mybir.AluOpType.bypass,
                replica_groups=[list(range(NC_N))],
                ins=[ag3_in.opt()],
                outs=[ag3_out.opt()],
            )
            agd = spool.tile([1, 64], F32, tag="agd")
            nc.sync.dma_start(out=agd[:], in_=ag3_out[:])
            tot = spool.tile([1, 1], F32, tag="tot")
            nc.vector.reduce_sum(out=tot[:], in_=agd[:], axis=mybir.AxisListType.X)
            lnz = spool.tile([1, 1], F32, tag="lnz")
            nc.scalar.activation(lnz[:], tot[:], mybir.ActivationFunctionType.Ln)
            corr = spool.tile([1, 1], F32, tag="corr")
            nc.vector.tensor_sub(corr[:], lnz[:], ln_pg[:])
            corr_b = bcast(corr[:])
            final = spool.tile([P, MT], F32, tag="final")
            nc.vector.tensor_scalar_sub(final[:], logits[:], corr_b[:])
            nc.sync.dma_start(out=vocab_out[:], in_=final[:])

    _split_multi_waits(nc)
    return nc


# -------------------------------------------------------------- host side ---
def _colmajor(v, ncol):
    return np.ascontiguousarray(v.reshape(ncol, P).T)


def _prep_inputs(inputs):
    f32 = np.float32
    idx = int(np.asarray(inputs["input_idx"]).ravel()[0])
    emb = np.asarray(inputs["emb"], f32)
    embedded = emb[idx]
    trigger = np.asarray(inputs["trigger"], f32)
    x = np.concatenate([embedded, trigger])
    h = np.asarray(inputs["hidden"], f32)[0, 0]
    enc = np.asarray(inputs["encoder_outputs"], f32)
    pg_mat = np.asarray(inputs["pg_mat"], f32)
    attn_W = np.asarray(inputs["attn_W"], f32)
    comb_W = np.asarray(inputs["comb_W"], f32)
    comb_b = np.asarray(inputs["comb_b"], f32)
    W_ih = np.asarray(inputs["W_ih"], f32)
    W_hh = np.asarray(inputs["W_hh"], f32)
    b_ih = np.asarray(inputs["b_ih"], f32)
    b_hh = np.asarray(inputs["b_hh"], f32)
    out_W = np.asarray(inputs["out_W"], f32)
    out_b = np.asarray(inputs["out_b"], f32)
    wh_W = np.asarray(inputs["wh_W"], f32)[0]
    ws_W = np.asarray(inputs["ws_W"], f32)[0]
    wx_W = np.asarray(inputs["wx_W"], f32)[0]
    wx_b = np.asarray(inputs["wx_b"], f32)[0]

    et = np.zeros((H, SP), f32)
    et[:, :S] = enc.T
    ep = np.zeros((SP, H), f32)
    ep[:S] = enc
    pgp = np.zeros((SP, EP_COLS), f32)
    pgp[:S, :E] = pg_mat
    pgp[:S, E:] = 1.0
    sm_flat = np.zeros(SP, f32)
    sm_flat[S:] = -1e30

    def chunked(a, k, m):
        """[k*128, m] row-major -> [128, k*m]  ("(k p) m -> p (k m)")"""
        return a.reshape(k, P, m).transpose(1, 0, 2).reshape(P, k * m)

    def pad16(col):
        out = np.zeros((P, 16), f32)
        out[:, : col.shape[1]] = col
        return out

    # shared bf16 pack pieces (order must match OFF_* in the device code)
    et_pk = chunked(et, 8, SP)
    ep_pk = chunked(ep, ET_M, H)
    pg_pk = chunked(pgp, ET_M, EP_COLS)
    wh_pk = pad16(_colmajor(wh_W, 8))
    wx_pk = pad16(_colmajor(wx_W, 8))
    x_pk = pad16(_colmajor(x, 16))
    h_pk = pad16(_colmajor(h, 8))
    emb_pk = pad16(_colmajor(embedded, 8))

    ident = np.eye(P, dtype=f32)
    consts_col = np.zeros((P, 8), f32)
    consts_col[0, 0] = 1.0
    consts_col[0, 1] = wx_b
    smask_col = _colmajor(sm_flat, 4)
    cb_col = _colmajor(comb_b, 8)

    in_maps = []
    for c in range(NC_N):
        s = slice(P * c, P * (c + 1))
        rows = np.r_[P * c : P * (c + 1), H + P * c : H + P * (c + 1),
                     2 * H + P * c : 2 * H + P * (c + 1)]
        wsh = np.zeros((VP, H), f32)
        wsh[:VR] = out_W[VR * c : VR * (c + 1)]
        ob = np.full(VP, -40.0, f32)
        ob[:VR] = out_b[VR * c : VR * (c + 1)]
        cmask = np.zeros((P, 8), f32)
        cmask[:, c] = 1.0

        p1 = np.zeros((P, NB1), f32)
        p1[:, OFF_WIH : OFF_WIH + 16 * 384] = chunked(
            np.ascontiguousarray(W_ih[rows].T), 16, 384)
        p1[:, OFF_WHH : OFF_WHH + 8 * 384] = chunked(
            np.ascontiguousarray(W_hh[rows].T), 8, 384)
        p1[:, OFF_X : OFF_X + 16] = x_pk
        p1[:, OFF_HV : OFF_HV + 16] = h_pk

        p2 = np.zeros((P, NB2), f32)
        p2[:, OFF_ATTN : OFF_ATTN + H] = attn_W[:, s].T
        p2[:, OFF_ET : OFF_ET + 8 * SP] = et_pk
        p2[:, OFF_EP : OFF_EP + ET_M * H] = ep_pk
        p2[:, OFF_CH : OFF_CH + H] = comb_W[:, s].T
        p2[:, OFF_CA : OFF_CA + H] = comb_W[:, H + P * c : H + P * (c + 1)].T
        p2[:, OFF_PG : OFF_PG + ET_M * EP_COLS] = pg_pk
        p2[:, OFF_WH : OFF_WH + 16] = wh_pk
        p2[:, OFF_WS] = ws_W[s]
        p2[:, OFF_WX : OFF_WX + 16] = wx_pk
        p2[:, OFF_EMB : OFF_EMB + 16] = emb_pk

        pack_f32 = np.zeros((P, NF), f32)
        pack_f32[:, OFF_OUTB : OFF_OUTB + MT] = _colmajor(ob, MT)
        pack_f32[:, OFF_B01] = b_ih[s] + b_hh[s]
        pack_f32[:, OFF_B01 + 1] = (b_ih[H + P * c : H + P * (c + 1)]
                                    + b_hh[H + P * c : H + P * (c + 1)])
        pack_f32[:, OFF_BIHN] = b_ih[2 * H + P * c : 2 * H + P * (c + 1)]
        pack_f32[:, OFF_BHHN] = b_hh[2 * H + P * c : 2 * H + P * (c + 1)]
        pack_f32[:, OFF_CB : OFF_CB + 8] = cb_col
        pack_f32[:, OFF_HCOL] = h[s]
        pack_f32[:, OFF_CMASK : OFF_CMASK + 8] = cmask
        pack_f32[:, OFF_SMASK : OFF_SMASK + 4] = smask_col
        pack_f32[:, OFF_CONSTS : OFF_CONSTS + 8] = consts_col
        pack_f32[:, OFF_IDENT : OFF_IDENT + P] = ident

        m = {
            "wo_t": np.ascontiguousarray(wsh.T).astype(NPBF),
            "pack1": p1.astype(NPBF),
            "pack2": p2.astype(NPBF),
            "pack_f32": pack_f32,
        }
        in_maps.append(m)
    return in_maps


def kernel(**inputs):
    global _NC_CACHE, LAST_RESULT
    in_maps = _prep_inputs(inputs)
    if _NC_CACHE is None:
        _NC_CACHE = _build_nc()
    res = run_bass_kernel_spmd(
        _NC_CACHE, in_maps, list(range(NC_N)), trace=TRACE
    )
    LAST_RESULT = res

    vocab = np.concatenate(
        [res.results[c]["vocab_out"].T.reshape(-1)[:VR] for c in range(NC_N)]
    )
    atten = res.results[0]["atten_out"].T.reshape(-1)[:E]
    output = np.concatenate([vocab, atten])[None, :].astype(np.float32)
    h_new = res.results[0]["hnew_out"].T.reshape(-1)[None, None, :].astype(np.float32)
    attn_weights = (
        res.results[0]["attnw_out"].T.reshape(-1)[:S][None, :].astype(np.float32)
    )
    return output, h_new, attn_weights
